# revision 1
# baseline (speedup 1.0000x reference)
"""ConvLSTM2D (Keras gate order, hard_sigmoid) + inference BatchNorm on 8
Trainium2 NeuronCores.

Sharding: batch (2) x H-slabs (4) -> 8 cores, fully local. The sequential
T=16 recurrence needs neighbor rows of h each step; instead of exchanging
halos we compute a shrinking halo: at step t each core computes rows
[r0-(16-t), r1+(16-t)) so the final 16 own rows are exact. Rows outside the
global image are computed-but-masked-to-zero so one uniform SPMD program
serves all cores (edge behavior is data: zero-padded x + per-core mask).

Layout: channels-on-partitions. zin (128 x 3300 bf16) holds x_t on
partitions 0-63 and h_{t-1} on 64-127, rows width-padded to 66 cols with
zero guard cols; a 3x3 conv tap (dy,dx) is the single col offset dy*66+dx.
One matmul contracts x AND h channels at once (lhsT = [Wx_tap; Wh_tap]),
so z_t = conv(x,Wx)+conv(h,Wh) is 9 taps x 2 gate-halves = 18 accumulating
matmuls per pixel tile into PSUM (full 128x128 PE utilization).
"""
import math
import numpy as np

import concourse.bass as bass
import concourse.mybir as mybir
import concourse.tile as tile
from concourse.bass_utils import run_bass_kernel_spmd

F16 = np.float16
F32 = np.float32

T, F, C, W = 16, 64, 64, 64
NR = 50            # buffer rows: [r0-17, r1+17)
WP = W + 2         # width-padded row (guard col each side)
NCOL = NR * WP     # 3300
OWN_LO, OWN_HI = 17 * WP, 33 * WP   # own 16 rows within the buffer
TAPS = [(dy, dx) for dy in (-1, 0, 1) for dx in (-1, 0, 1)]

TRACE_SIM = False
_PROG = None
_LAST_TC = None

# ---------------------------------------------------------------------------
# Workaround: this walrus build accepts at most ONE sync wait per
# instruction; Tile attaches several. Hoist extras onto same-engine NOPs
# inserted right before the instruction (per-engine order preserved).
_MAX_WAITS = 1


def _split_multi_waits(nc):
    for fn in nc.m.functions:
        for bb in fn.blocks:
            lst = bb.instructions
            out, changed = [], False
            for ins in lst:
                si = ins.sync_info
                if si is not None and len(si.on_wait) > _MAX_WAITS:
                    waits = list(si.on_wait)
                    extra, keep = waits[:-_MAX_WAITS], waits[-_MAX_WAITS:]
                    for j, w in enumerate(extra):
                        nop = mybir.InstNoOp(
                            name=f"{ins.name}.sw{j}", ins=[], outs=[],
                            text_hint="split_wait", bass_nofuse=True)
                        nop.engine = ins.engine
                        nop.sync_info = mybir.SyncInfo(on_wait=[w], on_update=[])
                        out.append(nop)
                    ins.sync_info = mybir.SyncInfo(
                        on_wait=keep, on_update=list(si.on_update))
                    changed = True
                out.append(ins)
            if changed:
                try:
                    bb.instructions = out
                except Exception:
                    lst.clear()
                    lst.extend(out)


def _chunks(c0, c1, maxn=512):
    L = c1 - c0
    n = max(1, math.ceil(L / maxn))
    base, rem = divmod(L, n)
    sizes = [base + (1 if i < rem else 0) for i in range(n)]
    out, p = [], c0
    for s in sizes:
        out.append((p, s))
        p += s
    return out


def _build():
    nc = bass.Bass(target_bir_lowering=False)
    f32, bf16 = mybir.dt.float32, mybir.dt.float16

    xT_d = nc.dram_tensor("xT", [T, C, NCOL], bf16, kind="ExternalInput")
    w_d = nc.dram_tensor("w", [128, 18 * 128], bf16, kind="ExternalInput")
    mask_d = nc.dram_tensor("mask", [F, NCOL], f32, kind="ExternalInput")
    hsbif_d = nc.dram_tensor("hsb_if", [128, 1], f32, kind="ExternalInput")
    bg_d = nc.dram_tensor("bg", [F, 1], f32, kind="ExternalInput")
    hsbo_d = nc.dram_tensor("hsb_o", [F, 1], f32, kind="ExternalInput")
    bns_d = nc.dram_tensor("bns", [F, 1], f32, kind="ExternalInput")
    bnb_d = nc.dram_tensor("bnb", [F, 1], f32, kind="ExternalInput")
    out_d = nc.dram_tensor("out", [T, F, 16 * WP], f32, kind="ExternalOutput")

    Relu = mybir.ActivationFunctionType.Relu
    Tanh = mybir.ActivationFunctionType.Tanh
    Ident = mybir.ActivationFunctionType.Identity

    with tile.TileContext(nc, trace_sim=TRACE_SIM) as tc:
        with (
            tc.tile_pool(name="const", bufs=1) as cpool,
            tc.tile_pool(name="state", bufs=1) as spool,
            tc.tile_pool(name="work", bufs=3) as wpool,
            tc.tile_pool(name="ostage", bufs=2) as opool,
            tc.psum_pool(name="ps", bufs=2) as pspool,
        ):
            w_sb = cpool.tile([128, 18 * 128], bf16)
            mask_sb = cpool.tile([F, NCOL], f32)
            hsbif_sb = cpool.tile([128, 1], f32)
            bg_sb = cpool.tile([F, 1], f32)
            hsbo_sb = cpool.tile([F, 1], f32)
            bns_sb = cpool.tile([F, 1], f32)
            bnb_sb = cpool.tile([F, 1], f32)
            nc.sync.dma_start(w_sb[:], w_d[:])
            nc.sync.dma_start(mask_sb[:], mask_d[:])
            nc.sync.dma_start(hsbif_sb[:], hsbif_d[:])
            nc.sync.dma_start(bg_sb[:], bg_d[:])
            nc.sync.dma_start(hsbo_sb[:], hsbo_d[:])
            nc.sync.dma_start(bns_sb[:], bns_d[:])
            nc.sync.dma_start(bnb_sb[:], bnb_d[:])

            zin = [spool.tile([128, NCOL], bf16, name=f"zin{i}", tag=f"zin{i}")
                   for i in range(2)]
            c_sb = spool.tile([F, NCOL], f32, tag="cstate")
            # Full clear: the t=1 matmul tap window reaches one col left of
            # the x DMA range, and NaN garbage survives the h mask multiply.
            nc.vector.memset(zin[0][:, :], 0.0)
            nc.vector.memset(zin[1][:, :], 0.0)
            nc.gpsimd.memset(c_sb[:], 0.0)

            for t in range(1, T + 1):
                cur = zin[(t - 1) % 2]
                nxt = zin[t % 2]
                # x_t into the current buffer's top half (rows [t, 50-t))
                xc0, xc1 = t * WP, (NR - t) * WP
                nc.sync.dma_start(cur[0:64, xc0:xc1], xT_d[t - 1, :, xc0:xc1])

                stage = opool.tile([F, 16 * WP], f32, tag="ostage")
                for p0, n in _chunks((t + 1) * WP, (NR - 1 - t) * WP):
                    ps_if = pspool.tile([128, n], f32, tag="psif")
                    ps_go = pspool.tile([128, n], f32, tag="psgo")
                    for k, (dy, dx) in enumerate(TAPS):
                        off = p0 + dy * WP + dx
                        nc.tensor.matmul(
                            ps_if[:], w_sb[:, k * 128:(k + 1) * 128],
                            cur[:, off:off + n], start=(k == 0), stop=(k == 8))
                    for k, (dy, dx) in enumerate(TAPS):
                        off = p0 + dy * WP + dx
                        nc.tensor.matmul(
                            ps_go[:], w_sb[:, 1152 + k * 128:1152 + (k + 1) * 128],
                            cur[:, off:off + n], start=(k == 0), stop=(k == 8))

                    sig_i = wpool.tile([F, n], f32, tag="sig_i")
                    sig_f = wpool.tile([F, n], f32, tag="sig_f")
                    tanh_g = wpool.tile([F, n], f32, tag="tanh_g")
                    sig_o = wpool.tile([F, n], f32, tag="sig_o")
                    t1 = wpool.tile([F, n], f32, tag="t1")
                    t2 = wpool.tile([F, n], f32, tag="t2")
                    tanh_c = wpool.tile([F, n], f32, tag="tanh_c")
                    h32 = wpool.tile([F, n], f32, tag="h32")

                    nc.scalar.activation(sig_i[:], ps_if[0:64, :], Relu,
                                         bias=hsbif_sb[0:64, 0:1], scale=0.2)
                    nc.scalar.activation(sig_f[:], ps_if[64:128, :], Relu,
                                         bias=hsbif_sb[64:128, 0:1], scale=0.2)
                    nc.gpsimd.tensor_scalar_min(sig_i[:], sig_i[:], 1.0)
                    nc.gpsimd.tensor_scalar_min(sig_f[:], sig_f[:], 1.0)
                    nc.scalar.activation(tanh_g[:], ps_go[0:64, :], Tanh,
                                         bias=bg_sb[:, 0:1], scale=1.0)
                    nc.scalar.activation(sig_o[:], ps_go[64:128, :], Relu,
                                         bias=hsbo_sb[:, 0:1], scale=0.2)
                    nc.gpsimd.tensor_scalar_min(sig_o[:], sig_o[:], 1.0)
                    nc.vector.tensor_mul(t1[:], sig_i[:], tanh_g[:])
                    nc.vector.tensor_mul(t2[:], sig_f[:], c_sb[:, p0:p0 + n])
                    nc.vector.tensor_add(c_sb[:, p0:p0 + n], t1[:], t2[:])
                    nc.scalar.activation(tanh_c[:], c_sb[:, p0:p0 + n], Tanh)
                    nc.vector.tensor_mul(h32[:], sig_o[:], tanh_c[:])
                    if t < T:
                        nc.vector.tensor_mul(nxt[64:128, p0:p0 + n],
                                             h32[:], mask_sb[:, p0:p0 + n])
                    lo, hi = max(p0, OWN_LO), min(p0 + n, OWN_HI)
                    if lo < hi:
                        nc.scalar.activation(
                            stage[:, lo - OWN_LO:hi - OWN_LO],
                            h32[:, lo - p0:hi - p0], Ident,
                            bias=bnb_sb[:, 0:1], scale=bns_sb[:, 0:1])
                nc.sync.dma_start(out_d[t - 1], stage[:])

        global _LAST_TC
        _LAST_TC = tc
    _split_multi_waits(nc)
    return nc


def _prep_inputs(x, Wx, Wh, b, gamma, beta, moving_mean, moving_var):
    x = np.asarray(x, F32)
    Wx = np.asarray(Wx, F32)
    Wh = np.asarray(Wh, F32)
    b = np.asarray(b, F32)
    wstack = np.zeros((128, 18 * 128), F32)
    for k, (dy, dx) in enumerate(TAPS):
        ky, kx = dy + 1, dx + 1
        wstack[0:64, k * 128:(k + 1) * 128] = Wx[ky, kx, :, 0:128]
        wstack[64:128, k * 128:(k + 1) * 128] = Wh[ky, kx, :, 0:128]
        wstack[0:64, 1152 + k * 128:1152 + (k + 1) * 128] = Wx[ky, kx, :, 128:256]
        wstack[64:128, 1152 + k * 128:1152 + (k + 1) * 128] = Wh[ky, kx, :, 128:256]
    wstack = wstack.astype(F16)

    hsb_if = (0.2 * b[0:128] + 0.5).reshape(128, 1).astype(F32)
    bg = b[128:192].reshape(64, 1).astype(F32)
    hsb_o = (0.2 * b[192:256] + 0.5).reshape(64, 1).astype(F32)
    inv = (np.asarray(gamma, F32) /
           np.sqrt(np.asarray(moving_var, F32) + 1e-3))
    bns = inv.reshape(64, 1).astype(F32)
    bnb = (np.asarray(beta, F32) -
           np.asarray(moving_mean, F32) * inv).reshape(64, 1).astype(F32)

    in_maps = []
    for core in range(8):
        bidx, s = core // 4, core % 4
        r0 = 16 * s
        glo, ghi = max(0, r0 - 17), min(64, r0 + 33)
        i0 = glo - (r0 - 17)
        xpad = np.zeros((T, NR, WP, C), F32)
        xpad[:, i0:i0 + (ghi - glo), 1:65, :] = x[bidx, :, glo:ghi, :, :]
        xT = np.ascontiguousarray(
            xpad.transpose(0, 3, 1, 2).reshape(T, C, NCOL)).astype(F16)
        m = np.zeros((NR, WP), F32)
        for i in range(NR):
            if 0 <= (r0 - 17 + i) < 64:
                m[i, 1:65] = 1.0
        mask = np.broadcast_to(m.reshape(1, NCOL), (64, NCOL)).copy()
        in_maps.append({
            "xT": xT, "w": wstack, "mask": mask, "hsb_if": hsb_if,
            "bg": bg, "hsb_o": hsb_o, "bns": bns, "bnb": bnb,
        })
    return in_maps


def kernel(x, Wx, Wh, b, gamma, beta, moving_mean, moving_var):
    global _PROG
    if _PROG is None:
        _PROG = _build()
    in_maps = _prep_inputs(x, Wx, Wh, b, gamma, beta, moving_mean, moving_var)
    res = run_bass_kernel_spmd(_PROG, in_maps, core_ids=list(range(8)))
    out = np.empty((2, T, 64, W, F), F32)
    for core in range(8):
        bidx, s = core // 4, core % 4
        oc = res.results[core]["out"].reshape(T, F, 16, WP)[:, :, :, 1:65]
        out[bidx, :, 16 * s:16 * s + 16] = oc.transpose(0, 2, 3, 1)
    return out



# revision 21
# speedup vs baseline: 1.3760x; 1.3760x over previous
"""ConvLSTM2D (Keras gate order, hard_sigmoid) + inference BatchNorm on 8
Trainium2 NeuronCores.

Sharding: batch (2) x H-slabs (4) -> 8 cores. The T=16 recurrence is split
into 4 blocks of 4 steps. Within a block each core computes a shrinking halo
(depth 4); at block boundaries (t=4,8,12) cores exchange 4 boundary rows of
BOTH states (h and c) with their slab neighbors via one AllGather per epoch
(replica groups = the two 4-slab groups). This cuts redundant conv work from
1.94x (17-deep shrinking halo) to 1.19x while keeping the program uniform
SPMD: out-of-image edges are data (zero-padded x, h-mask, zero select masks).

Layout: channels-on-partitions. zin (128 x 1716 fp16) holds x_t on
partitions 0-63 and h_{t-1} on 64-127; 26 rows of 66 cols (64 + guard col
each side); a 3x3 conv tap (dy,dx) is the col offset dy*66+dx. One matmul
contracts x AND h channels at once (lhsT = [Wx_tap; Wh_tap]) so
z = conv(x,Wx)+conv(h,Wh) is 9 taps x 2 gate-halves of accumulating matmuls
per chunk (full 128x128 PE). Gate-i/f/o weight columns are pre-scaled by 0.2
so hard_sigmoid is Relu(psum + (0.2b+0.5)) then min(.,1). i|f activations are
fused into single 128-partition ops; pointwise math runs in fp16 on DVE
(2x rate) with c kept in fp32; BN is one DVE tensor_scalar (scale+bias) into
an fp16 output.
"""
import math
import numpy as np

import concourse.bass as bass
import concourse.mybir as mybir
import concourse.tile as tile
from concourse.bass_utils import run_bass_kernel_spmd

F16 = np.float16
F32 = np.float32

T, F, C, W = 16, 64, 64, 64
L = 4              # block length (steps between exchanges)
HALO = 4           # halo depth = L
NR = 16 + 2 * HALO + 2   # 26 buffer rows: [r0-5, r1+5)
WP = W + 2         # 66
NCOL = NR * WP     # 1716
OWN_LO, OWN_HI = 5 * WP, 21 * WP     # own 16 rows: buffer rows [5, 21)
HB = HALO * WP     # 264: one halo bundle (4 rows)
TAPS = [(dy, dx) for dy in (-1, 0, 1) for dx in (-1, 0, 1)]

TRACE_SIM = False
_PROG = None
_LAST_TC = None

# ---------------------------------------------------------------------------
# Workaround: this walrus build accepts at most ONE sync wait per
# instruction; Tile attaches several. Hoist extras onto same-engine NOPs
# inserted right before the instruction (per-engine order preserved).
_MAX_WAITS = 1


def _split_multi_waits(nc):
    for fn in nc.m.functions:
        for bb in fn.blocks:
            lst = bb.instructions
            out, changed = [], False
            for ins in lst:
                si = ins.sync_info
                if si is not None and len(si.on_wait) > _MAX_WAITS:
                    waits = list(si.on_wait)
                    extra, keep = waits[:-_MAX_WAITS], waits[-_MAX_WAITS:]
                    for j, w in enumerate(extra):
                        nop = mybir.InstNoOp(
                            name=f"{ins.name}.sw{j}", ins=[], outs=[],
                            text_hint="split_wait", bass_nofuse=True)
                        nop.engine = ins.engine
                        nop.sync_info = mybir.SyncInfo(on_wait=[w], on_update=[])
                        out.append(nop)
                    ins.sync_info = mybir.SyncInfo(
                        on_wait=keep, on_update=list(si.on_update))
                    changed = True
                out.append(ins)
            if changed:
                try:
                    bb.instructions = out
                except Exception:
                    lst.clear()
                    lst.extend(out)


def _chunks(c0, c1, maxn=512):
    Ln = c1 - c0
    n = max(1, math.ceil(Ln / maxn))
    base, rem = divmod(Ln, n)
    sizes = [base + (1 if i < rem else 0) for i in range(n)]
    out, p = [], c0
    for s in sizes:
        out.append((p, s))
        p += s
    return out


def _build():
    nc = bass.Bass(target_bir_lowering=False)
    f32, f16 = mybir.dt.float32, mybir.dt.float16

    xT_d = nc.dram_tensor("xT", [T, C, NCOL], f16, kind="ExternalInput")
    w_d = nc.dram_tensor("w", [128, 18 * 128], f16, kind="ExternalInput")
    mask_d = nc.dram_tensor("mask", [F, NCOL], f16, kind="ExternalInput")
    bif_d = nc.dram_tensor("b_if", [128, 1], f32, kind="ExternalInput")
    bg_d = nc.dram_tensor("bg", [F, 1], f32, kind="ExternalInput")
    bo_d = nc.dram_tensor("b_o", [F, 1], f32, kind="ExternalInput")
    bns_d = nc.dram_tensor("bns", [F, 1], f32, kind="ExternalInput")
    bnb_d = nc.dram_tensor("bnb", [F, 1], f32, kind="ExternalInput")
    sel_d = nc.dram_tensor("sel", [F, 8], f32, kind="ExternalInput")
    out_d = nc.dram_tensor("out", [T, F, 16 * WP], f16, kind="ExternalOutput")

    Relu = mybir.ActivationFunctionType.Relu
    Tanh = mybir.ActivationFunctionType.Tanh
    MULT = mybir.AluOpType.mult
    ADD = mybir.AluOpType.add

    with tile.TileContext(nc, trace_sim=TRACE_SIM) as tc:
        with (
            tc.tile_pool(name="const", bufs=1) as cpool,
            tc.tile_pool(name="state", bufs=1) as spool,
            tc.tile_pool(name="work", bufs=4) as wpool,
            tc.tile_pool(name="ostage", bufs=2) as opool,
            tc.tile_pool(name="agx", bufs=2) as agpool,
            tc.tile_pool(name="dram", bufs=2, space="DRAM") as dpool,
            tc.psum_pool(name="ps", bufs=4) as pspool,
        ):
            w_sb = cpool.tile([128, 18 * 128], f16)
            mask_sb = cpool.tile([F, NCOL], f16)
            bif_sb = cpool.tile([128, 1], f32)
            bg_sb = cpool.tile([F, 1], f32)
            bo_sb = cpool.tile([F, 1], f32)
            bns_sb = cpool.tile([F, 1], f32)
            bnb_sb = cpool.tile([F, 1], f32)
            sel_sb = cpool.tile([F, 8], f32)
            # weights + x on the SP queue (feed PE first); small consts on
            # the ACT queue so they don't head-of-line-block the x DMAs
            nc.sync.dma_start(w_sb[:], w_d[:])
            nc.scalar.dma_start(mask_sb[:], mask_d[:])
            nc.scalar.dma_start(bif_sb[:], bif_d[:])
            nc.scalar.dma_start(bg_sb[:], bg_d[:])
            nc.scalar.dma_start(bo_sb[:], bo_d[:])
            nc.scalar.dma_start(bns_sb[:], bns_d[:])
            nc.scalar.dma_start(bnb_sb[:], bnb_d[:])
            nc.scalar.dma_start(sel_sb[:], sel_d[:])

            # 4 rotating buffers: step t reads x_t+h_{t-1} from zin[(t-1)%4]
            # and writes h_t into zin[t%4]. 4 (not 2) so a step's x DMA never
            # lands in a buffer whose x a not-yet-emitted chunk still reads.
            zin = [spool.tile([128, NCOL], f16, name=f"zin{i}", tag=f"zin{i}")
                   for i in range(4)]
            c_sb = spool.tile([F, NCOL], f32, tag="cstate")
            # h-halves and x guard rows must start zero (NaN garbage would
            # survive the h mask multiply via guard-col taps); x interior is
            # fully overwritten by the per-step DMA.
            for i in range(4):
                eng = nc.vector if i % 2 == 0 else nc.gpsimd
                eng.memset(zin[i][64:128, :], 0.0)
                eng.memset(zin[i][0:64, 0:WP], 0.0)
                eng.memset(zin[i][0:64, NCOL - WP:NCOL], 0.0)
            nc.gpsimd.memset(c_sb[:], 0.0)

            def emit_chunk(t, stage, p0, n):
                cur = zin[(t - 1) % 4]
                nxt = zin[t % 4]
                ps_if = pspool.tile([128, n], f32, tag="psif")
                ps_go = pspool.tile([128, n], f32, tag="psgo")
                for k, (dy, dx) in enumerate(TAPS):
                    off = p0 + dy * WP + dx
                    nc.tensor.matmul(
                        ps_if[:], w_sb[:, k * 128:(k + 1) * 128],
                        cur[:, off:off + n], start=(k == 0), stop=(k == 8))
                for k, (dy, dx) in enumerate(TAPS):
                    off = p0 + dy * WP + dx
                    nc.tensor.matmul(
                        ps_go[:], w_sb[:, 1152 + k * 128:1152 + (k + 1) * 128],
                        cur[:, off:off + n], start=(k == 0), stop=(k == 8))

                sig_if = wpool.tile([128, n], f16, tag="sig_if")
                f_low = wpool.tile([F, n], f16, tag="f_low")
                tanh_g = wpool.tile([F, n], f16, tag="tanh_g")
                sig_o = wpool.tile([F, n], f16, tag="sig_o")
                t1 = wpool.tile([F, n], f16, tag="t1")
                t2 = wpool.tile([F, n], f32, tag="t2")
                tanh_c = wpool.tile([F, n], f16, tag="tanh_c")
                h16 = wpool.tile([F, n], f16, tag="h16")

                # i|f fused: weights pre-scaled by 0.2 -> Relu(ps + 0.2b+0.5)
                nc.scalar.activation(sig_if[:], ps_if[:], Relu,
                                     bias=bif_sb[:, 0:1])
                # DVE TensorTensor needs equal input base partitions: min the
                # f-half down to partitions 0-63 while clipping it
                nc.vector.tensor_scalar_min(sig_if[0:64, :],
                                            sig_if[0:64, :], 1.0)
                nc.vector.tensor_scalar_min(f_low[:], sig_if[64:128, :], 1.0)
                nc.scalar.activation(tanh_g[:], ps_go[0:64, :], Tanh,
                                     bias=bg_sb[:, 0:1])
                nc.scalar.activation(sig_o[:], ps_go[64:128, :], Relu,
                                     bias=bo_sb[:, 0:1])
                nc.vector.tensor_scalar_min(sig_o[:], sig_o[:], 1.0)
                nc.vector.tensor_mul(t1[:], sig_if[0:64, :], tanh_g[:])
                nc.vector.tensor_mul(t2[:], f_low[:], c_sb[:, p0:p0 + n])
                nc.vector.tensor_add(c_sb[:, p0:p0 + n], t1[:], t2[:])
                nc.scalar.activation(tanh_c[:], c_sb[:, p0:p0 + n], Tanh)
                nc.vector.tensor_mul(h16[:], sig_o[:], tanh_c[:])
                if t < T:
                    nc.vector.tensor_mul(nxt[64:128, p0:p0 + n],
                                         h16[:], mask_sb[:, p0:p0 + n])
                lo, hi = max(p0, OWN_LO), min(p0 + n, OWN_HI)
                if lo < hi:
                    # BN on Pool: off the critical recurrence path, so the
                    # collective blocking Pool only delays the output stage
                    nc.gpsimd.tensor_scalar(
                        stage[:, lo - OWN_LO:hi - OWN_LO],
                        h16[:, lo - p0:hi - p0],
                        bns_sb[:, 0:1], bnb_sb[:, 0:1], MULT, ADD)

            def emit_exchange_send(t):
                # ---- exchange epoch: ship h,c boundary rows (4 each) ----
                nxt = zin[t % 4]
                cbf = agpool.tile([F, 2 * HB], f16, tag="cbf")
                nc.vector.tensor_scalar_mul(
                    cbf[:, 0:HB], c_sb[:, OWN_LO:OWN_LO + HB], 1.0)
                nc.vector.tensor_scalar_mul(
                    cbf[:, HB:2 * HB], c_sb[:, OWN_HI - HB:OWN_HI], 1.0)
                agin = dpool.tile([F, 4 * HB], f16, tag="agin")
                agout = dpool.tile([4 * F, 4 * HB], f16, tag="agout")
                nc.sync.dma_start(agin[:, 0:HB],
                                  nxt[64:128, OWN_LO:OWN_LO + HB])
                nc.sync.dma_start(agin[:, HB:2 * HB],
                                  nxt[64:128, OWN_HI - HB:OWN_HI])
                nc.sync.dma_start(agin[:, 2 * HB:4 * HB], cbf[:])
                nc.gpsimd.collective_compute(
                    "AllGather", mybir.AluOpType.bypass,
                    ins=[agin.opt()], outs=[agout.opt()],
                    replica_groups=[[0, 1, 2, 3], [4, 5, 6, 7]],
                )
                return agout

            def emit_exchange_recv(t, agout):
                nxt = zin[t % 4]
                agsb = agpool.tile([F, 16 * HB], f16, tag="agsb")
                for j in range(4):
                    nc.sync.dma_start(
                        agsb[:, j * 4 * HB:(j + 1) * 4 * HB],
                        agout[j * F:(j + 1) * F, :])
                # select: up-halo <- Sum_j bundle_j * sel_up_j, etc.
                # bundles within agsb[j]: [h_top | h_bot | c_top | c_bot]
                tsel = [agpool.tile([F, HB], f16, name=f"tsel{i}",
                                    tag=f"tsel{i}") for i in range(4)]
                csel = [agpool.tile([F, HB], f32, name=f"csel{i}",
                                    tag=f"csel{i}") for i in range(4)]

                def select(dst_ap, boff, scol, tiles, eng):
                    # dst = sum_j agsb[:, j*4HB+boff : +HB] * sel[:, scol+j]
                    for j in range(4):
                        eng.tensor_scalar_mul(
                            tiles[j][:],
                            agsb[:, j * 4 * HB + boff:j * 4 * HB + boff + HB],
                            sel_sb[:, scol + j:scol + j + 1])
                    eng.tensor_add(tiles[0][:], tiles[0][:], tiles[1][:])
                    eng.tensor_add(tiles[2][:], tiles[2][:], tiles[3][:])
                    eng.tensor_add(dst_ap, tiles[0][:], tiles[2][:])

                # h up-halo rows [1,5): from up-neighbor's h_bot (boff=HB)
                select(nxt[64:128, WP:WP + HB], HB, 0, tsel, nc.vector)
                # h down-halo rows [21,25): from down-neighbor's h_top
                select(nxt[64:128, OWN_HI:OWN_HI + HB], 0, 4, tsel,
                       nc.vector)
                # c halos (f32 state) on gpsimd to offload DVE
                select(c_sb[:, WP:WP + HB], 3 * HB, 0, csel, nc.gpsimd)
                select(c_sb[:, OWN_HI:OWN_HI + HB], 2 * HB, 4, csel,
                       nc.gpsimd)

            # Nested halo-independent interiors: I_j needs only I_{j-1}'s h
            # (67-col tap margin), so all interiors of a block are runnable
            # while the preceding epoch's AllGather is still in flight.
            INT = [(397 + 67 * j, 1319 - 67 * j) for j in range(4)]

            pending = None           # (epoch step, agout) awaiting receive
            for b in range(4):
                stages = {}
                for j in range(4):
                    t = 4 * b + j + 1
                    cur = zin[(t - 1) % 4]
                    # full interior rows every step: keeps every buffer's
                    # x-half fresh (no stale-x reads from 4 steps ago)
                    nc.sync.dma_start(cur[0:64, WP:NCOL - WP],
                                      xT_d[t - 1, :, WP:NCOL - WP])
                    stages[t] = opool.tile([F, 16 * WP], f16, name=f"stage{t}",
                                           tag=f"stage{j}")
                # phase A: interiors (halo-independent), step order — these
                # overlap the in-flight AllGather from the previous block
                for j in range(4):
                    t = 4 * b + j + 1
                    i0, i1 = INT[j]
                    for p0, n in _chunks(i0, i1):
                        emit_chunk(t, stages[t], p0, n)
                # receive the previous epoch's halos only now, so no engine
                # queue stalls on the collective before phase A is dispatched
                if pending is not None:
                    emit_exchange_recv(*pending)
                    pending = None
                # phase B: boundary chunks, step order; epoch step's boundary
                # feeds the exchange
                for j in range(4):
                    t = 4 * b + j + 1
                    s = 3 - j
                    i0, i1 = INT[j]
                    c0, c1 = (5 - s) * WP, (21 + s) * WP
                    emit_chunk(t, stages[t], c0, i0 - c0)
                    emit_chunk(t, stages[t], i1, c1 - i1)
                for j in range(4):
                    t = 4 * b + j + 1
                    nc.sync.dma_start(out_d[t - 1], stages[t][:])
                if b < 3:
                    te = 4 * b + 4
                    pending = (te, emit_exchange_send(te))

        global _LAST_TC
        _LAST_TC = tc
    _split_multi_waits(nc)
    return nc


def _prep_inputs(x, Wx, Wh, b, gamma, beta, moving_mean, moving_var):
    x = np.asarray(x, F32)
    Wx = np.asarray(Wx, F32)
    Wh = np.asarray(Wh, F32)
    b = np.asarray(b, F32)
    # gate order along 4F: [i | f | g | o]; half1 = [i|f], half2 = [g|o].
    # Pre-scale i/f/o columns by 0.2 (hard_sigmoid slope).
    wstack = np.zeros((128, 18 * 128), F32)
    for k, (dy, dx) in enumerate(TAPS):
        ky, kx = dy + 1, dx + 1
        wstack[0:64, k * 128:(k + 1) * 128] = Wx[ky, kx, :, 0:128] * 0.2
        wstack[64:128, k * 128:(k + 1) * 128] = Wh[ky, kx, :, 0:128] * 0.2
        h2 = np.concatenate([Wx[ky, kx, :, 128:192],
                             Wx[ky, kx, :, 192:256] * 0.2], axis=1)
        wstack[0:64, 1152 + k * 128:1152 + (k + 1) * 128] = h2
        h2h = np.concatenate([Wh[ky, kx, :, 128:192],
                              Wh[ky, kx, :, 192:256] * 0.2], axis=1)
        wstack[64:128, 1152 + k * 128:1152 + (k + 1) * 128] = h2h
    wstack = wstack.astype(F16)

    b_if = (0.2 * b[0:128] + 0.5).reshape(128, 1).astype(F32)
    bg = b[128:192].reshape(64, 1).astype(F32)
    b_o = (0.2 * b[192:256] + 0.5).reshape(64, 1).astype(F32)
    inv = (np.asarray(gamma, F32) /
           np.sqrt(np.asarray(moving_var, F32) + 1e-3))
    bns = inv.reshape(64, 1).astype(F32)
    bnb = (np.asarray(beta, F32) -
           np.asarray(moving_mean, F32) * inv).reshape(64, 1).astype(F32)

    in_maps = []
    for core in range(8):
        bidx, sl = core // 4, core % 4
        r0 = 16 * sl
        glo, ghi = max(0, r0 - 5), min(64, r0 + 21)
        i0 = glo - (r0 - 5)
        xpad = np.zeros((T, NR, WP, C), F32)
        xpad[:, i0:i0 + (ghi - glo), 1:65, :] = x[bidx, :, glo:ghi, :, :]
        xT = np.ascontiguousarray(
            xpad.transpose(0, 3, 1, 2).reshape(T, C, NCOL)).astype(F16)
        m = np.zeros((NR, WP), F32)
        for i in range(NR):
            if 0 <= (r0 - 5 + i) < 64:
                m[i, 1:65] = 1.0
        mask = np.broadcast_to(
            m.reshape(1, NCOL), (64, NCOL)).astype(F16).copy()
        # select masks: sel[:, 0:4] = up (choose group-rank sl-1),
        # sel[:, 4:8] = down (choose group-rank sl+1)
        sel = np.zeros((64, 8), F32)
        if sl > 0:
            sel[:, sl - 1] = 1.0
        if sl < 3:
            sel[:, 4 + sl + 1] = 1.0
        in_maps.append({
            "xT": xT, "w": wstack, "mask": mask, "b_if": b_if,
            "bg": bg, "b_o": b_o, "bns": bns, "bnb": bnb, "sel": sel,
        })
    return in_maps


def kernel(x, Wx, Wh, b, gamma, beta, moving_mean, moving_var):
    global _PROG
    if _PROG is None:
        _PROG = _build()
    in_maps = _prep_inputs(x, Wx, Wh, b, gamma, beta, moving_mean, moving_var)
    res = run_bass_kernel_spmd(_PROG, in_maps, core_ids=list(range(8)))
    out = np.empty((2, T, 64, W, F), F32)
    for core in range(8):
        bidx, sl = core // 4, core % 4
        oc = res.results[core]["out"].astype(F32).reshape(
            T, F, 16, WP)[:, :, :, 1:65]
        out[bidx, :, 16 * sl:16 * sl + 16] = oc.transpose(0, 2, 3, 1)
    return out


# revision 34
# speedup vs baseline: 1.4431x; 1.0488x over previous
"""ConvLSTM2D (Keras gate order, hard_sigmoid) + inference BatchNorm on 8
Trainium2 NeuronCores.

Sharding: batch (2) x H-slabs (4) -> 8 cores. The T=16 recurrence is split
into 4 blocks of 4 steps. Within a block each core computes a shrinking halo
(depth 4); at block boundaries (t=4,8,12) cores exchange 4 boundary rows of
BOTH states (h and c) with their slab neighbors via one AllGather per epoch
(replica groups = the two 4-slab groups). This cuts redundant conv work from
1.94x (17-deep shrinking halo) to 1.19x while keeping the program uniform
SPMD: out-of-image edges are data (zero-padded x, h-mask, zero select masks).

Layout: channels-on-partitions. zin (128 x 1716 fp16) holds x_t on
partitions 0-63 and h_{t-1} on 64-127; 26 rows of 66 cols (64 + guard col
each side); a 3x3 conv tap (dy,dx) is the col offset dy*66+dx. One matmul
contracts x AND h channels at once (lhsT = [Wx_tap; Wh_tap]) so
z = conv(x,Wx)+conv(h,Wh) is 9 taps x 2 gate-halves of accumulating matmuls
per chunk (full 128x128 PE). Gate-i/f/o weight columns are pre-scaled by 0.2
so hard_sigmoid is Relu(psum + (0.2b+0.5)) then min(.,1). i|f activations are
fused into single 128-partition ops; pointwise math runs in fp16 on DVE
(2x rate) with c kept in fp32; BN is one DVE tensor_scalar (scale+bias) into
an fp16 output.
"""
import math
import numpy as np

import concourse.bass as bass
import concourse.mybir as mybir
import concourse.tile as tile
from concourse.bass_utils import run_bass_kernel_spmd

F16 = np.float16
F32 = np.float32

T, F, C, W = 16, 64, 64, 64
L = 4              # block length (steps between exchanges)
HALO = 4           # halo depth = L
NR = 16 + 2 * HALO + 2   # 26 buffer rows: [r0-5, r1+5)
WP = W + 2         # 66
NCOL = NR * WP     # 1716
OWN_LO, OWN_HI = 5 * WP, 21 * WP     # own 16 rows: buffer rows [5, 21)
HB = HALO * WP     # 264: one halo bundle (4 rows)
TAPS = [(dy, dx) for dy in (-1, 0, 1) for dx in (-1, 0, 1)]

TRACE_SIM = False
_PROG = None
_LAST_TC = None

# ---------------------------------------------------------------------------
# Workaround: this walrus build accepts at most ONE sync wait per
# instruction; Tile attaches several. Hoist extras onto same-engine NOPs
# inserted right before the instruction (per-engine order preserved).
_MAX_WAITS = 1


def _split_multi_waits(nc):
    for fn in nc.m.functions:
        for bb in fn.blocks:
            lst = bb.instructions
            out, changed = [], False
            for ins in lst:
                si = ins.sync_info
                if si is not None and len(si.on_wait) > _MAX_WAITS:
                    waits = list(si.on_wait)
                    extra, keep = waits[:-_MAX_WAITS], waits[-_MAX_WAITS:]
                    for j, w in enumerate(extra):
                        nop = mybir.InstNoOp(
                            name=f"{ins.name}.sw{j}", ins=[], outs=[],
                            text_hint="split_wait", bass_nofuse=True)
                        nop.engine = ins.engine
                        nop.sync_info = mybir.SyncInfo(on_wait=[w], on_update=[])
                        out.append(nop)
                    ins.sync_info = mybir.SyncInfo(
                        on_wait=keep, on_update=list(si.on_update))
                    changed = True
                out.append(ins)
            if changed:
                try:
                    bb.instructions = out
                except Exception:
                    lst.clear()
                    lst.extend(out)


def _chunks(c0, c1, maxn=512):
    Ln = c1 - c0
    n = max(1, math.ceil(Ln / maxn))
    base, rem = divmod(Ln, n)
    sizes = [base + (1 if i < rem else 0) for i in range(n)]
    out, p = [], c0
    for s in sizes:
        out.append((p, s))
        p += s
    return out


def _build():
    nc = bass.Bass(target_bir_lowering=False)
    f32, f16 = mybir.dt.float32, mybir.dt.float16

    xT_d = nc.dram_tensor("xT", [T, C, NCOL], f16, kind="ExternalInput")
    w_d = nc.dram_tensor("w", [128, 18 * 128], f16, kind="ExternalInput")
    mask_d = nc.dram_tensor("mask", [F, NCOL], f16, kind="ExternalInput")
    bif_d = nc.dram_tensor("b_if", [128, 1], f32, kind="ExternalInput")
    bg_d = nc.dram_tensor("bg", [F, 1], f32, kind="ExternalInput")
    bo_d = nc.dram_tensor("b_o", [F, 1], f32, kind="ExternalInput")
    bns_d = nc.dram_tensor("bns", [128, 1], f32, kind="ExternalInput")
    bnb_d = nc.dram_tensor("bnb", [128, 1], f32, kind="ExternalInput")
    sel_d = nc.dram_tensor("sel", [F, 8], f32, kind="ExternalInput")
    out_d = nc.dram_tensor("out", [T, F, 16 * WP], f16, kind="ExternalOutput")

    Relu = mybir.ActivationFunctionType.Relu
    Tanh = mybir.ActivationFunctionType.Tanh
    MULT = mybir.AluOpType.mult
    ADD = mybir.AluOpType.add

    with tile.TileContext(nc, trace_sim=TRACE_SIM) as tc:
        with (
            tc.tile_pool(name="const", bufs=1) as cpool,
            tc.tile_pool(name="state", bufs=1) as spool,
            tc.tile_pool(name="work", bufs=4) as wpool,
            tc.tile_pool(name="ostage", bufs=2) as opool,
            tc.tile_pool(name="agx", bufs=2) as agpool,
            tc.tile_pool(name="dram", bufs=2, space="DRAM") as dpool,
            tc.psum_pool(name="ps", bufs=4) as pspool,
        ):
            w_sb = cpool.tile([128, 18 * 128], f16)
            mask_sb = cpool.tile([F, NCOL], f16)
            bif_sb = cpool.tile([128, 1], f32)
            bg_sb = cpool.tile([F, 1], f32)
            bo_sb = cpool.tile([F, 1], f32)
            bns_sb = cpool.tile([128, 1], f32)
            bnb_sb = cpool.tile([128, 1], f32)
            sel_sb = cpool.tile([F, 8], f32)
            # x on the SP queue (feed PE first); weights + small consts on
            # the ACT queue so they don't head-of-line-block the x DMAs
            nc.scalar.dma_start(w_sb[:], w_d[:])
            nc.scalar.dma_start(mask_sb[:], mask_d[:])
            nc.scalar.dma_start(bif_sb[:], bif_d[:])
            nc.scalar.dma_start(bg_sb[:], bg_d[:])
            nc.scalar.dma_start(bo_sb[:], bo_d[:])
            nc.scalar.dma_start(bns_sb[:], bns_d[:])
            nc.scalar.dma_start(bnb_sb[:], bnb_d[:])
            nc.scalar.dma_start(sel_sb[:], sel_d[:])

            # 4 rotating buffers: step t reads x_t+h_{t-1} from zin[(t-1)%4]
            # and writes h_t into zin[t%4]. 4 (not 2) so a step's x DMA never
            # lands in a buffer whose x a not-yet-emitted chunk still reads.
            zin = [spool.tile([128, NCOL], f16, name=f"zin{i}", tag=f"zin{i}")
                   for i in range(4)]
            c_sb = spool.tile([F, NCOL], f32, tag="cstate")
            # h-halves and x guard rows must start zero (NaN garbage would
            # survive the h mask multiply via guard-col taps); x interior is
            # fully overwritten by the per-step DMA.
            for i in range(4):
                eng = nc.vector if i % 2 == 0 else nc.gpsimd
                eng.memset(zin[i][64:128, :], 0.0)
                eng.memset(zin[i][0:64, 0:WP], 0.0)
                eng.memset(zin[i][0:64, NCOL - WP:NCOL], 0.0)
            nc.gpsimd.memset(c_sb[:], 0.0)

            def emit_chunk_head(t, p0, n):
                cur = zin[(t - 1) % 4]
                ps_if = pspool.tile([128, n], f32, tag="psif")
                ps_go = pspool.tile([128, n], f32, tag="psgo")
                for k, (dy, dx) in enumerate(TAPS):
                    off = p0 + dy * WP + dx
                    nc.tensor.matmul(
                        ps_if[:], w_sb[:, k * 128:(k + 1) * 128],
                        cur[:, off:off + n], start=(k == 0), stop=(k == 8))
                for k, (dy, dx) in enumerate(TAPS):
                    off = p0 + dy * WP + dx
                    nc.tensor.matmul(
                        ps_go[:], w_sb[:, 1152 + k * 128:1152 + (k + 1) * 128],
                        cur[:, off:off + n], start=(k == 0), stop=(k == 8))

                sig_if = wpool.tile([128, n], f16, tag="sig_if")
                f_low = wpool.tile([F, n], f16, tag="f_low")
                tanh_g = wpool.tile([F, n], f16, tag="tanh_g")
                sig_o = wpool.tile([F, n], f16, tag="sig_o")
                t1 = wpool.tile([F, n], f16, tag="t1")
                t2 = wpool.tile([F, n], f32, tag="t2")

                # i|f fused: weights pre-scaled by 0.2 -> Relu(ps + 0.2b+0.5)
                nc.scalar.activation(sig_if[:], ps_if[:], Relu,
                                     bias=bif_sb[:, 0:1])
                # DVE TensorTensor needs equal input base partitions: min the
                # f-half down to partitions 0-63 while clipping it
                nc.vector.tensor_scalar_min(sig_if[0:64, :],
                                            sig_if[0:64, :], 1.0)
                nc.vector.tensor_scalar_min(f_low[:], sig_if[64:128, :], 1.0)
                nc.scalar.activation(tanh_g[:], ps_go[0:64, :], Tanh,
                                     bias=bg_sb[:, 0:1])
                nc.scalar.activation(sig_o[:], ps_go[64:128, :], Relu,
                                     bias=bo_sb[:, 0:1])
                # min(o,1) and the h edge-mask fused: mask is 1 in-image (so
                # min(o, 1)) and 0 outside (so o -> 0 -> h = 0)
                nc.vector.tensor_tensor(sig_o[:], sig_o[:],
                                        mask_sb[:, p0:p0 + n],
                                        mybir.AluOpType.min)
                nc.vector.tensor_mul(t1[:], sig_if[0:64, :], tanh_g[:])
                nc.vector.tensor_mul(t2[:], f_low[:], c_sb[:, p0:p0 + n])
                nc.vector.tensor_add(c_sb[:, p0:p0 + n], t1[:], t2[:])
                return sig_o

            def emit_chunk_tail(t, stage, p0, n, sig_o):
                # second pass: tanh(c) and h, emitted after every chunk's
                # gate work so a blocked tanh_c can't head-of-line-block the
                # next chunk's ready activations in the ACT FIFO
                nxt = zin[t % 4]
                tanh_c = wpool.tile([F, n], f16, tag="tanh_c")
                nc.scalar.activation(tanh_c[:], c_sb[:, p0:p0 + n], Tanh)
                nc.vector.tensor_mul(nxt[64:128, p0:p0 + n],
                                     sig_o[:], tanh_c[:])
                lo, hi = max(p0, OWN_LO), min(p0 + n, OWN_HI)
                if lo < hi:
                    # BN on Pool: off the critical recurrence path, so the
                    # collective blocking Pool only delays the output stage
                    nc.gpsimd.tensor_scalar(
                        stage[:, lo - OWN_LO:hi - OWN_LO],
                        nxt[64:128, lo:hi],
                        bns_sb[64:128, 0:1], bnb_sb[64:128, 0:1], MULT, ADD)

            def emit_step(t, stage, chunk_list):
                heads = [(p0, n, emit_chunk_head(t, p0, n))
                         for p0, n in chunk_list]
                for p0, n, sig_o in heads:
                    emit_chunk_tail(t, stage, p0, n, sig_o)

            def emit_exchange_send(t):
                # ---- exchange epoch: ship h,c boundary rows (4 each) ----
                nxt = zin[t % 4]
                cbf = agpool.tile([F, 2 * HB], f16, tag="cbf")
                nc.vector.tensor_scalar_mul(
                    cbf[:, 0:HB], c_sb[:, OWN_LO:OWN_LO + HB], 1.0)
                nc.vector.tensor_scalar_mul(
                    cbf[:, HB:2 * HB], c_sb[:, OWN_HI - HB:OWN_HI], 1.0)
                agin = dpool.tile([F, 4 * HB], f16, tag="agin")
                agout = dpool.tile([4 * F, 4 * HB], f16, tag="agout")
                nc.sync.dma_start(agin[:, 0:HB],
                                  nxt[64:128, OWN_LO:OWN_LO + HB])
                nc.sync.dma_start(agin[:, HB:2 * HB],
                                  nxt[64:128, OWN_HI - HB:OWN_HI])
                nc.sync.dma_start(agin[:, 2 * HB:4 * HB], cbf[:])
                nc.gpsimd.collective_compute(
                    "AllGather", mybir.AluOpType.bypass,
                    ins=[agin.opt()], outs=[agout.opt()],
                    replica_groups=[[0, 1, 2, 3], [4, 5, 6, 7]],
                )
                return agout

            def emit_exchange_recv(t, agout):
                nxt = zin[t % 4]
                agsb = agpool.tile([F, 16 * HB], f16, tag="agsb")
                nc.sync.dma_start(
                    agsb[:].rearrange("p (j c) -> p j c", j=4),
                    agout[:].rearrange("(j p) c -> p j c", j=4))
                # select: up-halo <- Sum_j bundle_j * sel_up_j, etc.
                # bundles within agsb[j]: [h_top | h_bot | c_top | c_bot]
                tsel = [agpool.tile([F, HB], f16, name=f"tsel{i}",
                                    tag=f"tsel{i}") for i in range(4)]
                csel = [agpool.tile([F, HB], f32, name=f"csel{i}",
                                    tag=f"csel{i}") for i in range(4)]

                def select(dst_ap, boff, scol, tiles, eng):
                    # dst = sum_j agsb[:, j*4HB+boff : +HB] * sel[:, scol+j]
                    for j in range(4):
                        eng.tensor_scalar_mul(
                            tiles[j][:],
                            agsb[:, j * 4 * HB + boff:j * 4 * HB + boff + HB],
                            sel_sb[:, scol + j:scol + j + 1])
                    eng.tensor_add(tiles[0][:], tiles[0][:], tiles[1][:])
                    eng.tensor_add(tiles[2][:], tiles[2][:], tiles[3][:])
                    eng.tensor_add(dst_ap, tiles[0][:], tiles[2][:])

                # h up-halo rows [1,5): from up-neighbor's h_bot (boff=HB)
                select(nxt[64:128, WP:WP + HB], HB, 0, tsel, nc.vector)
                # h down-halo rows [21,25): from down-neighbor's h_top
                select(nxt[64:128, OWN_HI:OWN_HI + HB], 0, 4, tsel,
                       nc.vector)
                # c halos (f32 state) on gpsimd to offload DVE
                select(c_sb[:, WP:WP + HB], 3 * HB, 0, csel, nc.gpsimd)
                select(c_sb[:, OWN_HI:OWN_HI + HB], 2 * HB, 4, csel,
                       nc.gpsimd)

            # Nested halo-independent interiors: I_j needs only I_{j-1}'s h
            # (67-col tap margin), so all interiors of a block are runnable
            # while the preceding epoch's AllGather is still in flight.
            INT = [(397 + 67 * j, 1319 - 67 * j) for j in range(4)]

            pending = None           # (epoch step, agout) awaiting receive
            for b in range(4):
                stages = {}
                for j in range(4):
                    t = 4 * b + j + 1
                    cur = zin[(t - 1) % 4]
                    # full interior rows every step: keeps every buffer's
                    # x-half fresh (no stale-x reads from 4 steps ago)
                    nc.sync.dma_start(cur[0:64, WP:NCOL - WP],
                                      xT_d[t - 1, :, WP:NCOL - WP])
                    stages[t] = opool.tile([F, 16 * WP], f16, name=f"stage{t}",
                                           tag=f"stage{j}")
                # phase A: interiors (halo-independent), step order — these
                # overlap the in-flight AllGather from the previous block.
                # Split nested (67-col stagger) so step j+1's first chunk
                # depends only on step j's first chunk's pointwise chain.
                # 3 cones per interior: shrinking left cone + two
                # constant-width sliding cones; cone k of step j+1 depends
                # only on cones <= k of step j, so each transition chain is
                # covered by the later cones' matmuls
                for j in range(4):
                    t = 4 * b + j + 1
                    i0, i1 = INT[j]
                    q1, q2 = 891 - 67 * j, 1105 - 67 * j
                    emit_step(t, stages[t],
                              [(i0, q1 - i0), (q1, q2 - q1), (q2, i1 - q2)])
                # receive the previous epoch's halos only now, so no engine
                # queue stalls on the collective before phase A is dispatched
                if pending is not None:
                    emit_exchange_recv(*pending)
                    pending = None
                # phase B: boundary chunks, step order; epoch step's boundary
                # feeds the exchange
                for j in range(4):
                    t = 4 * b + j + 1
                    s = 3 - j
                    i0, i1 = INT[j]
                    c0, c1 = (5 - s) * WP, (21 + s) * WP
                    emit_step(t, stages[t],
                              [(c0, i0 - c0), (i1, c1 - i1)])
                for j in range(4):
                    t = 4 * b + j + 1
                    nc.sync.dma_start(out_d[t - 1], stages[t][:])
                if b < 3:
                    te = 4 * b + 4
                    pending = (te, emit_exchange_send(te))

        global _LAST_TC
        _LAST_TC = tc
    _split_multi_waits(nc)
    return nc


def _prep_inputs(x, Wx, Wh, b, gamma, beta, moving_mean, moving_var):
    x = np.asarray(x, F32)
    Wx = np.asarray(Wx, F32)
    Wh = np.asarray(Wh, F32)
    b = np.asarray(b, F32)
    # gate order along 4F: [i | f | g | o]; half1 = [i|f], half2 = [g|o].
    # Pre-scale i/f/o columns by 0.2 (hard_sigmoid slope).
    wstack = np.zeros((128, 18 * 128), F32)
    for k, (dy, dx) in enumerate(TAPS):
        ky, kx = dy + 1, dx + 1
        wstack[0:64, k * 128:(k + 1) * 128] = Wx[ky, kx, :, 0:128] * 0.2
        wstack[64:128, k * 128:(k + 1) * 128] = Wh[ky, kx, :, 0:128] * 0.2
        h2 = np.concatenate([Wx[ky, kx, :, 128:192],
                             Wx[ky, kx, :, 192:256] * 0.2], axis=1)
        wstack[0:64, 1152 + k * 128:1152 + (k + 1) * 128] = h2
        h2h = np.concatenate([Wh[ky, kx, :, 128:192],
                              Wh[ky, kx, :, 192:256] * 0.2], axis=1)
        wstack[64:128, 1152 + k * 128:1152 + (k + 1) * 128] = h2h
    wstack = wstack.astype(F16)

    b_if = (0.2 * b[0:128] + 0.5).reshape(128, 1).astype(F32)
    bg = b[128:192].reshape(64, 1).astype(F32)
    b_o = (0.2 * b[192:256] + 0.5).reshape(64, 1).astype(F32)
    inv = (np.asarray(gamma, F32) /
           np.sqrt(np.asarray(moving_var, F32) + 1e-3))
    bnb1 = (np.asarray(beta, F32) - np.asarray(moving_mean, F32) * inv)
    # duplicated into both partition halves: BN reads h at partitions 64-127
    bns = np.concatenate([inv, inv]).reshape(128, 1).astype(F32)
    bnb = np.concatenate([bnb1, bnb1]).reshape(128, 1).astype(F32)

    in_maps = []
    for core in range(8):
        bidx, sl = core // 4, core % 4
        r0 = 16 * sl
        glo, ghi = max(0, r0 - 5), min(64, r0 + 21)
        i0 = glo - (r0 - 5)
        xpad = np.zeros((T, NR, WP, C), F32)
        xpad[:, i0:i0 + (ghi - glo), 1:65, :] = x[bidx, :, glo:ghi, :, :]
        xT = np.ascontiguousarray(
            xpad.transpose(0, 3, 1, 2).reshape(T, C, NCOL)).astype(F16)
        m = np.zeros((NR, WP), F32)
        for i in range(NR):
            if 0 <= (r0 - 5 + i) < 64:
                m[i, 1:65] = 1.0
        mask = np.broadcast_to(
            m.reshape(1, NCOL), (64, NCOL)).astype(F16).copy()
        # select masks: sel[:, 0:4] = up (choose group-rank sl-1),
        # sel[:, 4:8] = down (choose group-rank sl+1)
        sel = np.zeros((64, 8), F32)
        if sl > 0:
            sel[:, sl - 1] = 1.0
        if sl < 3:
            sel[:, 4 + sl + 1] = 1.0
        in_maps.append({
            "xT": xT, "w": wstack, "mask": mask, "b_if": b_if,
            "bg": bg, "b_o": b_o, "bns": bns, "bnb": bnb, "sel": sel,
        })
    return in_maps


def kernel(x, Wx, Wh, b, gamma, beta, moving_mean, moving_var):
    global _PROG
    if _PROG is None:
        _PROG = _build()
    in_maps = _prep_inputs(x, Wx, Wh, b, gamma, beta, moving_mean, moving_var)
    res = run_bass_kernel_spmd(_PROG, in_maps, core_ids=list(range(8)))
    out = np.empty((2, T, 64, W, F), F32)
    for core in range(8):
        bidx, sl = core // 4, core % 4
        oc = res.results[core]["out"].astype(F32).reshape(
            T, F, 16, WP)[:, :, :, 1:65]
        out[bidx, :, 16 * sl:16 * sl + 16] = oc.transpose(0, 2, 3, 1)
    return out


# revision 43
# speedup vs baseline: 1.4461x; 1.0021x over previous
"""ConvLSTM2D (Keras gate order, hard_sigmoid) + inference BatchNorm on 8
Trainium2 NeuronCores.

Sharding: batch (2) x H-slabs (4) -> 8 cores. The T=16 recurrence is split
into 4 blocks of 4 steps. Within a block each core computes a shrinking halo
(depth 4); at block boundaries (t=4,8,12) cores exchange 4 boundary rows of
BOTH states (h and c) with their slab neighbors via one AllGather per epoch
(replica groups = the two 4-slab groups). This cuts redundant conv work from
1.94x (17-deep shrinking halo) to 1.19x while keeping the program uniform
SPMD: out-of-image edges are data (zero-padded x, h-mask, zero select masks).

Layout: channels-on-partitions. zin (128 x 1716 fp16) holds x_t on
partitions 0-63 and h_{t-1} on 64-127; 26 rows of 66 cols (64 + guard col
each side); a 3x3 conv tap (dy,dx) is the col offset dy*66+dx. One matmul
contracts x AND h channels at once (lhsT = [Wx_tap; Wh_tap]) so
z = conv(x,Wx)+conv(h,Wh) is 9 taps x 2 gate-halves of accumulating matmuls
per chunk (full 128x128 PE). Gate-i/f/o weight columns are pre-scaled by 0.2
so hard_sigmoid is Relu(psum + (0.2b+0.5)) then min(.,1). i|f activations are
fused into single 128-partition ops; pointwise math runs in fp16 on DVE
(2x rate) with c kept in fp32; BN is one DVE tensor_scalar (scale+bias) into
an fp16 output.
"""
import math
import numpy as np

import concourse.bass as bass
import concourse.mybir as mybir
import concourse.tile as tile
from concourse.bass_utils import run_bass_kernel_spmd

F16 = np.float16
F32 = np.float32

T, F, C, W = 16, 64, 64, 64
L = 4              # block length (steps between exchanges)
HALO = 4           # halo depth = L
NR = 16 + 2 * HALO + 2   # 26 buffer rows: [r0-5, r1+5)
WP = W + 2         # 66
NCOL = NR * WP     # 1716
OWN_LO, OWN_HI = 5 * WP, 21 * WP     # own 16 rows: buffer rows [5, 21)
HB = HALO * WP     # 264: one halo bundle (4 rows)
TAPS = [(dy, dx) for dy in (-1, 0, 1) for dx in (-1, 0, 1)]

TRACE_SIM = False
_PROG = None
_LAST_TC = None

# ---------------------------------------------------------------------------
# Workaround: this walrus build accepts at most ONE sync wait per
# instruction; Tile attaches several. Hoist extras onto same-engine NOPs
# inserted right before the instruction (per-engine order preserved).
_MAX_WAITS = 1


def _split_multi_waits(nc):
    for fn in nc.m.functions:
        for bb in fn.blocks:
            lst = bb.instructions
            out, changed = [], False
            for ins in lst:
                si = ins.sync_info
                if si is not None and len(si.on_wait) > _MAX_WAITS:
                    waits = list(si.on_wait)
                    extra, keep = waits[:-_MAX_WAITS], waits[-_MAX_WAITS:]
                    for j, w in enumerate(extra):
                        nop = mybir.InstNoOp(
                            name=f"{ins.name}.sw{j}", ins=[], outs=[],
                            text_hint="split_wait", bass_nofuse=True)
                        nop.engine = ins.engine
                        nop.sync_info = mybir.SyncInfo(on_wait=[w], on_update=[])
                        out.append(nop)
                    ins.sync_info = mybir.SyncInfo(
                        on_wait=keep, on_update=list(si.on_update))
                    changed = True
                out.append(ins)
            if changed:
                try:
                    bb.instructions = out
                except Exception:
                    lst.clear()
                    lst.extend(out)


def _chunks(c0, c1, maxn=512):
    Ln = c1 - c0
    n = max(1, math.ceil(Ln / maxn))
    base, rem = divmod(Ln, n)
    sizes = [base + (1 if i < rem else 0) for i in range(n)]
    out, p = [], c0
    for s in sizes:
        out.append((p, s))
        p += s
    return out


def _build():
    nc = bass.Bass(target_bir_lowering=False)
    f32, f16 = mybir.dt.float32, mybir.dt.float16

    xT_d = nc.dram_tensor("xT", [T, C, NCOL], f16, kind="ExternalInput")
    w_d = nc.dram_tensor("w", [128, 18 * 128], f16, kind="ExternalInput")
    mask_d = nc.dram_tensor("mask", [F, NCOL], f16, kind="ExternalInput")
    bif_d = nc.dram_tensor("b_if", [128, 1], f32, kind="ExternalInput")
    bg_d = nc.dram_tensor("bg", [F, 1], f32, kind="ExternalInput")
    bo_d = nc.dram_tensor("b_o", [128, 1], f32, kind="ExternalInput")
    bns_d = nc.dram_tensor("bns", [128, 1], f32, kind="ExternalInput")
    bnb_d = nc.dram_tensor("bnb", [128, 1], f32, kind="ExternalInput")
    sel_d = nc.dram_tensor("sel", [F, 8], f32, kind="ExternalInput")
    out_d = nc.dram_tensor("out", [T, F, 16 * WP], f16, kind="ExternalOutput")

    Relu = mybir.ActivationFunctionType.Relu
    Tanh = mybir.ActivationFunctionType.Tanh
    MULT = mybir.AluOpType.mult
    ADD = mybir.AluOpType.add

    with tile.TileContext(nc, trace_sim=TRACE_SIM) as tc:
        with (
            tc.tile_pool(name="const", bufs=1) as cpool,
            tc.tile_pool(name="state", bufs=1) as spool,
            tc.tile_pool(name="work", bufs=4) as wpool,
            tc.tile_pool(name="ostage", bufs=2) as opool,
            tc.tile_pool(name="agx", bufs=2) as agpool,
            tc.tile_pool(name="dram", bufs=2, space="DRAM") as dpool,
            tc.psum_pool(name="ps", bufs=4) as pspool,
        ):
            w_sb = cpool.tile([128, 18 * 128], f16)
            mask_sb = cpool.tile([F, NCOL], f16)
            bif_sb = cpool.tile([128, 1], f32)
            bg_sb = cpool.tile([F, 1], f32)
            bo_sb = cpool.tile([128, 1], f32)
            bns_sb = cpool.tile([128, 1], f32)
            bnb_sb = cpool.tile([128, 1], f32)
            sel_sb = cpool.tile([F, 8], f32)
            # x on the SP queue (feed PE first); weights + small consts on
            # the ACT queue so they don't head-of-line-block the x DMAs
            nc.scalar.dma_start(w_sb[:], w_d[:])       # first: feeds PE
            nc.scalar.dma_start(mask_sb[:], mask_d[:])
            nc.scalar.dma_start(bif_sb[:], bif_d[:])
            nc.scalar.dma_start(bg_sb[:], bg_d[:])
            nc.scalar.dma_start(bo_sb[:], bo_d[:])
            nc.scalar.dma_start(bns_sb[:], bns_d[:])
            nc.scalar.dma_start(bnb_sb[:], bnb_d[:])
            nc.scalar.dma_start(sel_sb[:], sel_d[:])

            # 4 rotating buffers: step t reads x_t+h_{t-1} from zin[(t-1)%4]
            # and writes h_t into zin[t%4]. 4 (not 2) so a step's x DMA never
            # lands in a buffer whose x a not-yet-emitted chunk still reads.
            zin = [spool.tile([128, NCOL], f16, name=f"zin{i}", tag=f"zin{i}")
                   for i in range(4)]
            c_sb = spool.tile([F, NCOL], f32, tag="cstate")
            # h-halves and x guard rows must start zero (NaN garbage would
            # survive the h mask multiply via guard-col taps); x interior is
            # fully overwritten by the per-step DMA.
            for i in range(4):
                eng = nc.vector if i % 2 == 0 else nc.gpsimd
                eng.memset(zin[i][64:128, :], 0.0)
                eng.memset(zin[i][0:64, 0:WP], 0.0)
                eng.memset(zin[i][0:64, NCOL - WP:NCOL], 0.0)
            nc.gpsimd.memset(c_sb[:], 0.0)

            def emit_chunk_head(t, p0, n):
                cur = zin[(t - 1) % 4]
                ps_if = pspool.tile([128, n], f32, tag="psif")
                ps_go = pspool.tile([128, n], f32, tag="psgo")
                for k, (dy, dx) in enumerate(TAPS):
                    off = p0 + dy * WP + dx
                    nc.tensor.matmul(
                        ps_if[:], w_sb[:, k * 128:(k + 1) * 128],
                        cur[:, off:off + n], start=(k == 0), stop=(k == 8))
                for k, (dy, dx) in enumerate(TAPS):
                    off = p0 + dy * WP + dx
                    nc.tensor.matmul(
                        ps_go[:], w_sb[:, 1152 + k * 128:1152 + (k + 1) * 128],
                        cur[:, off:off + n], start=(k == 0), stop=(k == 8))

                sig_if = wpool.tile([128, n], f16, tag="sig_if")
                f_low = wpool.tile([F, n], f16, tag="f_low")
                tanh_g = wpool.tile([F, n], f16, tag="tanh_g")
                sig_o = wpool.tile([F, n], f16, tag="sig_o")
                t1 = wpool.tile([F, n], f16, tag="t1")
                t2 = wpool.tile([F, n], f32, tag="t2")

                # i|f fused: weights pre-scaled by 0.2 -> Relu(ps + 0.2b+0.5)
                nc.scalar.activation(sig_if[:], ps_if[:], Relu,
                                     bias=bif_sb[:, 0:1])
                # DVE TensorTensor needs equal input base partitions: min the
                # f-half down to partitions 0-63 while clipping it
                nc.vector.tensor_scalar_min(sig_if[0:64, :],
                                            sig_if[0:64, :], 1.0)
                nc.vector.tensor_scalar_min(f_low[:], sig_if[64:128, :], 1.0)
                nc.scalar.activation(tanh_g[:], ps_go[0:64, :], Tanh,
                                     bias=bg_sb[:, 0:1])
                nc.scalar.activation(sig_o[:], ps_go[64:128, :], Relu,
                                     bias=bo_sb[64:128, 0:1])
                # min(o,1) and the h edge-mask fused: mask is 1 in-image (so
                # min(o, 1)) and 0 outside (so o -> 0 -> h = 0)
                nc.vector.tensor_tensor(sig_o[:], sig_o[:],
                                        mask_sb[:, p0:p0 + n],
                                        mybir.AluOpType.min)
                nc.vector.tensor_mul(t1[:], sig_if[0:64, :], tanh_g[:])
                nc.vector.tensor_mul(t2[:], f_low[:], c_sb[:, p0:p0 + n])
                nc.vector.tensor_add(c_sb[:, p0:p0 + n], t1[:], t2[:])
                return sig_o

            def emit_chunk_tail(t, stage, p0, n, sig_o):
                # second pass: tanh(c) and h, emitted after every chunk's
                # gate work so a blocked tanh_c can't head-of-line-block the
                # next chunk's ready activations in the ACT FIFO
                nxt = zin[t % 4]
                tanh_c = wpool.tile([F, n], f16, tag="tanh_c")
                nc.scalar.activation(tanh_c[:], c_sb[:, p0:p0 + n], Tanh)
                nc.vector.tensor_mul(nxt[64:128, p0:p0 + n],
                                     sig_o[:], tanh_c[:])
                lo, hi = max(p0, OWN_LO), min(p0 + n, OWN_HI)
                if lo < hi:
                    # BN on Pool: off the critical recurrence path, so the
                    # collective blocking Pool only delays the output stage
                    nc.gpsimd.tensor_scalar(
                        stage[:, lo - OWN_LO:hi - OWN_LO],
                        nxt[64:128, lo:hi],
                        bns_sb[64:128, 0:1], bnb_sb[64:128, 0:1], MULT, ADD)

            def emit_step(t, stage, chunk_list):
                heads = [(p0, n, emit_chunk_head(t, p0, n))
                         for p0, n in chunk_list]
                for p0, n, sig_o in heads:
                    emit_chunk_tail(t, stage, p0, n, sig_o)

            def emit_exchange_send(t):
                # ---- exchange epoch: ship h,c boundary rows (4 each) ----
                nxt = zin[t % 4]
                cbf = agpool.tile([F, 2 * HB], f16, tag="cbf")
                nc.vector.tensor_scalar_mul(
                    cbf[:, 0:HB], c_sb[:, OWN_LO:OWN_LO + HB], 1.0)
                nc.vector.tensor_scalar_mul(
                    cbf[:, HB:2 * HB], c_sb[:, OWN_HI - HB:OWN_HI], 1.0)
                agin = dpool.tile([F, 4 * HB], f16, tag="agin")
                agout = dpool.tile([4 * F, 4 * HB], f16, tag="agout")
                nc.sync.dma_start(agin[:, 0:HB],
                                  nxt[64:128, OWN_LO:OWN_LO + HB])
                nc.sync.dma_start(agin[:, HB:2 * HB],
                                  nxt[64:128, OWN_HI - HB:OWN_HI])
                nc.sync.dma_start(agin[:, 2 * HB:4 * HB], cbf[:])
                nc.gpsimd.collective_compute(
                    "AllGather", mybir.AluOpType.bypass,
                    ins=[agin.opt()], outs=[agout.opt()],
                    replica_groups=[[0, 1, 2, 3], [4, 5, 6, 7]],
                )
                return agout

            def emit_exchange_recv(t, agout):
                nxt = zin[t % 4]
                # h and c halves land via parallel DMA queues (SP / ACT)
                agsb_h = agpool.tile([F, 8 * HB], f16, tag="agsb_h")
                agsb_c = agpool.tile([F, 8 * HB], f16, tag="agsb_c")
                nc.sync.dma_start(
                    agsb_h[:].rearrange("p (j c) -> p j c", j=4),
                    agout[:, 0:2 * HB].rearrange("(j p) c -> p j c", j=4))
                nc.scalar.dma_start(
                    agsb_c[:].rearrange("p (j c) -> p j c", j=4),
                    agout[:, 2 * HB:4 * HB].rearrange("(j p) c -> p j c",
                                                      j=4))
                # select: up-halo <- Sum_j bundle_j * sel_up_j, etc.
                # bundles within agsb_*[j]: [top | bottom]
                tsel = [agpool.tile([F, HB], f16, name=f"tsel{i}",
                                    tag=f"tsel{i}") for i in range(4)]
                csel = [agpool.tile([F, HB], f32, name=f"csel{i}",
                                    tag=f"csel{i}") for i in range(4)]

                def select(src, dst_ap, boff, scol, tiles, eng):
                    # dst = sum_j src[:, j*2HB+boff : +HB] * sel[:, scol+j]
                    for j in range(4):
                        eng.tensor_scalar_mul(
                            tiles[j][:],
                            src[:, j * 2 * HB + boff:j * 2 * HB + boff + HB],
                            sel_sb[:, scol + j:scol + j + 1])
                    eng.tensor_add(tiles[0][:], tiles[0][:], tiles[1][:])
                    eng.tensor_add(tiles[2][:], tiles[2][:], tiles[3][:])
                    eng.tensor_add(dst_ap, tiles[0][:], tiles[2][:])

                # h up-halo rows [1,5): from up-neighbor's h_bot (boff=HB)
                select(agsb_h, nxt[64:128, WP:WP + HB], HB, 0, tsel,
                       nc.vector)
                # h down-halo rows [21,25): from down-neighbor's h_top
                select(agsb_h, nxt[64:128, OWN_HI:OWN_HI + HB], 0, 4, tsel,
                       nc.vector)
                # c halos (f32 state) on gpsimd to offload DVE
                select(agsb_c, c_sb[:, WP:WP + HB], HB, 0, csel, nc.gpsimd)
                select(agsb_c, c_sb[:, OWN_HI:OWN_HI + HB], 0, 4, csel,
                       nc.gpsimd)

            # Nested halo-independent interiors: I_j needs only I_{j-1}'s h
            # (67-col tap margin), so all interiors of a block are runnable
            # while the preceding epoch's AllGather is still in flight.
            INT = [(397 + 67 * j, 1319 - 67 * j) for j in range(4)]

            pending = None           # (epoch step, agout) awaiting receive
            for b in range(4):
                stages = {}
                for j in range(4):
                    t = 4 * b + j + 1
                    cur = zin[(t - 1) % 4]
                    # full interior rows every step: keeps every buffer's
                    # x-half fresh (no stale-x reads from 4 steps ago)
                    nc.sync.dma_start(cur[0:64, WP:NCOL - WP],
                                      xT_d[t - 1, :, WP:NCOL - WP])
                    stages[t] = opool.tile([F, 16 * WP], f16, name=f"stage{t}",
                                           tag=f"stage{j}")
                # phase A: interiors (halo-independent), step order — these
                # overlap the in-flight AllGather from the previous block.
                # Split nested (67-col stagger) so step j+1's first chunk
                # depends only on step j's first chunk's pointwise chain.
                # 3 cones per interior: shrinking left cone + two
                # constant-width sliding cones; cone k of step j+1 depends
                # only on cones <= k of step j, so each transition chain is
                # covered by the later cones' matmuls
                for j in range(4):
                    t = 4 * b + j + 1
                    i0, i1 = INT[j]
                    q1, q2 = 891 - 67 * j, 1105 - 67 * j
                    emit_step(t, stages[t],
                              [(i0, q1 - i0), (q1, q2 - q1), (q2, i1 - q2)])
                # receive the previous epoch's halos only now, so no engine
                # queue stalls on the collective before phase A is dispatched
                if pending is not None:
                    emit_exchange_recv(*pending)
                    pending = None
                # phase B: boundary chunks, step order; epoch step's boundary
                # feeds the exchange
                for j in range(4):
                    t = 4 * b + j + 1
                    s = 3 - j
                    i0, i1 = INT[j]
                    c0, c1 = (5 - s) * WP, (21 + s) * WP
                    emit_step(t, stages[t],
                              [(c0, i0 - c0), (i1, c1 - i1)])
                for j in range(4):
                    t = 4 * b + j + 1
                    # Pool queue: the out DMA waits on Pool BNs, which are
                    # earlier in the same queue - so it never stalls the
                    # queue head, and it can't block SP's agsb DMA either
                    nc.gpsimd.dma_start(out_d[t - 1], stages[t][:])
                if b < 3:
                    te = 4 * b + 4
                    pending = (te, emit_exchange_send(te))

        global _LAST_TC
        _LAST_TC = tc
    _split_multi_waits(nc)
    return nc


def _prep_inputs(x, Wx, Wh, b, gamma, beta, moving_mean, moving_var):
    x = np.asarray(x, F32)
    Wx = np.asarray(Wx, F32)
    Wh = np.asarray(Wh, F32)
    b = np.asarray(b, F32)
    # gate order along 4F: [i | f | g | o]; half1 = [i|f], half2 = [g|o].
    # Pre-scale i/f/o columns by 0.2 (hard_sigmoid slope).
    wstack = np.zeros((128, 18 * 128), F32)
    for k, (dy, dx) in enumerate(TAPS):
        ky, kx = dy + 1, dx + 1
        wstack[0:64, k * 128:(k + 1) * 128] = Wx[ky, kx, :, 0:128] * 0.2
        wstack[64:128, k * 128:(k + 1) * 128] = Wh[ky, kx, :, 0:128] * 0.2
        h2 = np.concatenate([Wx[ky, kx, :, 128:192],
                             Wx[ky, kx, :, 192:256] * 0.2], axis=1)
        wstack[0:64, 1152 + k * 128:1152 + (k + 1) * 128] = h2
        h2h = np.concatenate([Wh[ky, kx, :, 128:192],
                              Wh[ky, kx, :, 192:256] * 0.2], axis=1)
        wstack[64:128, 1152 + k * 128:1152 + (k + 1) * 128] = h2h
    wstack = wstack.astype(F16)

    b_if = (0.2 * b[0:128] + 0.5).reshape(128, 1).astype(F32)
    bg = b[128:192].reshape(64, 1).astype(F32)
    bo1 = 0.2 * b[192:256] + 0.5
    b_o = np.concatenate([bo1, bo1]).reshape(128, 1).astype(F32)
    inv = (np.asarray(gamma, F32) /
           np.sqrt(np.asarray(moving_var, F32) + 1e-3))
    bnb1 = (np.asarray(beta, F32) - np.asarray(moving_mean, F32) * inv)
    # duplicated into both partition halves: BN reads h at partitions 64-127
    bns = np.concatenate([inv, inv]).reshape(128, 1).astype(F32)
    bnb = np.concatenate([bnb1, bnb1]).reshape(128, 1).astype(F32)

    in_maps = []
    for core in range(8):
        bidx, sl = core // 4, core % 4
        r0 = 16 * sl
        glo, ghi = max(0, r0 - 5), min(64, r0 + 21)
        i0 = glo - (r0 - 5)
        xpad = np.zeros((T, NR, WP, C), F32)
        xpad[:, i0:i0 + (ghi - glo), 1:65, :] = x[bidx, :, glo:ghi, :, :]
        xT = np.ascontiguousarray(
            xpad.transpose(0, 3, 1, 2).reshape(T, C, NCOL)).astype(F16)
        m = np.zeros((NR, WP), F32)
        for i in range(NR):
            if 0 <= (r0 - 5 + i) < 64:
                m[i, 1:65] = 1.0
        mask = np.broadcast_to(
            m.reshape(1, NCOL), (64, NCOL)).astype(F16).copy()
        # select masks: sel[:, 0:4] = up (choose group-rank sl-1),
        # sel[:, 4:8] = down (choose group-rank sl+1)
        sel = np.zeros((64, 8), F32)
        if sl > 0:
            sel[:, sl - 1] = 1.0
        if sl < 3:
            sel[:, 4 + sl + 1] = 1.0
        in_maps.append({
            "xT": xT, "w": wstack, "mask": mask, "b_if": b_if,
            "bg": bg, "b_o": b_o, "bns": bns, "bnb": bnb, "sel": sel,
        })
    return in_maps


def kernel(x, Wx, Wh, b, gamma, beta, moving_mean, moving_var):
    global _PROG
    if _PROG is None:
        _PROG = _build()
    in_maps = _prep_inputs(x, Wx, Wh, b, gamma, beta, moving_mean, moving_var)
    res = run_bass_kernel_spmd(_PROG, in_maps, core_ids=list(range(8)))
    out = np.empty((2, T, 64, W, F), F32)
    for core in range(8):
        bidx, sl = core // 4, core % 4
        oc = res.results[core]["out"].astype(F32).reshape(
            T, F, 16, WP)[:, :, :, 1:65]
        out[bidx, :, 16 * sl:16 * sl + 16] = oc.transpose(0, 2, 3, 1)
    return out


# revision 47
# speedup vs baseline: 1.4490x; 1.0020x over previous
"""ConvLSTM2D (Keras gate order, hard_sigmoid) + inference BatchNorm on 8
Trainium2 NeuronCores.

Sharding: batch (2) x H-slabs (4) -> 8 cores. The T=16 recurrence is split
into 4 blocks of 4 steps. Within a block each core computes a shrinking halo
(depth 4); at block boundaries (t=4,8,12) cores exchange 4 boundary rows of
BOTH states (h and c) with their slab neighbors via one AllGather per epoch
(replica groups = the two 4-slab groups). This cuts redundant conv work from
1.94x (17-deep shrinking halo) to 1.19x while keeping the program uniform
SPMD: out-of-image edges are data (zero-padded x, h-mask, zero select masks).

Layout: channels-on-partitions. zin (128 x 1716 fp16) holds x_t on
partitions 0-63 and h_{t-1} on 64-127; 26 rows of 66 cols (64 + guard col
each side); a 3x3 conv tap (dy,dx) is the col offset dy*66+dx. One matmul
contracts x AND h channels at once (lhsT = [Wx_tap; Wh_tap]) so
z = conv(x,Wx)+conv(h,Wh) is 9 taps x 2 gate-halves of accumulating matmuls
per chunk (full 128x128 PE). Gate-i/f/o weight columns are pre-scaled by 0.2
so hard_sigmoid is Relu(psum + (0.2b+0.5)) then min(.,1). i|f activations are
fused into single 128-partition ops; pointwise math runs in fp16 on DVE
(2x rate) with c kept in fp32; BN is one DVE tensor_scalar (scale+bias) into
an fp16 output.
"""
import math
import numpy as np

import concourse.bass as bass
import concourse.mybir as mybir
import concourse.tile as tile
from concourse.bass_utils import run_bass_kernel_spmd

F16 = np.float16
F32 = np.float32

T, F, C, W = 16, 64, 64, 64
L = 4              # block length (steps between exchanges)
HALO = 4           # halo depth = L
NR = 16 + 2 * HALO + 2   # 26 buffer rows: [r0-5, r1+5)
WP = W + 2         # 66
NCOL = NR * WP     # 1716
OWN_LO, OWN_HI = 5 * WP, 21 * WP     # own 16 rows: buffer rows [5, 21)
HB = HALO * WP     # 264: one halo bundle (4 rows)
TAPS = [(dy, dx) for dy in (-1, 0, 1) for dx in (-1, 0, 1)]

TRACE_SIM = False
_PROG = None
_LAST_TC = None

# ---------------------------------------------------------------------------
# Workaround: this walrus build accepts at most ONE sync wait per
# instruction; Tile attaches several. Hoist extras onto same-engine NOPs
# inserted right before the instruction (per-engine order preserved).
_MAX_WAITS = 1


def _split_multi_waits(nc):
    for fn in nc.m.functions:
        for bb in fn.blocks:
            lst = bb.instructions
            out, changed = [], False
            for ins in lst:
                si = ins.sync_info
                if si is not None and len(si.on_wait) > _MAX_WAITS:
                    waits = list(si.on_wait)
                    extra, keep = waits[:-_MAX_WAITS], waits[-_MAX_WAITS:]
                    for j, w in enumerate(extra):
                        nop = mybir.InstNoOp(
                            name=f"{ins.name}.sw{j}", ins=[], outs=[],
                            text_hint="split_wait", bass_nofuse=True)
                        nop.engine = ins.engine
                        nop.sync_info = mybir.SyncInfo(on_wait=[w], on_update=[])
                        out.append(nop)
                    ins.sync_info = mybir.SyncInfo(
                        on_wait=keep, on_update=list(si.on_update))
                    changed = True
                out.append(ins)
            if changed:
                try:
                    bb.instructions = out
                except Exception:
                    lst.clear()
                    lst.extend(out)


def _chunks(c0, c1, maxn=512):
    Ln = c1 - c0
    n = max(1, math.ceil(Ln / maxn))
    base, rem = divmod(Ln, n)
    sizes = [base + (1 if i < rem else 0) for i in range(n)]
    out, p = [], c0
    for s in sizes:
        out.append((p, s))
        p += s
    return out


def _build():
    nc = bass.Bass(target_bir_lowering=False)
    f32, f16 = mybir.dt.float32, mybir.dt.float16

    xT_d = nc.dram_tensor("xT", [T, C, NCOL], f16, kind="ExternalInput")
    w_d = nc.dram_tensor("w", [128, 18 * 128], f16, kind="ExternalInput")
    mask_d = nc.dram_tensor("mask", [F, NCOL], f16, kind="ExternalInput")
    bif_d = nc.dram_tensor("b_if", [128, 1], f32, kind="ExternalInput")
    bg_d = nc.dram_tensor("bg", [F, 1], f32, kind="ExternalInput")
    bo_d = nc.dram_tensor("b_o", [128, 1], f32, kind="ExternalInput")
    bns_d = nc.dram_tensor("bns", [128, 1], f32, kind="ExternalInput")
    bnb_d = nc.dram_tensor("bnb", [128, 1], f32, kind="ExternalInput")
    sel_d = nc.dram_tensor("sel", [F, 8], f32, kind="ExternalInput")
    out_d = nc.dram_tensor("out", [T, F, 16 * WP], f16, kind="ExternalOutput")

    Relu = mybir.ActivationFunctionType.Relu
    Tanh = mybir.ActivationFunctionType.Tanh
    MULT = mybir.AluOpType.mult
    ADD = mybir.AluOpType.add

    with tile.TileContext(nc, trace_sim=TRACE_SIM) as tc:
        with (
            tc.tile_pool(name="const", bufs=1) as cpool,
            tc.tile_pool(name="state", bufs=1) as spool,
            tc.tile_pool(name="work", bufs=4) as wpool,
            tc.tile_pool(name="ostage", bufs=2) as opool,
            tc.tile_pool(name="agx", bufs=2) as agpool,
            tc.tile_pool(name="dram", bufs=2, space="DRAM") as dpool,
            tc.psum_pool(name="ps", bufs=4) as pspool,
        ):
            w_sb = cpool.tile([128, 18 * 128], f16)
            mask_sb = cpool.tile([F, NCOL], f16)
            bif_sb = cpool.tile([128, 1], f32)
            bg_sb = cpool.tile([F, 1], f32)
            bo_sb = cpool.tile([128, 1], f32)
            bns_sb = cpool.tile([128, 1], f32)
            bnb_sb = cpool.tile([128, 1], f32)
            sel_sb = cpool.tile([F, 8], f32)
            # x on the SP queue (feed PE first); weights + small consts on
            # the ACT queue so they don't head-of-line-block the x DMAs
            nc.scalar.dma_start(w_sb[:], w_d[:])       # first: feeds PE
            nc.scalar.dma_start(mask_sb[:], mask_d[:])
            nc.scalar.dma_start(bif_sb[:], bif_d[:])
            nc.scalar.dma_start(bg_sb[:], bg_d[:])
            nc.scalar.dma_start(bo_sb[:], bo_d[:])
            nc.scalar.dma_start(bns_sb[:], bns_d[:])
            nc.scalar.dma_start(bnb_sb[:], bnb_d[:])
            nc.scalar.dma_start(sel_sb[:], sel_d[:])

            # 4 rotating buffers: step t reads x_t+h_{t-1} from zin[(t-1)%4]
            # and writes h_t into zin[t%4]. 4 (not 2) so a step's x DMA never
            # lands in a buffer whose x a not-yet-emitted chunk still reads.
            zin = [spool.tile([128, NCOL], f16, name=f"zin{i}", tag=f"zin{i}")
                   for i in range(4)]
            c_sb = spool.tile([F, NCOL], f32, tag="cstate")
            # h-halves and x guard rows must start zero (NaN garbage would
            # survive the h mask multiply via guard-col taps); x interior is
            # fully overwritten by the per-step DMA.
            for i in range(4):
                eng = nc.vector if i % 2 == 0 else nc.gpsimd
                eng.memset(zin[i][64:128, :], 0.0)
                eng.memset(zin[i][0:64, 0:WP], 0.0)
                eng.memset(zin[i][0:64, NCOL - WP:NCOL], 0.0)
            nc.gpsimd.memset(c_sb[:], 0.0)

            def emit_chunk_head(t, p0, n):
                cur = zin[(t - 1) % 4]
                ps_if = pspool.tile([128, n], f32, tag="psif")
                ps_go = pspool.tile([128, n], f32, tag="psgo")
                for k, (dy, dx) in enumerate(TAPS):
                    off = p0 + dy * WP + dx
                    nc.tensor.matmul(
                        ps_if[:], w_sb[:, k * 128:(k + 1) * 128],
                        cur[:, off:off + n], start=(k == 0), stop=(k == 8))
                for k, (dy, dx) in enumerate(TAPS):
                    off = p0 + dy * WP + dx
                    nc.tensor.matmul(
                        ps_go[:], w_sb[:, 1152 + k * 128:1152 + (k + 1) * 128],
                        cur[:, off:off + n], start=(k == 0), stop=(k == 8))

                sig_if = wpool.tile([128, n], f16, tag="sig_if")
                f_low = wpool.tile([F, n], f16, tag="f_low")
                tanh_g = wpool.tile([F, n], f16, tag="tanh_g")
                sig_o = wpool.tile([F, n], f16, tag="sig_o")
                t1 = wpool.tile([F, n], f16, tag="t1")
                t2 = wpool.tile([F, n], f32, tag="t2")

                # i|f fused: weights pre-scaled by 0.2 -> Relu(ps + 0.2b+0.5)
                nc.scalar.activation(sig_if[:], ps_if[:], Relu,
                                     bias=bif_sb[:, 0:1])
                # DVE TensorTensor needs equal input base partitions: min the
                # f-half down to partitions 0-63 while clipping it
                nc.vector.tensor_scalar_min(sig_if[0:64, :],
                                            sig_if[0:64, :], 1.0)
                nc.vector.tensor_scalar_min(f_low[:], sig_if[64:128, :], 1.0)
                nc.scalar.activation(tanh_g[:], ps_go[0:64, :], Tanh,
                                     bias=bg_sb[:, 0:1])
                nc.scalar.activation(sig_o[:], ps_go[64:128, :], Relu,
                                     bias=bo_sb[64:128, 0:1])
                # min(o,1) and the h edge-mask fused: mask is 1 in-image (so
                # min(o, 1)) and 0 outside (so o -> 0 -> h = 0)
                nc.vector.tensor_tensor(sig_o[:], sig_o[:],
                                        mask_sb[:, p0:p0 + n],
                                        mybir.AluOpType.min)
                nc.vector.tensor_mul(t1[:], sig_if[0:64, :], tanh_g[:])
                nc.vector.tensor_mul(t2[:], f_low[:], c_sb[:, p0:p0 + n])
                nc.vector.tensor_add(c_sb[:, p0:p0 + n], t1[:], t2[:])
                return sig_o

            def emit_chunk_tail(t, stage, p0, n, sig_o):
                # second pass: tanh(c) and h, emitted after every chunk's
                # gate work so a blocked tanh_c can't head-of-line-block the
                # next chunk's ready activations in the ACT FIFO
                nxt = zin[t % 4]
                tanh_c = wpool.tile([F, n], f16, tag="tanh_c")
                nc.scalar.activation(tanh_c[:], c_sb[:, p0:p0 + n], Tanh)
                nc.vector.tensor_mul(nxt[64:128, p0:p0 + n],
                                     sig_o[:], tanh_c[:])
                lo, hi = max(p0, OWN_LO), min(p0 + n, OWN_HI)
                if lo < hi:
                    # BN on Pool: off the critical recurrence path, so the
                    # collective blocking Pool only delays the output stage
                    nc.gpsimd.tensor_scalar(
                        stage[:, lo - OWN_LO:hi - OWN_LO],
                        nxt[64:128, lo:hi],
                        bns_sb[64:128, 0:1], bnb_sb[64:128, 0:1], MULT, ADD)

            def emit_step(t, stage, chunk_list):
                heads = [(p0, n, emit_chunk_head(t, p0, n))
                         for p0, n in chunk_list]
                for p0, n, sig_o in heads:
                    emit_chunk_tail(t, stage, p0, n, sig_o)

            def emit_exchange_send(t):
                # ---- exchange epoch: ship h,c boundary rows (4 each) ----
                nxt = zin[t % 4]
                cbf = agpool.tile([F, 2 * HB], f16, tag="cbf")
                nc.vector.tensor_scalar_mul(
                    cbf[:, 0:HB], c_sb[:, OWN_LO:OWN_LO + HB], 1.0)
                nc.vector.tensor_scalar_mul(
                    cbf[:, HB:2 * HB], c_sb[:, OWN_HI - HB:OWN_HI], 1.0)
                agin = dpool.tile([F, 4 * HB], f16, tag="agin")
                agout = dpool.tile([4 * F, 4 * HB], f16, tag="agout")
                nc.sync.dma_start(agin[:, 0:HB],
                                  nxt[64:128, OWN_LO:OWN_LO + HB])
                nc.sync.dma_start(agin[:, HB:2 * HB],
                                  nxt[64:128, OWN_HI - HB:OWN_HI])
                nc.sync.dma_start(agin[:, 2 * HB:4 * HB], cbf[:])
                nc.gpsimd.collective_compute(
                    "AllGather", mybir.AluOpType.bypass,
                    ins=[agin.opt()], outs=[agout.opt()],
                    replica_groups=[[0, 1, 2, 3], [4, 5, 6, 7]],
                )
                return agout

            def emit_exchange_recv(t, agout):
                nxt = zin[t % 4]
                # h and c halves land via parallel DMA queues (SP / ACT)
                agsb_h = agpool.tile([F, 8 * HB], f16, tag="agsb_h")
                agsb_c = agpool.tile([F, 8 * HB], f16, tag="agsb_c")
                nc.sync.dma_start(
                    agsb_h[:].rearrange("p (j c) -> p j c", j=4),
                    agout[:, 0:2 * HB].rearrange("(j p) c -> p j c", j=4))
                nc.scalar.dma_start(
                    agsb_c[:].rearrange("p (j c) -> p j c", j=4),
                    agout[:, 2 * HB:4 * HB].rearrange("(j p) c -> p j c",
                                                      j=4))
                # select: up-halo <- Sum_j bundle_j * sel_up_j, etc.
                # bundles within agsb_*[j]: [top | bottom]
                tsel = [agpool.tile([F, HB], f16, name=f"tsel{i}",
                                    tag=f"tsel{i}") for i in range(4)]
                csel = [agpool.tile([F, HB], f32, name=f"csel{i}",
                                    tag=f"csel{i}") for i in range(4)]

                def select(src, dst_ap, boff, scol, tiles, eng):
                    # dst = sum_j src[:, j*2HB+boff : +HB] * sel[:, scol+j]
                    for j in range(4):
                        eng.tensor_scalar_mul(
                            tiles[j][:],
                            src[:, j * 2 * HB + boff:j * 2 * HB + boff + HB],
                            sel_sb[:, scol + j:scol + j + 1])
                    eng.tensor_add(tiles[0][:], tiles[0][:], tiles[1][:])
                    eng.tensor_add(tiles[2][:], tiles[2][:], tiles[3][:])
                    eng.tensor_add(dst_ap, tiles[0][:], tiles[2][:])

                # h up-halo rows [1,5): from up-neighbor's h_bot (boff=HB)
                select(agsb_h, nxt[64:128, WP:WP + HB], HB, 0, tsel,
                       nc.vector)
                # h down-halo rows [21,25): from down-neighbor's h_top
                select(agsb_h, nxt[64:128, OWN_HI:OWN_HI + HB], 0, 4, tsel,
                       nc.vector)
                # c halos (f32 state) on gpsimd to offload DVE
                select(agsb_c, c_sb[:, WP:WP + HB], HB, 0, csel, nc.gpsimd)
                select(agsb_c, c_sb[:, OWN_HI:OWN_HI + HB], 0, 4, csel,
                       nc.gpsimd)

            # Nested halo-independent interiors: I_j needs only I_{j-1}'s h
            # (67-col tap margin), so all interiors of a block are runnable
            # while the preceding epoch's AllGather is still in flight.
            INT = [(397 + 67 * j, 1319 - 67 * j) for j in range(4)]

            pending = None           # (epoch step, agout) awaiting receive
            for b in range(4):
                stages = {}
                for j in range(4):
                    t = 4 * b + j + 1
                    cur = zin[(t - 1) % 4]
                    # full interior rows every step: keeps every buffer's
                    # x-half fresh (no stale-x reads from 4 steps ago)
                    nc.sync.dma_start(cur[0:64, WP:NCOL - WP],
                                      xT_d[t - 1, :, WP:NCOL - WP])
                    stages[t] = opool.tile([F, 16 * WP], f16, name=f"stage{t}",
                                           tag=f"stage{j}")
                # phase A: interiors (halo-independent), step order — these
                # overlap the in-flight AllGather from the previous block.
                # Split nested (67-col stagger) so step j+1's first chunk
                # depends only on step j's first chunk's pointwise chain.
                # 3 cones per interior: shrinking left cone + two
                # constant-width sliding cones; cone k of step j+1 depends
                # only on cones <= k of step j, so each transition chain is
                # covered by the later cones' matmuls
                for j in range(4):
                    t = 4 * b + j + 1
                    i0, i1 = INT[j]
                    q1, q2 = 891 - 67 * j, 1105 - 67 * j
                    emit_step(t, stages[t],
                              [(i0, q1 - i0), (q1, q2 - q1), (q2, i1 - q2)])
                # receive the previous epoch's halos only now, so no engine
                # queue stalls on the collective before phase A is dispatched
                if pending is not None:
                    emit_exchange_recv(*pending)
                    pending = None
                # phase B: boundary chunks, step order; epoch step's boundary
                # feeds the exchange
                for j in range(4):
                    t = 4 * b + j + 1
                    s = 3 - j
                    i0, i1 = INT[j]
                    c0, c1 = (5 - s) * WP, (21 + s) * WP
                    if t == T:
                        # final step: halve the right chunk so the kernel's
                        # tail chain (gates -> BN -> out DMA) is shorter
                        m = (i1 + c1) // 2
                        emit_step(t, stages[t],
                                  [(c0, i0 - c0), (i1, m - i1), (m, c1 - m)])
                    else:
                        emit_step(t, stages[t],
                                  [(c0, i0 - c0), (i1, c1 - i1)])
                for j in range(4):
                    t = 4 * b + j + 1
                    # Pool queue: the out DMA waits on Pool BNs, which are
                    # earlier in the same queue - so it never stalls the
                    # queue head, and it can't block SP's agsb DMA either
                    if t == T:
                        # split halves on the (idle-by-now) SP queue so the
                        # first half ships while the last BN finishes
                        nc.sync.dma_start(out_d[t - 1, :, 0:788],
                                          stages[t][:, 0:788])
                        nc.sync.dma_start(out_d[t - 1, :, 788:16 * WP],
                                          stages[t][:, 788:16 * WP])
                    else:
                        nc.gpsimd.dma_start(out_d[t - 1], stages[t][:])
                if b < 3:
                    te = 4 * b + 4
                    pending = (te, emit_exchange_send(te))

        global _LAST_TC
        _LAST_TC = tc
    _split_multi_waits(nc)
    return nc


def _prep_inputs(x, Wx, Wh, b, gamma, beta, moving_mean, moving_var):
    x = np.asarray(x, F32)
    Wx = np.asarray(Wx, F32)
    Wh = np.asarray(Wh, F32)
    b = np.asarray(b, F32)
    # gate order along 4F: [i | f | g | o]; half1 = [i|f], half2 = [g|o].
    # Pre-scale i/f/o columns by 0.2 (hard_sigmoid slope).
    wstack = np.zeros((128, 18 * 128), F32)
    for k, (dy, dx) in enumerate(TAPS):
        ky, kx = dy + 1, dx + 1
        wstack[0:64, k * 128:(k + 1) * 128] = Wx[ky, kx, :, 0:128] * 0.2
        wstack[64:128, k * 128:(k + 1) * 128] = Wh[ky, kx, :, 0:128] * 0.2
        h2 = np.concatenate([Wx[ky, kx, :, 128:192],
                             Wx[ky, kx, :, 192:256] * 0.2], axis=1)
        wstack[0:64, 1152 + k * 128:1152 + (k + 1) * 128] = h2
        h2h = np.concatenate([Wh[ky, kx, :, 128:192],
                              Wh[ky, kx, :, 192:256] * 0.2], axis=1)
        wstack[64:128, 1152 + k * 128:1152 + (k + 1) * 128] = h2h
    wstack = wstack.astype(F16)

    b_if = (0.2 * b[0:128] + 0.5).reshape(128, 1).astype(F32)
    bg = b[128:192].reshape(64, 1).astype(F32)
    bo1 = 0.2 * b[192:256] + 0.5
    b_o = np.concatenate([bo1, bo1]).reshape(128, 1).astype(F32)
    inv = (np.asarray(gamma, F32) /
           np.sqrt(np.asarray(moving_var, F32) + 1e-3))
    bnb1 = (np.asarray(beta, F32) - np.asarray(moving_mean, F32) * inv)
    # duplicated into both partition halves: BN reads h at partitions 64-127
    bns = np.concatenate([inv, inv]).reshape(128, 1).astype(F32)
    bnb = np.concatenate([bnb1, bnb1]).reshape(128, 1).astype(F32)

    in_maps = []
    for core in range(8):
        bidx, sl = core // 4, core % 4
        r0 = 16 * sl
        glo, ghi = max(0, r0 - 5), min(64, r0 + 21)
        i0 = glo - (r0 - 5)
        xpad = np.zeros((T, NR, WP, C), F32)
        xpad[:, i0:i0 + (ghi - glo), 1:65, :] = x[bidx, :, glo:ghi, :, :]
        xT = np.ascontiguousarray(
            xpad.transpose(0, 3, 1, 2).reshape(T, C, NCOL)).astype(F16)
        m = np.zeros((NR, WP), F32)
        for i in range(NR):
            if 0 <= (r0 - 5 + i) < 64:
                m[i, 1:65] = 1.0
        mask = np.broadcast_to(
            m.reshape(1, NCOL), (64, NCOL)).astype(F16).copy()
        # select masks: sel[:, 0:4] = up (choose group-rank sl-1),
        # sel[:, 4:8] = down (choose group-rank sl+1)
        sel = np.zeros((64, 8), F32)
        if sl > 0:
            sel[:, sl - 1] = 1.0
        if sl < 3:
            sel[:, 4 + sl + 1] = 1.0
        in_maps.append({
            "xT": xT, "w": wstack, "mask": mask, "b_if": b_if,
            "bg": bg, "b_o": b_o, "bns": bns, "bnb": bnb, "sel": sel,
        })
    return in_maps


def kernel(x, Wx, Wh, b, gamma, beta, moving_mean, moving_var):
    global _PROG
    if _PROG is None:
        _PROG = _build()
    in_maps = _prep_inputs(x, Wx, Wh, b, gamma, beta, moving_mean, moving_var)
    res = run_bass_kernel_spmd(_PROG, in_maps, core_ids=list(range(8)))
    out = np.empty((2, T, 64, W, F), F32)
    for core in range(8):
        bidx, sl = core // 4, core % 4
        oc = res.results[core]["out"].astype(F32).reshape(
            T, F, 16, WP)[:, :, :, 1:65]
        out[bidx, :, 16 * sl:16 * sl + 16] = oc.transpose(0, 2, 3, 1)
    return out


# revision 58
# speedup vs baseline: 1.4758x; 1.0185x over previous
"""ConvLSTM2D (Keras gate order, hard_sigmoid) + inference BatchNorm on 8
Trainium2 NeuronCores.

Sharding: batch (2) x H-slabs (4) -> 8 cores. The T=16 recurrence is split
into 4 blocks of 4 steps. Within a block each core computes a shrinking halo
(depth 4); at block boundaries (t=4,8,12) cores exchange 4 boundary rows of
BOTH states (h and c) with their slab neighbors via one AllGather per epoch
(replica groups = the two 4-slab groups). This cuts redundant conv work from
1.94x (17-deep shrinking halo) to 1.19x while keeping the program uniform
SPMD: out-of-image edges are data (zero-padded x, h-mask, zero select masks).

Layout: channels-on-partitions. zin (128 x 1716 fp16) holds x_t on
partitions 0-63 and h_{t-1} on 64-127; 26 rows of 66 cols (64 + guard col
each side); a 3x3 conv tap (dy,dx) is the col offset dy*66+dx. One matmul
contracts x AND h channels at once (lhsT = [Wx_tap; Wh_tap]) so
z = conv(x,Wx)+conv(h,Wh) is 9 taps x 2 gate-halves of accumulating matmuls
per chunk (full 128x128 PE). Gate-i/f/o weight columns are pre-scaled by 0.2
so hard_sigmoid is Relu(psum + (0.2b+0.5)) then min(.,1). i|f activations are
fused into single 128-partition ops; pointwise math runs in fp16 on DVE
(2x rate) with c kept in fp32; BN is one DVE tensor_scalar (scale+bias) into
an fp16 output.
"""
import math
import numpy as np

import concourse.bass as bass
import concourse.mybir as mybir
import concourse.tile as tile
from concourse.bass_utils import run_bass_kernel_spmd

F16 = np.float16
F32 = np.float32

T, F, C, W = 16, 64, 64, 64
L = 4              # block length (steps between exchanges)
HALO = 4           # halo depth = L
NR = 16 + 2 * HALO + 2   # 26 buffer rows: [r0-5, r1+5)
WP = W + 2         # 66
NCOL = NR * WP     # 1716
OWN_LO, OWN_HI = 5 * WP, 21 * WP     # own 16 rows: buffer rows [5, 21)
HB = HALO * WP     # 264: one halo bundle (4 rows)
TAPS = [(dy, dx) for dy in (-1, 0, 1) for dx in (-1, 0, 1)]

TRACE_SIM = False
_PROG = None
_LAST_TC = None

# ---------------------------------------------------------------------------
# Workaround: this walrus build accepts at most ONE sync wait per
# instruction; Tile attaches several. Hoist extras onto same-engine NOPs
# inserted right before the instruction (per-engine order preserved).
_MAX_WAITS = 1


def _split_multi_waits(nc):
    for fn in nc.m.functions:
        for bb in fn.blocks:
            lst = bb.instructions
            out, changed = [], False
            for ins in lst:
                si = ins.sync_info
                if si is not None and len(si.on_wait) > _MAX_WAITS:
                    waits = list(si.on_wait)
                    extra, keep = waits[:-_MAX_WAITS], waits[-_MAX_WAITS:]
                    for j, w in enumerate(extra):
                        nop = mybir.InstNoOp(
                            name=f"{ins.name}.sw{j}", ins=[], outs=[],
                            text_hint="split_wait", bass_nofuse=True)
                        nop.engine = ins.engine
                        nop.sync_info = mybir.SyncInfo(on_wait=[w], on_update=[])
                        out.append(nop)
                    ins.sync_info = mybir.SyncInfo(
                        on_wait=keep, on_update=list(si.on_update))
                    changed = True
                out.append(ins)
            if changed:
                try:
                    bb.instructions = out
                except Exception:
                    lst.clear()
                    lst.extend(out)


def _chunks(c0, c1, maxn=512):
    Ln = c1 - c0
    n = max(1, math.ceil(Ln / maxn))
    base, rem = divmod(Ln, n)
    sizes = [base + (1 if i < rem else 0) for i in range(n)]
    out, p = [], c0
    for s in sizes:
        out.append((p, s))
        p += s
    return out


def _build():
    nc = bass.Bass(target_bir_lowering=False)
    f32, f16 = mybir.dt.float32, mybir.dt.float16

    xT_d = nc.dram_tensor("xT", [T, C, NCOL], f16, kind="ExternalInput")
    w_d = nc.dram_tensor("w", [128, 18 * 128], f16, kind="ExternalInput")
    mask_d = nc.dram_tensor("mask", [F, NCOL], f16, kind="ExternalInput")
    bif_d = nc.dram_tensor("b_if", [128, 1], f32, kind="ExternalInput")
    bg_d = nc.dram_tensor("bg", [F, 1], f32, kind="ExternalInput")
    bo_d = nc.dram_tensor("b_o", [128, 1], f32, kind="ExternalInput")
    bns_d = nc.dram_tensor("bns", [128, 1], f32, kind="ExternalInput")
    bnb_d = nc.dram_tensor("bnb", [128, 1], f32, kind="ExternalInput")
    sel_d = nc.dram_tensor("sel", [F, 8], f32, kind="ExternalInput")
    out_d = nc.dram_tensor("out", [T, F, 16 * WP], f16, kind="ExternalOutput")

    Relu = mybir.ActivationFunctionType.Relu
    Tanh = mybir.ActivationFunctionType.Tanh
    MULT = mybir.AluOpType.mult
    ADD = mybir.AluOpType.add

    with tile.TileContext(nc, trace_sim=TRACE_SIM) as tc:
        with (
            tc.tile_pool(name="const", bufs=1) as cpool,
            tc.tile_pool(name="state", bufs=1) as spool,
            tc.tile_pool(name="work", bufs=4) as wpool,
            tc.tile_pool(name="ostage", bufs=2) as opool,
            tc.tile_pool(name="agx", bufs=2) as agpool,
            tc.tile_pool(name="dram", bufs=2, space="DRAM") as dpool,
            tc.psum_pool(name="ps", bufs=4) as pspool,
        ):
            w_sb = cpool.tile([128, 18 * 128], f16)
            mask_sb = cpool.tile([F, NCOL], f16)
            bif_sb = cpool.tile([128, 1], f32)
            bg_sb = cpool.tile([F, 1], f32)
            bo_sb = cpool.tile([128, 1], f32)
            bns_sb = cpool.tile([128, 1], f32)
            bnb_sb = cpool.tile([128, 1], f32)
            sel_sb = cpool.tile([F, 8], f32)
            # x on the SP queue (feed PE first); weights + small consts on
            # the ACT queue so they don't head-of-line-block the x DMAs
            # warm the ACT function table first: its engine time overlaps
            # the const DMAs' sequencer work on the same queue
            warm = cpool.tile([64, 1], f32)
            nc.vector.memset(warm[:], 0.0)
            nc.scalar.activation(warm[:], warm[:],
                                 mybir.ActivationFunctionType.Tanh)
            # ACT queue: only what the first gate chain needs, w first
            nc.scalar.dma_start(w_sb[:, 0:1152], w_d[:, 0:1152])
            nc.scalar.dma_start(w_sb[:, 1152:2304], w_d[:, 1152:2304])
            nc.scalar.dma_start(bif_sb[:], bif_d[:])
            nc.scalar.dma_start(mask_sb[:], mask_d[:])
            # Pool queue is idle early (BNs start late): other consts here
            nc.gpsimd.dma_start(bg_sb[:], bg_d[:])
            nc.gpsimd.dma_start(bo_sb[:], bo_d[:])

            # 4 rotating buffers: step t reads x_t+h_{t-1} from zin[(t-1)%4]
            # and writes h_t into zin[t%4]. 4 (not 2) so a step's x DMA never
            # lands in a buffer whose x a not-yet-emitted chunk still reads.
            zin = [spool.tile([128, NCOL], f16, name=f"zin{i}", tag=f"zin{i}")
                   for i in range(4)]
            c_sb = spool.tile([F, NCOL], f32, tag="cstate")
            # h-halves and x guard rows must start zero (NaN garbage would
            # survive the h mask multiply via guard-col taps); x interior is
            # fully overwritten by the per-step DMA.
            nc.gpsimd.memset(c_sb[:], 0.0)   # first: feeds t=1's f*c early
            for i in range(4):
                eng = nc.vector if i % 2 == 0 else nc.gpsimd
                eng.memset(zin[i][64:128, :], 0.0)
                eng.memset(zin[i][0:64, 0:WP], 0.0)
                eng.memset(zin[i][0:64, NCOL - WP:NCOL], 0.0)
            # BN/select consts after the state memsets (needed later)
            nc.gpsimd.dma_start(bns_sb[:], bns_d[:])
            nc.gpsimd.dma_start(bnb_sb[:], bnb_d[:])
            nc.gpsimd.dma_start(sel_sb[:], sel_d[:])

            def emit_chunk_head(t, p0, n):
                cur = zin[(t - 1) % 4]
                ps_if = pspool.tile([128, n], f32, tag="psif")
                ps_go = pspool.tile([128, n], f32, tag="psgo")
                for k, (dy, dx) in enumerate(TAPS):
                    off = p0 + dy * WP + dx
                    nc.tensor.matmul(
                        ps_if[:], w_sb[:, k * 128:(k + 1) * 128],
                        cur[:, off:off + n], start=(k == 0), stop=(k == 8))
                for k, (dy, dx) in enumerate(TAPS):
                    off = p0 + dy * WP + dx
                    nc.tensor.matmul(
                        ps_go[:], w_sb[:, 1152 + k * 128:1152 + (k + 1) * 128],
                        cur[:, off:off + n], start=(k == 0), stop=(k == 8))

                sig_if = wpool.tile([128, n], f16, tag="sig_if")
                f_low = wpool.tile([F, n], f16, tag="f_low")
                tanh_g = wpool.tile([F, n], f16, tag="tanh_g")
                sig_o = wpool.tile([F, n], f16, tag="sig_o")
                t1 = wpool.tile([F, n], f16, tag="t1")
                t2 = wpool.tile([F, n], f32, tag="t2")

                # i|f fused: weights pre-scaled by 0.2 -> Relu(ps + 0.2b+0.5)
                nc.scalar.activation(sig_if[:], ps_if[:], Relu,
                                     bias=bif_sb[:, 0:1])
                # DVE TensorTensor needs equal input base partitions: min the
                # f-half down to partitions 0-63 while clipping it
                nc.vector.tensor_scalar_min(sig_if[0:64, :],
                                            sig_if[0:64, :], 1.0)
                nc.vector.tensor_scalar_min(f_low[:], sig_if[64:128, :], 1.0)
                nc.scalar.activation(tanh_g[:], ps_go[0:64, :], Tanh,
                                     bias=bg_sb[:, 0:1])
                nc.scalar.activation(sig_o[:], ps_go[64:128, :], Relu,
                                     bias=bo_sb[64:128, 0:1])
                # min(o,1) and the h edge-mask fused: mask is 1 in-image (so
                # min(o, 1)) and 0 outside (so o -> 0 -> h = 0)
                nc.vector.tensor_tensor(sig_o[:], sig_o[:],
                                        mask_sb[:, p0:p0 + n],
                                        mybir.AluOpType.min)
                nc.vector.tensor_mul(t1[:], sig_if[0:64, :], tanh_g[:])
                nc.vector.tensor_mul(t2[:], f_low[:], c_sb[:, p0:p0 + n])
                nc.vector.tensor_add(c_sb[:, p0:p0 + n], t1[:], t2[:])
                return sig_o

            def emit_chunk_tail(t, stage, p0, n, sig_o):
                # second pass: tanh(c) and h, emitted after every chunk's
                # gate work so a blocked tanh_c can't head-of-line-block the
                # next chunk's ready activations in the ACT FIFO
                nxt = zin[t % 4]
                tanh_c = wpool.tile([F, n], f16, tag="tanh_c")
                nc.scalar.activation(tanh_c[:], c_sb[:, p0:p0 + n], Tanh)
                nc.vector.tensor_mul(nxt[64:128, p0:p0 + n],
                                     sig_o[:], tanh_c[:])
                lo, hi = max(p0, OWN_LO), min(p0 + n, OWN_HI)
                if lo < hi:
                    # BN on Pool: off the critical recurrence path, so the
                    # collective blocking Pool only delays the output stage
                    nc.gpsimd.tensor_scalar(
                        stage[:, lo - OWN_LO:hi - OWN_LO],
                        nxt[64:128, lo:hi],
                        bns_sb[64:128, 0:1], bnb_sb[64:128, 0:1], MULT, ADD)

            def emit_step(t, stage, chunk_list):
                heads = [(p0, n, emit_chunk_head(t, p0, n))
                         for p0, n in chunk_list]
                for p0, n, sig_o in heads:
                    emit_chunk_tail(t, stage, p0, n, sig_o)

            def emit_exchange_send(t):
                # ---- exchange epoch: ship h,c boundary rows (4 each) ----
                nxt = zin[t % 4]
                cbf = agpool.tile([F, 2 * HB], f16, tag="cbf")
                nc.vector.tensor_scalar_mul(
                    cbf[:, 0:HB], c_sb[:, OWN_LO:OWN_LO + HB], 1.0)
                nc.vector.tensor_scalar_mul(
                    cbf[:, HB:2 * HB], c_sb[:, OWN_HI - HB:OWN_HI], 1.0)
                agin = dpool.tile([F, 4 * HB], f16, tag="agin")
                agout = dpool.tile([4 * F, 4 * HB], f16, tag="agout")
                nc.sync.dma_start(agin[:, 0:HB],
                                  nxt[64:128, OWN_LO:OWN_LO + HB])
                nc.sync.dma_start(agin[:, HB:2 * HB],
                                  nxt[64:128, OWN_HI - HB:OWN_HI])
                nc.sync.dma_start(agin[:, 2 * HB:4 * HB], cbf[:])
                nc.gpsimd.collective_compute(
                    "AllGather", mybir.AluOpType.bypass,
                    ins=[agin.opt()], outs=[agout.opt()],
                    replica_groups=[[0, 1, 2, 3], [4, 5, 6, 7]],
                )
                return agout

            def emit_exchange_recv(t, agout):
                nxt = zin[t % 4]
                # h and c halves land via parallel DMA queues (SP / ACT)
                agsb_h = agpool.tile([F, 8 * HB], f16, tag="agsb_h")
                agsb_c = agpool.tile([F, 8 * HB], f16, tag="agsb_c")
                nc.sync.dma_start(
                    agsb_h[:].rearrange("p (j c) -> p j c", j=4),
                    agout[:, 0:2 * HB].rearrange("(j p) c -> p j c", j=4))
                nc.scalar.dma_start(
                    agsb_c[:].rearrange("p (j c) -> p j c", j=4),
                    agout[:, 2 * HB:4 * HB].rearrange("(j p) c -> p j c",
                                                      j=4))
                # select: up-halo <- Sum_j bundle_j * sel_up_j, etc.
                # bundles within agsb_*[j]: [top | bottom]
                tsel = [agpool.tile([F, HB], f16, name=f"tsel{i}",
                                    tag=f"tsel{i}") for i in range(4)]
                csel = [agpool.tile([F, HB], f32, name=f"csel{i}",
                                    tag=f"csel{i}") for i in range(4)]

                def select(src, dst_ap, boff, scol, tiles, eng):
                    # dst = sum_j src[:, j*2HB+boff : +HB] * sel[:, scol+j]
                    for j in range(4):
                        eng.tensor_scalar_mul(
                            tiles[j][:],
                            src[:, j * 2 * HB + boff:j * 2 * HB + boff + HB],
                            sel_sb[:, scol + j:scol + j + 1])
                    eng.tensor_add(tiles[0][:], tiles[0][:], tiles[1][:])
                    eng.tensor_add(tiles[2][:], tiles[2][:], tiles[3][:])
                    eng.tensor_add(dst_ap, tiles[0][:], tiles[2][:])

                # h up-halo rows [1,5): from up-neighbor's h_bot (boff=HB)
                select(agsb_h, nxt[64:128, WP:WP + HB], HB, 0, tsel,
                       nc.vector)
                # h down-halo rows [21,25): from down-neighbor's h_top
                select(agsb_h, nxt[64:128, OWN_HI:OWN_HI + HB], 0, 4, tsel,
                       nc.vector)
                # c halos (f32 state) on gpsimd to offload DVE
                select(agsb_c, c_sb[:, WP:WP + HB], HB, 0, csel, nc.gpsimd)
                select(agsb_c, c_sb[:, OWN_HI:OWN_HI + HB], 0, 4, csel,
                       nc.gpsimd)

            # Nested halo-independent interiors: I_j needs only I_{j-1}'s h
            # (67-col tap margin), so all interiors of a block are runnable
            # while the preceding epoch's AllGather is still in flight.
            INT = [(397 + 67 * j, 1319 - 67 * j) for j in range(4)]

            pending = None           # (epoch step, agout) awaiting receive
            deferred_outs = []       # (step, stage tile) from previous block
            for b in range(4):
                stages = {}
                for j in range(4):
                    t = 4 * b + j + 1
                    cur = zin[(t - 1) % 4]
                    # full interior rows every step: keeps every buffer's
                    # x-half fresh (no stale-x reads from 4 steps ago)
                    if t == 1:
                        # first chunk's tap range first: unblocks MM #1 early
                        nc.sync.dma_start(cur[0:64, 330:958],
                                          xT_d[t - 1, :, 330:958])
                        nc.sync.dma_start(cur[0:64, WP:330],
                                          xT_d[t - 1, :, WP:330])
                        nc.sync.dma_start(cur[0:64, 958:NCOL - WP],
                                          xT_d[t - 1, :, 958:NCOL - WP])
                    else:
                        nc.sync.dma_start(cur[0:64, WP:NCOL - WP],
                                          xT_d[t - 1, :, WP:NCOL - WP])
                    stages[t] = opool.tile([F, 16 * WP], f16, name=f"stage{t}",
                                           tag=f"stage{j}")
                # phase A: interiors (halo-independent), step order — these
                # overlap the in-flight AllGather from the previous block.
                # Split nested (67-col stagger) so step j+1's first chunk
                # depends only on step j's first chunk's pointwise chain.
                # 3 cones per interior: shrinking left cone + two
                # constant-width sliding cones; cone k of step j+1 depends
                # only on cones <= k of step j, so each transition chain is
                # covered by the later cones' matmuls
                for j in range(4):
                    t = 4 * b + j + 1
                    i0, i1 = INT[j]
                    q1, q2 = 891 - 67 * j, 1105 - 67 * j
                    emit_step(t, stages[t],
                              [(i0, q1 - i0), (q1, q2 - q1), (q2, i1 - q2)])
                # receive the previous epoch's halos only now, so no engine
                # queue stalls on the collective before phase A is dispatched
                if pending is not None:
                    emit_exchange_recv(*pending)
                    pending = None
                # previous block's output DMAs: deferred to here so their
                # BN deps are long done (they'd stall whichever queue they
                # sat in otherwise); SP is past its x DMAs by now
                for t_prev, stg in deferred_outs:
                    nc.sync.dma_start(out_d[t_prev - 1], stg[:])
                deferred_outs = []
                # phase B: boundary chunks, step order; epoch step's boundary
                # feeds the exchange
                for j in range(4):
                    t = 4 * b + j + 1
                    s = 3 - j
                    i0, i1 = INT[j]
                    c0, c1 = (5 - s) * WP, (21 + s) * WP
                    if t == T:
                        # final step: halve the right chunk so the kernel's
                        # tail chain (gates -> BN -> out DMA) is shorter
                        m = (i1 + c1) // 2
                        emit_step(t, stages[t],
                                  [(c0, i0 - c0), (i1, m - i1), (m, c1 - m)])
                    else:
                        emit_step(t, stages[t],
                                  [(c0, i0 - c0), (i1, c1 - i1)])
                for j in range(4):
                    t = 4 * b + j + 1
                    if t == T:
                        # last block drains on the (idle-by-now) SP queue,
                        # t=16 split in halves so the first half ships while
                        # the last BN finishes
                        nc.sync.dma_start(out_d[t - 1, :, 0:788],
                                          stages[t][:, 0:788])
                        nc.sync.dma_start(out_d[t - 1, :, 788:16 * WP],
                                          stages[t][:, 788:16 * WP])
                    elif b == 3:
                        nc.sync.dma_start(out_d[t - 1], stages[t][:])
                    else:
                        deferred_outs.append((t, stages[t]))
                if b < 3:
                    te = 4 * b + 4
                    pending = (te, emit_exchange_send(te))

        global _LAST_TC
        _LAST_TC = tc
    _split_multi_waits(nc)
    return nc


def _prep_inputs(x, Wx, Wh, b, gamma, beta, moving_mean, moving_var):
    x = np.asarray(x, F32)
    Wx = np.asarray(Wx, F32)
    Wh = np.asarray(Wh, F32)
    b = np.asarray(b, F32)
    # gate order along 4F: [i | f | g | o]; half1 = [i|f], half2 = [g|o].
    # Pre-scale i/f/o columns by 0.2 (hard_sigmoid slope).
    wstack = np.zeros((128, 18 * 128), F32)
    for k, (dy, dx) in enumerate(TAPS):
        ky, kx = dy + 1, dx + 1
        wstack[0:64, k * 128:(k + 1) * 128] = Wx[ky, kx, :, 0:128] * 0.2
        wstack[64:128, k * 128:(k + 1) * 128] = Wh[ky, kx, :, 0:128] * 0.2
        h2 = np.concatenate([Wx[ky, kx, :, 128:192],
                             Wx[ky, kx, :, 192:256] * 0.2], axis=1)
        wstack[0:64, 1152 + k * 128:1152 + (k + 1) * 128] = h2
        h2h = np.concatenate([Wh[ky, kx, :, 128:192],
                              Wh[ky, kx, :, 192:256] * 0.2], axis=1)
        wstack[64:128, 1152 + k * 128:1152 + (k + 1) * 128] = h2h
    wstack = wstack.astype(F16)

    b_if = (0.2 * b[0:128] + 0.5).reshape(128, 1).astype(F32)
    bg = b[128:192].reshape(64, 1).astype(F32)
    bo1 = 0.2 * b[192:256] + 0.5
    b_o = np.concatenate([bo1, bo1]).reshape(128, 1).astype(F32)
    inv = (np.asarray(gamma, F32) /
           np.sqrt(np.asarray(moving_var, F32) + 1e-3))
    bnb1 = (np.asarray(beta, F32) - np.asarray(moving_mean, F32) * inv)
    # duplicated into both partition halves: BN reads h at partitions 64-127
    bns = np.concatenate([inv, inv]).reshape(128, 1).astype(F32)
    bnb = np.concatenate([bnb1, bnb1]).reshape(128, 1).astype(F32)

    in_maps = []
    for core in range(8):
        bidx, sl = core // 4, core % 4
        r0 = 16 * sl
        glo, ghi = max(0, r0 - 5), min(64, r0 + 21)
        i0 = glo - (r0 - 5)
        xpad = np.zeros((T, NR, WP, C), F32)
        xpad[:, i0:i0 + (ghi - glo), 1:65, :] = x[bidx, :, glo:ghi, :, :]
        xT = np.ascontiguousarray(
            xpad.transpose(0, 3, 1, 2).reshape(T, C, NCOL)).astype(F16)
        m = np.zeros((NR, WP), F32)
        for i in range(NR):
            if 0 <= (r0 - 5 + i) < 64:
                m[i, 1:65] = 1.0
        mask = np.broadcast_to(
            m.reshape(1, NCOL), (64, NCOL)).astype(F16).copy()
        # select masks: sel[:, 0:4] = up (choose group-rank sl-1),
        # sel[:, 4:8] = down (choose group-rank sl+1)
        sel = np.zeros((64, 8), F32)
        if sl > 0:
            sel[:, sl - 1] = 1.0
        if sl < 3:
            sel[:, 4 + sl + 1] = 1.0
        in_maps.append({
            "xT": xT, "w": wstack, "mask": mask, "b_if": b_if,
            "bg": bg, "b_o": b_o, "bns": bns, "bnb": bnb, "sel": sel,
        })
    return in_maps


def kernel(x, Wx, Wh, b, gamma, beta, moving_mean, moving_var):
    global _PROG
    if _PROG is None:
        _PROG = _build()
    in_maps = _prep_inputs(x, Wx, Wh, b, gamma, beta, moving_mean, moving_var)
    res = run_bass_kernel_spmd(_PROG, in_maps, core_ids=list(range(8)))
    out = np.empty((2, T, 64, W, F), F32)
    for core in range(8):
        bidx, sl = core // 4, core % 4
        oc = res.results[core]["out"].astype(F32).reshape(
            T, F, 16, WP)[:, :, :, 1:65]
        out[bidx, :, 16 * sl:16 * sl + 16] = oc.transpose(0, 2, 3, 1)
    return out


# revision 59
# speedup vs baseline: 1.5364x; 1.0411x over previous
"""ConvLSTM2D (Keras gate order, hard_sigmoid) + inference BatchNorm on 8
Trainium2 NeuronCores.

Sharding: batch (2) x H-slabs (4) -> 8 cores. The T=16 recurrence is split
into 4 blocks of 4 steps. Within a block each core computes a shrinking halo
(depth 4); at block boundaries (t=4,8,12) cores exchange 4 boundary rows of
BOTH states (h and c) with their slab neighbors via one AllGather per epoch
(replica groups = the two 4-slab groups). This cuts redundant conv work from
1.94x (17-deep shrinking halo) to 1.19x while keeping the program uniform
SPMD: out-of-image edges are data (zero-padded x, h-mask, zero select masks).

Layout: channels-on-partitions. zin (128 x 1716 fp16) holds x_t on
partitions 0-63 and h_{t-1} on 64-127; 26 rows of 66 cols (64 + guard col
each side); a 3x3 conv tap (dy,dx) is the col offset dy*66+dx. One matmul
contracts x AND h channels at once (lhsT = [Wx_tap; Wh_tap]) so
z = conv(x,Wx)+conv(h,Wh) is 9 taps x 2 gate-halves of accumulating matmuls
per chunk (full 128x128 PE). Gate-i/f/o weight columns are pre-scaled by 0.2
so hard_sigmoid is Relu(psum + (0.2b+0.5)) then min(.,1). i|f activations are
fused into single 128-partition ops; pointwise math runs in fp16 on DVE
(2x rate) with c kept in fp32; BN is one DVE tensor_scalar (scale+bias) into
an fp16 output.
"""
import math
import numpy as np

import concourse.bass as bass
import concourse.mybir as mybir
import concourse.tile as tile
from concourse.bass_utils import run_bass_kernel_spmd

F16 = np.float16
F32 = np.float32

T, F, C, W = 16, 64, 64, 64
L = 4              # block length (steps between exchanges)
HALO = 4           # halo depth = L
NR = 16 + 2 * HALO + 2   # 26 buffer rows: [r0-5, r1+5)
WP = W + 2         # 66
NCOL = NR * WP     # 1716
OWN_LO, OWN_HI = 5 * WP, 21 * WP     # own 16 rows: buffer rows [5, 21)
HB = HALO * WP     # 264: one halo bundle (4 rows)
TAPS = [(dy, dx) for dy in (-1, 0, 1) for dx in (-1, 0, 1)]

TRACE_SIM = False
_PROG = None
_LAST_TC = None

# ---------------------------------------------------------------------------
# Workaround: this walrus build accepts at most ONE sync wait per
# instruction; Tile attaches several. Hoist extras onto same-engine NOPs
# inserted right before the instruction (per-engine order preserved).
_MAX_WAITS = 1


def _split_multi_waits(nc):
    for fn in nc.m.functions:
        for bb in fn.blocks:
            lst = bb.instructions
            out, changed = [], False
            for ins in lst:
                si = ins.sync_info
                if si is not None and len(si.on_wait) > _MAX_WAITS:
                    waits = list(si.on_wait)
                    extra, keep = waits[:-_MAX_WAITS], waits[-_MAX_WAITS:]
                    for j, w in enumerate(extra):
                        nop = mybir.InstNoOp(
                            name=f"{ins.name}.sw{j}", ins=[], outs=[],
                            text_hint="split_wait", bass_nofuse=True)
                        nop.engine = ins.engine
                        nop.sync_info = mybir.SyncInfo(on_wait=[w], on_update=[])
                        out.append(nop)
                    ins.sync_info = mybir.SyncInfo(
                        on_wait=keep, on_update=list(si.on_update))
                    changed = True
                out.append(ins)
            if changed:
                try:
                    bb.instructions = out
                except Exception:
                    lst.clear()
                    lst.extend(out)


def _chunks(c0, c1, maxn=512):
    Ln = c1 - c0
    n = max(1, math.ceil(Ln / maxn))
    base, rem = divmod(Ln, n)
    sizes = [base + (1 if i < rem else 0) for i in range(n)]
    out, p = [], c0
    for s in sizes:
        out.append((p, s))
        p += s
    return out


def _build():
    nc = bass.Bass(target_bir_lowering=False)
    f32, f16 = mybir.dt.float32, mybir.dt.float16

    xT_d = nc.dram_tensor("xT", [T, C, NCOL], f16, kind="ExternalInput")
    w_d = nc.dram_tensor("w", [128, 18 * 128], f16, kind="ExternalInput")
    mask_d = nc.dram_tensor("mask", [F, NCOL], f16, kind="ExternalInput")
    bif_d = nc.dram_tensor("b_if", [128, 1], f32, kind="ExternalInput")
    bg_d = nc.dram_tensor("bg", [F, 1], f32, kind="ExternalInput")
    bo_d = nc.dram_tensor("b_o", [128, 1], f32, kind="ExternalInput")
    bns_d = nc.dram_tensor("bns", [128, 1], f32, kind="ExternalInput")
    bnb_d = nc.dram_tensor("bnb", [128, 1], f32, kind="ExternalInput")
    sel_d = nc.dram_tensor("sel", [F, 8], f32, kind="ExternalInput")
    out_d = nc.dram_tensor("out", [T, F, 16 * WP], f16, kind="ExternalOutput")

    Relu = mybir.ActivationFunctionType.Relu
    Tanh = mybir.ActivationFunctionType.Tanh
    MULT = mybir.AluOpType.mult
    ADD = mybir.AluOpType.add

    with tile.TileContext(nc, trace_sim=TRACE_SIM) as tc:
        with (
            tc.tile_pool(name="const", bufs=1) as cpool,
            tc.tile_pool(name="state", bufs=1) as spool,
            tc.tile_pool(name="work", bufs=4) as wpool,
            tc.tile_pool(name="ostage", bufs=2) as opool,
            tc.tile_pool(name="agx", bufs=2) as agpool,
            tc.tile_pool(name="dram", bufs=2, space="DRAM") as dpool,
            tc.psum_pool(name="ps", bufs=4) as pspool,
        ):
            w_sb = cpool.tile([128, 18 * 128], f16)
            mask_sb = cpool.tile([F, NCOL], f16)
            bif_sb = cpool.tile([128, 1], f32)
            bg_sb = cpool.tile([F, 1], f32)
            bo_sb = cpool.tile([128, 1], f32)
            bns_sb = cpool.tile([128, 1], f32)
            bnb_sb = cpool.tile([128, 1], f32)
            sel_sb = cpool.tile([F, 8], f32)
            # x on the SP queue (feed PE first); weights + small consts on
            # the ACT queue so they don't head-of-line-block the x DMAs
            # warm the ACT function table first: its engine time overlaps
            # the const DMAs' sequencer work on the same queue
            warm = cpool.tile([64, 1], f32)
            nc.vector.memset(warm[:], 0.0)
            nc.scalar.activation(warm[:], warm[:],
                                 mybir.ActivationFunctionType.Tanh)
            # ACT queue: only what the first gate chain needs, w first
            nc.scalar.dma_start(w_sb[:, 0:1152], w_d[:, 0:1152])
            nc.scalar.dma_start(w_sb[:, 1152:2304], w_d[:, 1152:2304])
            nc.scalar.dma_start(bif_sb[:], bif_d[:])
            nc.scalar.dma_start(mask_sb[:], mask_d[:])
            # Pool queue is idle early (BNs start late): other consts here
            nc.gpsimd.dma_start(bg_sb[:], bg_d[:])
            nc.gpsimd.dma_start(bo_sb[:], bo_d[:])

            # 4 rotating buffers: step t reads x_t+h_{t-1} from zin[(t-1)%4]
            # and writes h_t into zin[t%4]. 4 (not 2) so a step's x DMA never
            # lands in a buffer whose x a not-yet-emitted chunk still reads.
            zin = [spool.tile([128, NCOL], f16, name=f"zin{i}", tag=f"zin{i}")
                   for i in range(4)]
            c_sb = spool.tile([F, NCOL], f32, tag="cstate")
            # h-halves and x guard rows must start zero (NaN garbage would
            # survive the h mask multiply via guard-col taps); x interior is
            # fully overwritten by the per-step DMA.
            nc.gpsimd.memset(c_sb[:], 0.0)   # first: feeds t=1's f*c early
            for i in range(4):
                eng = nc.vector if i % 2 == 0 else nc.gpsimd
                eng.memset(zin[i][64:128, :], 0.0)
                eng.memset(zin[i][0:64, 0:WP], 0.0)
                eng.memset(zin[i][0:64, NCOL - WP:NCOL], 0.0)
            # BN/select consts after the state memsets (needed later)
            nc.gpsimd.dma_start(bns_sb[:], bns_d[:])
            nc.gpsimd.dma_start(bnb_sb[:], bnb_d[:])
            nc.gpsimd.dma_start(sel_sb[:], sel_d[:])

            def emit_chunk_head(t, p0, n):
                cur = zin[(t - 1) % 4]
                ps_if = pspool.tile([128, n], f32, tag="psif")
                ps_go = pspool.tile([128, n], f32, tag="psgo")
                for k, (dy, dx) in enumerate(TAPS):
                    off = p0 + dy * WP + dx
                    nc.tensor.matmul(
                        ps_if[:], w_sb[:, k * 128:(k + 1) * 128],
                        cur[:, off:off + n], start=(k == 0), stop=(k == 8))
                for k, (dy, dx) in enumerate(TAPS):
                    off = p0 + dy * WP + dx
                    nc.tensor.matmul(
                        ps_go[:], w_sb[:, 1152 + k * 128:1152 + (k + 1) * 128],
                        cur[:, off:off + n], start=(k == 0), stop=(k == 8))

                sig_if = wpool.tile([128, n], f16, tag="sig_if")
                f_low = wpool.tile([F, n], f16, tag="f_low")
                tanh_g = wpool.tile([F, n], f16, tag="tanh_g")
                sig_o = wpool.tile([F, n], f16, tag="sig_o")
                t1 = wpool.tile([F, n], f16, tag="t1")
                t2 = wpool.tile([F, n], f32, tag="t2")

                # i|f fused: weights pre-scaled by 0.2 -> Relu(ps + 0.2b+0.5)
                nc.scalar.activation(sig_if[:], ps_if[:], Relu,
                                     bias=bif_sb[:, 0:1])
                # DVE TensorTensor needs equal input base partitions: min the
                # f-half down to partitions 0-63 while clipping it
                nc.vector.tensor_scalar_min(sig_if[0:64, :],
                                            sig_if[0:64, :], 1.0)
                nc.vector.tensor_scalar_min(f_low[:], sig_if[64:128, :], 1.0)
                nc.scalar.activation(tanh_g[:], ps_go[0:64, :], Tanh,
                                     bias=bg_sb[:, 0:1])
                nc.scalar.activation(sig_o[:], ps_go[64:128, :], Relu,
                                     bias=bo_sb[64:128, 0:1])
                # min(o,1) and the h edge-mask fused: mask is 1 in-image (so
                # min(o, 1)) and 0 outside (so o -> 0 -> h = 0)
                nc.vector.tensor_tensor(sig_o[:], sig_o[:],
                                        mask_sb[:, p0:p0 + n],
                                        mybir.AluOpType.min)
                nc.vector.tensor_mul(t1[:], sig_if[0:64, :], tanh_g[:])
                nc.vector.tensor_mul(t2[:], f_low[:], c_sb[:, p0:p0 + n])
                nc.vector.tensor_add(c_sb[:, p0:p0 + n], t1[:], t2[:])
                return sig_o

            def emit_chunk_tail(t, stage, p0, n, sig_o):
                # second pass: tanh(c) and h, emitted after every chunk's
                # gate work so a blocked tanh_c can't head-of-line-block the
                # next chunk's ready activations in the ACT FIFO
                nxt = zin[t % 4]
                tanh_c = wpool.tile([F, n], f16, tag="tanh_c")
                nc.scalar.activation(tanh_c[:], c_sb[:, p0:p0 + n], Tanh)
                nc.vector.tensor_mul(nxt[64:128, p0:p0 + n],
                                     sig_o[:], tanh_c[:])
                lo, hi = max(p0, OWN_LO), min(p0 + n, OWN_HI)
                if lo < hi:
                    # BN on Pool: off the critical recurrence path, so the
                    # collective blocking Pool only delays the output stage
                    nc.gpsimd.tensor_scalar(
                        stage[:, lo - OWN_LO:hi - OWN_LO],
                        nxt[64:128, lo:hi],
                        bns_sb[64:128, 0:1], bnb_sb[64:128, 0:1], MULT, ADD)

            def emit_step(t, stage, chunk_list):
                heads = [(p0, n, emit_chunk_head(t, p0, n))
                         for p0, n in chunk_list]
                for p0, n, sig_o in heads:
                    emit_chunk_tail(t, stage, p0, n, sig_o)

            def emit_exchange_send(t):
                # ---- exchange epoch: ship h,c boundary rows (4 each) ----
                nxt = zin[t % 4]
                cbf = agpool.tile([F, 2 * HB], f16, tag="cbf")
                nc.vector.tensor_scalar_mul(
                    cbf[:, 0:HB], c_sb[:, OWN_LO:OWN_LO + HB], 1.0)
                nc.vector.tensor_scalar_mul(
                    cbf[:, HB:2 * HB], c_sb[:, OWN_HI - HB:OWN_HI], 1.0)
                agin = dpool.tile([F, 4 * HB], f16, tag="agin")
                agout = dpool.tile([4 * F, 4 * HB], f16, tag="agout")
                nc.sync.dma_start(agin[:, 0:HB],
                                  nxt[64:128, OWN_LO:OWN_LO + HB])
                nc.sync.dma_start(agin[:, HB:2 * HB],
                                  nxt[64:128, OWN_HI - HB:OWN_HI])
                nc.sync.dma_start(agin[:, 2 * HB:4 * HB], cbf[:])
                nc.gpsimd.collective_compute(
                    "AllGather", mybir.AluOpType.bypass,
                    ins=[agin.opt()], outs=[agout.opt()],
                    replica_groups=[[0, 1, 2, 3], [4, 5, 6, 7]],
                )
                return agout

            def emit_exchange_recv(t, agout):
                nxt = zin[t % 4]
                # 4 compact DMAs, critical halves first: the up-halo (from
                # neighbors' _bot bundles) gates B0-L, so h_bot/c_bot land
                # before h_top/c_top; h on SP, c on ACT (parallel queues)
                agsb = {}
                for key, col0, eng in (("hb", HB, nc.sync),
                                       ("cb", 3 * HB, nc.scalar),
                                       ("ht", 0, nc.sync),
                                       ("ct", 2 * HB, nc.scalar)):
                    tile_ = agpool.tile([F, 4 * HB], f16, name=f"agsb_{key}",
                                        tag=f"agsb_{key}")
                    eng.dma_start(
                        tile_[:].rearrange("p (j c) -> p j c", j=4),
                        agout[:, col0:col0 + HB].rearrange(
                            "(j p) c -> p j c", j=4))
                    agsb[key] = tile_
                tsel = [agpool.tile([F, HB], f16, name=f"tsel{i}",
                                    tag=f"tsel{i}") for i in range(4)]
                csel = [agpool.tile([F, HB], f32, name=f"csel{i}",
                                    tag=f"csel{i}") for i in range(4)]

                def select(src, dst_ap, scol, tiles, eng):
                    # dst = sum_j src[:, j*HB : +HB] * sel[:, scol+j]
                    for j in range(4):
                        eng.tensor_scalar_mul(
                            tiles[j][:], src[:, j * HB:(j + 1) * HB],
                            sel_sb[:, scol + j:scol + j + 1])
                    eng.tensor_add(tiles[0][:], tiles[0][:], tiles[1][:])
                    eng.tensor_add(tiles[2][:], tiles[2][:], tiles[3][:])
                    eng.tensor_add(dst_ap, tiles[0][:], tiles[2][:])

                # h up-halo rows [1,5): from up-neighbor's h_bot
                select(agsb["hb"], nxt[64:128, WP:WP + HB], 0, tsel,
                       nc.vector)
                # c up-halo on gpsimd (parallel with the DVE h select)
                select(agsb["cb"], c_sb[:, WP:WP + HB], 0, csel, nc.gpsimd)
                # down halos: from down-neighbor's _top bundles
                select(agsb["ht"], nxt[64:128, OWN_HI:OWN_HI + HB], 4, tsel,
                       nc.vector)
                select(agsb["ct"], c_sb[:, OWN_HI:OWN_HI + HB], 4, csel,
                       nc.gpsimd)

            # Nested halo-independent interiors: I_j needs only I_{j-1}'s h
            # (67-col tap margin), so all interiors of a block are runnable
            # while the preceding epoch's AllGather is still in flight.
            INT = [(397 + 67 * j, 1319 - 67 * j) for j in range(4)]

            pending = None           # (epoch step, agout) awaiting receive
            deferred_outs = []       # (step, stage tile) from previous block
            for b in range(4):
                stages = {}
                for j in range(4):
                    t = 4 * b + j + 1
                    cur = zin[(t - 1) % 4]
                    # full interior rows every step: keeps every buffer's
                    # x-half fresh (no stale-x reads from 4 steps ago)
                    if t == 1:
                        # first chunk's tap range first: unblocks MM #1 early
                        nc.sync.dma_start(cur[0:64, 330:958],
                                          xT_d[t - 1, :, 330:958])
                        nc.sync.dma_start(cur[0:64, WP:330],
                                          xT_d[t - 1, :, WP:330])
                        nc.sync.dma_start(cur[0:64, 958:NCOL - WP],
                                          xT_d[t - 1, :, 958:NCOL - WP])
                    else:
                        nc.sync.dma_start(cur[0:64, WP:NCOL - WP],
                                          xT_d[t - 1, :, WP:NCOL - WP])
                    stages[t] = opool.tile([F, 16 * WP], f16, name=f"stage{t}",
                                           tag=f"stage{j}")
                # phase A: interiors (halo-independent), step order — these
                # overlap the in-flight AllGather from the previous block.
                # Split nested (67-col stagger) so step j+1's first chunk
                # depends only on step j's first chunk's pointwise chain.
                # 3 cones per interior: shrinking left cone + two
                # constant-width sliding cones; cone k of step j+1 depends
                # only on cones <= k of step j, so each transition chain is
                # covered by the later cones' matmuls
                for j in range(4):
                    t = 4 * b + j + 1
                    i0, i1 = INT[j]
                    q1, q2 = 891 - 67 * j, 1105 - 67 * j
                    emit_step(t, stages[t],
                              [(i0, q1 - i0), (q1, q2 - q1), (q2, i1 - q2)])
                # receive the previous epoch's halos only now, so no engine
                # queue stalls on the collective before phase A is dispatched
                if pending is not None:
                    emit_exchange_recv(*pending)
                    pending = None
                # previous block's output DMAs: deferred to here so their
                # BN deps are long done (they'd stall whichever queue they
                # sat in otherwise); SP is past its x DMAs by now
                for t_prev, stg in deferred_outs:
                    nc.sync.dma_start(out_d[t_prev - 1], stg[:])
                deferred_outs = []
                # phase B: boundary chunks, step order; epoch step's boundary
                # feeds the exchange
                for j in range(4):
                    t = 4 * b + j + 1
                    s = 3 - j
                    i0, i1 = INT[j]
                    c0, c1 = (5 - s) * WP, (21 + s) * WP
                    if t == T:
                        # final step: halve the right chunk so the kernel's
                        # tail chain (gates -> BN -> out DMA) is shorter
                        m = (i1 + c1) // 2
                        emit_step(t, stages[t],
                                  [(c0, i0 - c0), (i1, m - i1), (m, c1 - m)])
                    else:
                        emit_step(t, stages[t],
                                  [(c0, i0 - c0), (i1, c1 - i1)])
                for j in range(4):
                    t = 4 * b + j + 1
                    if t == T:
                        # last block drains on the (idle-by-now) SP queue,
                        # t=16 split in halves so the first half ships while
                        # the last BN finishes
                        nc.sync.dma_start(out_d[t - 1, :, 0:788],
                                          stages[t][:, 0:788])
                        nc.sync.dma_start(out_d[t - 1, :, 788:16 * WP],
                                          stages[t][:, 788:16 * WP])
                    elif b == 3:
                        nc.sync.dma_start(out_d[t - 1], stages[t][:])
                    else:
                        deferred_outs.append((t, stages[t]))
                if b < 3:
                    te = 4 * b + 4
                    pending = (te, emit_exchange_send(te))

        global _LAST_TC
        _LAST_TC = tc
    _split_multi_waits(nc)
    return nc


def _prep_inputs(x, Wx, Wh, b, gamma, beta, moving_mean, moving_var):
    x = np.asarray(x, F32)
    Wx = np.asarray(Wx, F32)
    Wh = np.asarray(Wh, F32)
    b = np.asarray(b, F32)
    # gate order along 4F: [i | f | g | o]; half1 = [i|f], half2 = [g|o].
    # Pre-scale i/f/o columns by 0.2 (hard_sigmoid slope).
    wstack = np.zeros((128, 18 * 128), F32)
    for k, (dy, dx) in enumerate(TAPS):
        ky, kx = dy + 1, dx + 1
        wstack[0:64, k * 128:(k + 1) * 128] = Wx[ky, kx, :, 0:128] * 0.2
        wstack[64:128, k * 128:(k + 1) * 128] = Wh[ky, kx, :, 0:128] * 0.2
        h2 = np.concatenate([Wx[ky, kx, :, 128:192],
                             Wx[ky, kx, :, 192:256] * 0.2], axis=1)
        wstack[0:64, 1152 + k * 128:1152 + (k + 1) * 128] = h2
        h2h = np.concatenate([Wh[ky, kx, :, 128:192],
                              Wh[ky, kx, :, 192:256] * 0.2], axis=1)
        wstack[64:128, 1152 + k * 128:1152 + (k + 1) * 128] = h2h
    wstack = wstack.astype(F16)

    b_if = (0.2 * b[0:128] + 0.5).reshape(128, 1).astype(F32)
    bg = b[128:192].reshape(64, 1).astype(F32)
    bo1 = 0.2 * b[192:256] + 0.5
    b_o = np.concatenate([bo1, bo1]).reshape(128, 1).astype(F32)
    inv = (np.asarray(gamma, F32) /
           np.sqrt(np.asarray(moving_var, F32) + 1e-3))
    bnb1 = (np.asarray(beta, F32) - np.asarray(moving_mean, F32) * inv)
    # duplicated into both partition halves: BN reads h at partitions 64-127
    bns = np.concatenate([inv, inv]).reshape(128, 1).astype(F32)
    bnb = np.concatenate([bnb1, bnb1]).reshape(128, 1).astype(F32)

    in_maps = []
    for core in range(8):
        bidx, sl = core // 4, core % 4
        r0 = 16 * sl
        glo, ghi = max(0, r0 - 5), min(64, r0 + 21)
        i0 = glo - (r0 - 5)
        xpad = np.zeros((T, NR, WP, C), F32)
        xpad[:, i0:i0 + (ghi - glo), 1:65, :] = x[bidx, :, glo:ghi, :, :]
        xT = np.ascontiguousarray(
            xpad.transpose(0, 3, 1, 2).reshape(T, C, NCOL)).astype(F16)
        m = np.zeros((NR, WP), F32)
        for i in range(NR):
            if 0 <= (r0 - 5 + i) < 64:
                m[i, 1:65] = 1.0
        mask = np.broadcast_to(
            m.reshape(1, NCOL), (64, NCOL)).astype(F16).copy()
        # select masks: sel[:, 0:4] = up (choose group-rank sl-1),
        # sel[:, 4:8] = down (choose group-rank sl+1)
        sel = np.zeros((64, 8), F32)
        if sl > 0:
            sel[:, sl - 1] = 1.0
        if sl < 3:
            sel[:, 4 + sl + 1] = 1.0
        in_maps.append({
            "xT": xT, "w": wstack, "mask": mask, "b_if": b_if,
            "bg": bg, "b_o": b_o, "bns": bns, "bnb": bnb, "sel": sel,
        })
    return in_maps


def kernel(x, Wx, Wh, b, gamma, beta, moving_mean, moving_var):
    global _PROG
    if _PROG is None:
        _PROG = _build()
    in_maps = _prep_inputs(x, Wx, Wh, b, gamma, beta, moving_mean, moving_var)
    res = run_bass_kernel_spmd(_PROG, in_maps, core_ids=list(range(8)))
    out = np.empty((2, T, 64, W, F), F32)
    for core in range(8):
        bidx, sl = core // 4, core % 4
        oc = res.results[core]["out"].astype(F32).reshape(
            T, F, 16, WP)[:, :, :, 1:65]
        out[bidx, :, 16 * sl:16 * sl + 16] = oc.transpose(0, 2, 3, 1)
    return out


# revision 66
# speedup vs baseline: 1.5524x; 1.0104x over previous
"""ConvLSTM2D (Keras gate order, hard_sigmoid) + inference BatchNorm on 8
Trainium2 NeuronCores.

Sharding: batch (2) x H-slabs (4) -> 8 cores. The T=16 recurrence is split
into 4 blocks of 4 steps. Within a block each core computes a shrinking halo
(depth 4); at block boundaries (t=4,8,12) cores exchange 4 boundary rows of
BOTH states (h and c) with their slab neighbors via one AllGather per epoch
(replica groups = the two 4-slab groups). This cuts redundant conv work from
1.94x (17-deep shrinking halo) to 1.19x while keeping the program uniform
SPMD: out-of-image edges are data (zero-padded x, h-mask, zero select masks).

Layout: channels-on-partitions. zin (128 x 1716 fp16) holds x_t on
partitions 0-63 and h_{t-1} on 64-127; 26 rows of 66 cols (64 + guard col
each side); a 3x3 conv tap (dy,dx) is the col offset dy*66+dx. One matmul
contracts x AND h channels at once (lhsT = [Wx_tap; Wh_tap]) so
z = conv(x,Wx)+conv(h,Wh) is 9 taps x 2 gate-halves of accumulating matmuls
per chunk (full 128x128 PE). Gate-i/f/o weight columns are pre-scaled by 0.2
so hard_sigmoid is Relu(psum + (0.2b+0.5)) then min(.,1). i|f activations are
fused into single 128-partition ops; pointwise math runs in fp16 on DVE
(2x rate) with c kept in fp32; BN is one DVE tensor_scalar (scale+bias) into
an fp16 output.
"""
import math
import numpy as np

import concourse.bass as bass
import concourse.mybir as mybir
import concourse.tile as tile
from concourse.bass_utils import run_bass_kernel_spmd

F16 = np.float16
F32 = np.float32

T, F, C, W = 16, 64, 64, 64
L = 4              # block length (steps between exchanges)
HALO = 4           # halo depth = L
NR = 16 + 2 * HALO + 2   # 26 buffer rows: [r0-5, r1+5)
WP = W + 2         # 66
NCOL = NR * WP     # 1716
OWN_LO, OWN_HI = 5 * WP, 21 * WP     # own 16 rows: buffer rows [5, 21)
HB = HALO * WP     # 264: one halo bundle (4 rows)
TAPS = [(dy, dx) for dy in (-1, 0, 1) for dx in (-1, 0, 1)]

TRACE_SIM = False
_PROG = None
_LAST_TC = None

# ---------------------------------------------------------------------------
# Workaround: this walrus build accepts at most ONE sync wait per
# instruction; Tile attaches several. Hoist extras onto same-engine NOPs
# inserted right before the instruction (per-engine order preserved).
_MAX_WAITS = 1


def _split_multi_waits(nc):
    for fn in nc.m.functions:
        for bb in fn.blocks:
            lst = bb.instructions
            out, changed = [], False
            for ins in lst:
                si = ins.sync_info
                if si is not None and len(si.on_wait) > _MAX_WAITS:
                    waits = list(si.on_wait)
                    extra, keep = waits[:-_MAX_WAITS], waits[-_MAX_WAITS:]
                    for j, w in enumerate(extra):
                        nop = mybir.InstNoOp(
                            name=f"{ins.name}.sw{j}", ins=[], outs=[],
                            text_hint="split_wait", bass_nofuse=True)
                        nop.engine = ins.engine
                        nop.sync_info = mybir.SyncInfo(on_wait=[w], on_update=[])
                        out.append(nop)
                    ins.sync_info = mybir.SyncInfo(
                        on_wait=keep, on_update=list(si.on_update))
                    changed = True
                out.append(ins)
            if changed:
                try:
                    bb.instructions = out
                except Exception:
                    lst.clear()
                    lst.extend(out)


def _chunks(c0, c1, maxn=512):
    Ln = c1 - c0
    n = max(1, math.ceil(Ln / maxn))
    base, rem = divmod(Ln, n)
    sizes = [base + (1 if i < rem else 0) for i in range(n)]
    out, p = [], c0
    for s in sizes:
        out.append((p, s))
        p += s
    return out


def _build():
    nc = bass.Bass(target_bir_lowering=False)
    f32, f16 = mybir.dt.float32, mybir.dt.float16

    xT_d = nc.dram_tensor("xT", [T, C, NCOL], f16, kind="ExternalInput")
    w_d = nc.dram_tensor("w", [128, 18 * 128], f16, kind="ExternalInput")
    mask_d = nc.dram_tensor("mask", [F, NCOL], f16, kind="ExternalInput")
    bif_d = nc.dram_tensor("b_if", [128, 1], f32, kind="ExternalInput")
    bg_d = nc.dram_tensor("bg", [F, 1], f32, kind="ExternalInput")
    bo_d = nc.dram_tensor("b_o", [128, 1], f32, kind="ExternalInput")
    bns_d = nc.dram_tensor("bns", [128, 1], f32, kind="ExternalInput")
    bnb_d = nc.dram_tensor("bnb", [128, 1], f32, kind="ExternalInput")
    sel_d = nc.dram_tensor("sel", [F, 8], f32, kind="ExternalInput")
    out_d = nc.dram_tensor("out", [T, F, 16 * WP], f16, kind="ExternalOutput")

    Relu = mybir.ActivationFunctionType.Relu
    Tanh = mybir.ActivationFunctionType.Tanh
    MULT = mybir.AluOpType.mult
    ADD = mybir.AluOpType.add

    with tile.TileContext(nc, trace_sim=TRACE_SIM) as tc:
        with (
            tc.tile_pool(name="const", bufs=1) as cpool,
            tc.tile_pool(name="state", bufs=1) as spool,
            tc.tile_pool(name="work", bufs=4) as wpool,
            tc.tile_pool(name="ostage", bufs=2) as opool,
            tc.tile_pool(name="agx", bufs=2) as agpool,
            tc.tile_pool(name="dram", bufs=2, space="DRAM") as dpool,
            tc.psum_pool(name="ps", bufs=4) as pspool,
        ):
            w_sb = cpool.tile([128, 18 * 128], f16)
            mask_sb = cpool.tile([F, NCOL], f16)
            bif_sb = cpool.tile([128, 1], f32)
            bg_sb = cpool.tile([F, 1], f32)
            bo_sb = cpool.tile([128, 1], f32)
            bns_sb = cpool.tile([128, 1], f32)
            bnb_sb = cpool.tile([128, 1], f32)
            sel_sb = cpool.tile([F, 8], f32)
            # x on the SP queue (feed PE first); weights + small consts on
            # the ACT queue so they don't head-of-line-block the x DMAs
            # warm the ACT function table first: its engine time overlaps
            # the const DMAs' sequencer work on the same queue
            warm = cpool.tile([64, 1], f32)
            nc.vector.memset(warm[:], 0.0)
            nc.scalar.activation(warm[:], warm[:],
                                 mybir.ActivationFunctionType.Tanh)
            # ACT queue: only what the first gate chain needs, w first
            nc.scalar.dma_start(w_sb[:, 0:1152], w_d[:, 0:1152])
            nc.scalar.dma_start(w_sb[:, 1152:2304], w_d[:, 1152:2304])
            nc.scalar.dma_start(bif_sb[:], bif_d[:])
            nc.scalar.dma_start(mask_sb[:], mask_d[:])
            # Pool queue is idle early (BNs start late): other consts here
            nc.gpsimd.dma_start(bg_sb[:], bg_d[:])
            nc.gpsimd.dma_start(bo_sb[:], bo_d[:])

            # 4 rotating buffers: step t reads x_t+h_{t-1} from zin[(t-1)%4]
            # and writes h_t into zin[t%4]. 4 (not 2) so a step's x DMA never
            # lands in a buffer whose x a not-yet-emitted chunk still reads.
            zin = [spool.tile([128, NCOL], f16, name=f"zin{i}", tag=f"zin{i}")
                   for i in range(4)]
            c_sb = spool.tile([F, NCOL], f32, tag="cstate")
            # h-halves and x guard rows must start zero (NaN garbage would
            # survive the h mask multiply via guard-col taps); x interior is
            # fully overwritten by the per-step DMA.
            nc.gpsimd.memset(c_sb[:], 0.0)   # first: feeds t=1's f*c early
            for i in range(4):
                eng = nc.vector if i % 2 == 0 else nc.gpsimd
                eng.memset(zin[i][64:128, :], 0.0)
                eng.memset(zin[i][0:64, 0:WP], 0.0)
                eng.memset(zin[i][0:64, NCOL - WP:NCOL], 0.0)
            # BN/select consts after the state memsets (needed later)
            nc.gpsimd.dma_start(bns_sb[:], bns_d[:])
            nc.gpsimd.dma_start(bnb_sb[:], bnb_d[:])
            nc.gpsimd.dma_start(sel_sb[:], sel_d[:])

            def emit_chunk_head(t, p0, n):
                cur = zin[(t - 1) % 4]
                ps_if = pspool.tile([128, n], f32, tag="psif")
                ps_go = pspool.tile([128, n], f32, tag="psgo")
                for k, (dy, dx) in enumerate(TAPS):
                    off = p0 + dy * WP + dx
                    nc.tensor.matmul(
                        ps_if[:], w_sb[:, k * 128:(k + 1) * 128],
                        cur[:, off:off + n], start=(k == 0), stop=(k == 8))
                for k, (dy, dx) in enumerate(TAPS):
                    off = p0 + dy * WP + dx
                    nc.tensor.matmul(
                        ps_go[:], w_sb[:, 1152 + k * 128:1152 + (k + 1) * 128],
                        cur[:, off:off + n], start=(k == 0), stop=(k == 8))

                sig_if = wpool.tile([128, n], f16, tag="sig_if")
                f_low = wpool.tile([F, n], f16, tag="f_low")
                tanh_g = wpool.tile([F, n], f16, tag="tanh_g")
                sig_o = wpool.tile([F, n], f16, tag="sig_o")
                t1 = wpool.tile([F, n], f16, tag="t1")
                t2 = wpool.tile([F, n], f32, tag="t2")

                # i|f fused: weights pre-scaled by 0.2 -> Relu(ps + 0.2b+0.5)
                nc.scalar.activation(sig_if[:], ps_if[:], Relu,
                                     bias=bif_sb[:, 0:1])
                # DVE TensorTensor needs equal input base partitions: min the
                # f-half down to partitions 0-63 while clipping it
                nc.vector.tensor_scalar_min(sig_if[0:64, :],
                                            sig_if[0:64, :], 1.0)
                nc.vector.tensor_scalar_min(f_low[:], sig_if[64:128, :], 1.0)
                nc.scalar.activation(tanh_g[:], ps_go[0:64, :], Tanh,
                                     bias=bg_sb[:, 0:1])
                nc.scalar.activation(sig_o[:], ps_go[64:128, :], Relu,
                                     bias=bo_sb[64:128, 0:1])
                # min(o,1) and the h edge-mask fused: mask is 1 in-image (so
                # min(o, 1)) and 0 outside (so o -> 0 -> h = 0)
                nc.vector.tensor_tensor(sig_o[:], sig_o[:],
                                        mask_sb[:, p0:p0 + n],
                                        mybir.AluOpType.min)
                nc.vector.tensor_mul(t1[:], sig_if[0:64, :], tanh_g[:])
                nc.vector.tensor_mul(t2[:], f_low[:], c_sb[:, p0:p0 + n])
                nc.vector.tensor_add(c_sb[:, p0:p0 + n], t1[:], t2[:])
                return sig_o

            def emit_chunk_tail(t, stage, p0, n, sig_o):
                # second pass: tanh(c) and h, emitted after every chunk's
                # gate work so a blocked tanh_c can't head-of-line-block the
                # next chunk's ready activations in the ACT FIFO
                nxt = zin[t % 4]
                tanh_c = wpool.tile([F, n], f16, tag="tanh_c")
                nc.scalar.activation(tanh_c[:], c_sb[:, p0:p0 + n], Tanh)
                nc.vector.tensor_mul(nxt[64:128, p0:p0 + n],
                                     sig_o[:], tanh_c[:])
                lo, hi = max(p0, OWN_LO), min(p0 + n, OWN_HI)
                if lo < hi:
                    # BN on Pool: off the critical recurrence path, so the
                    # collective blocking Pool only delays the output stage
                    nc.gpsimd.tensor_scalar(
                        stage[:, lo - OWN_LO:hi - OWN_LO],
                        nxt[64:128, lo:hi],
                        bns_sb[64:128, 0:1], bnb_sb[64:128, 0:1], MULT, ADD)

            def emit_step(t, stage, chunk_list):
                heads = [(p0, n, emit_chunk_head(t, p0, n))
                         for p0, n in chunk_list]
                for p0, n, sig_o in heads:
                    emit_chunk_tail(t, stage, p0, n, sig_o)

            def emit_exchange_send(t):
                # ---- exchange epoch: ship h,c boundary rows (4 each) ----
                nxt = zin[t % 4]
                cbf = agpool.tile([F, 2 * HB], f16, tag="cbf")
                nc.vector.tensor_scalar_mul(
                    cbf[:, 0:HB], c_sb[:, OWN_LO:OWN_LO + HB], 1.0)
                nc.vector.tensor_scalar_mul(
                    cbf[:, HB:2 * HB], c_sb[:, OWN_HI - HB:OWN_HI], 1.0)
                agin = dpool.tile([F, 4 * HB], f16, tag="agin")
                agout = dpool.tile([4 * F, 4 * HB], f16, tag="agout")
                nc.sync.dma_start(agin[:, 0:HB],
                                  nxt[64:128, OWN_LO:OWN_LO + HB])
                nc.sync.dma_start(agin[:, HB:2 * HB],
                                  nxt[64:128, OWN_HI - HB:OWN_HI])
                nc.sync.dma_start(agin[:, 2 * HB:4 * HB], cbf[:])
                nc.gpsimd.collective_compute(
                    "AllGather", mybir.AluOpType.bypass,
                    ins=[agin.opt()], outs=[agout.opt()],
                    replica_groups=[[0, 1, 2, 3], [4, 5, 6, 7]],
                )
                return agout

            def _select(src, dst_ap, scol, tiles, eng):
                # dst = sum_j src[:, j*HB : +HB] * sel[:, scol+j]
                for j in range(4):
                    eng.tensor_scalar_mul(
                        tiles[j][:], src[:, j * HB:(j + 1) * HB],
                        sel_sb[:, scol + j:scol + j + 1])
                eng.tensor_add(tiles[0][:], tiles[0][:], tiles[1][:])
                eng.tensor_add(tiles[2][:], tiles[2][:], tiles[3][:])
                eng.tensor_add(dst_ap, tiles[0][:], tiles[2][:])

            def emit_exchange_recv_pre(t, agout):
                # Emitted BEFORE the next block's phase A. All 4 compact
                # DMAs go on SP (it stalls on the AG, which only delays the
                # deferred out DMAs queued behind), critical halves (_bot,
                # feeding the up-halo that gates B0-L) first. The c selects
                # go FIRST on the Pool queue: Pool is blocked by the in-
                # flight collective anyway, and this keeps them ahead of
                # the phase-A BN backlog that would otherwise gate B0.
                agsb = {}
                for key, col0 in (("hb", HB), ("cb", 3 * HB),
                                  ("ht", 0), ("ct", 2 * HB)):
                    tile_ = agpool.tile([F, 4 * HB], f16, name=f"agsb_{key}",
                                        tag=f"agsb_{key}")
                    nc.sync.dma_start(
                        tile_[:].rearrange("p (j c) -> p j c", j=4),
                        agout[:, col0:col0 + HB].rearrange(
                            "(j p) c -> p j c", j=4))
                    agsb[key] = tile_
                csel = [agpool.tile([F, HB], f32, name=f"csel{i}",
                                    tag=f"csel{i}") for i in range(4)]
                _select(agsb["cb"], c_sb[:, WP:WP + HB], 0, csel, nc.gpsimd)
                _select(agsb["ct"], c_sb[:, OWN_HI:OWN_HI + HB], 4, csel,
                        nc.gpsimd)
                return agsb

            def emit_exchange_recv_post(t, agsb):
                # h selects on DVE, emitted after phase A so the DVE gate
                # chains are dispatched first
                nxt = zin[t % 4]
                tsel = [agpool.tile([F, HB], f16, name=f"tsel{i}",
                                    tag=f"tsel{i}") for i in range(4)]
                _select(agsb["hb"], nxt[64:128, WP:WP + HB], 0, tsel,
                        nc.vector)
                _select(agsb["ht"], nxt[64:128, OWN_HI:OWN_HI + HB], 4, tsel,
                        nc.vector)

            # Nested halo-independent interiors: I_j needs only I_{j-1}'s h
            # (67-col tap margin), so all interiors of a block are runnable
            # while the preceding epoch's AllGather is still in flight.
            INT = [(397 + 67 * j, 1319 - 67 * j) for j in range(4)]

            pending = None           # (epoch step, agout) awaiting receive
            deferred_outs = []       # (step, stage tile) from previous block
            for b in range(4):
                stages = {}
                for j in range(4):
                    t = 4 * b + j + 1
                    cur = zin[(t - 1) % 4]
                    # full interior rows every step: keeps every buffer's
                    # x-half fresh (no stale-x reads from 4 steps ago)
                    if t == 1:
                        # first chunk's tap range first: unblocks MM #1 early
                        nc.sync.dma_start(cur[0:64, 330:958],
                                          xT_d[t - 1, :, 330:958])
                        nc.sync.dma_start(cur[0:64, WP:330],
                                          xT_d[t - 1, :, WP:330])
                        nc.sync.dma_start(cur[0:64, 958:NCOL - WP],
                                          xT_d[t - 1, :, 958:NCOL - WP])
                    else:
                        nc.sync.dma_start(cur[0:64, WP:NCOL - WP],
                                          xT_d[t - 1, :, WP:NCOL - WP])
                    stages[t] = opool.tile([F, 16 * WP], f16, name=f"stage{t}",
                                           tag=f"stage{j}")
                if pending is not None:
                    recv_tiles = emit_exchange_recv_pre(*pending)
                # phase A: interiors (halo-independent), step order — these
                # overlap the in-flight AllGather from the previous block.
                # Split nested (67-col stagger) so step j+1's first chunk
                # depends only on step j's first chunk's pointwise chain.
                # 3 cones per interior: shrinking left cone + two
                # constant-width sliding cones; cone k of step j+1 depends
                # only on cones <= k of step j, so each transition chain is
                # covered by the later cones' matmuls
                for j in range(4):
                    t = 4 * b + j + 1
                    i0, i1 = INT[j]
                    q1, q2 = 891 - 67 * j, 1105 - 67 * j
                    emit_step(t, stages[t],
                              [(i0, q1 - i0), (q1, q2 - q1), (q2, i1 - q2)])
                if pending is not None:
                    emit_exchange_recv_post(pending[0], recv_tiles)
                    pending = None
                # previous block's output DMAs: deferred to here so their
                # BN deps are long done (they'd stall whichever queue they
                # sat in otherwise); SP is past its x DMAs by now
                for t_prev, stg in deferred_outs:
                    nc.sync.dma_start(out_d[t_prev - 1], stg[:])
                deferred_outs = []
                # phase B: boundary chunks, step order; epoch step's boundary
                # feeds the exchange
                for j in range(4):
                    t = 4 * b + j + 1
                    s = 3 - j
                    i0, i1 = INT[j]
                    c0, c1 = (5 - s) * WP, (21 + s) * WP
                    if t == T:
                        # final step: halve the right chunk so the kernel's
                        # tail chain (gates -> BN -> out DMA) is shorter
                        m = (i1 + c1) // 2
                        emit_step(t, stages[t],
                                  [(c0, i0 - c0), (i1, m - i1), (m, c1 - m)])
                    else:
                        emit_step(t, stages[t],
                                  [(c0, i0 - c0), (i1, c1 - i1)])
                for j in range(4):
                    t = 4 * b + j + 1
                    if t == T:
                        # last block drains on the (idle-by-now) SP queue,
                        # t=16 split in halves so the first half ships while
                        # the last BN finishes
                        nc.sync.dma_start(out_d[t - 1, :, 0:788],
                                          stages[t][:, 0:788])
                        nc.sync.dma_start(out_d[t - 1, :, 788:16 * WP],
                                          stages[t][:, 788:16 * WP])
                    elif b == 3:
                        nc.sync.dma_start(out_d[t - 1], stages[t][:])
                    else:
                        deferred_outs.append((t, stages[t]))
                if b < 3:
                    te = 4 * b + 4
                    pending = (te, emit_exchange_send(te))

        global _LAST_TC
        _LAST_TC = tc
    _split_multi_waits(nc)
    return nc


def _prep_inputs(x, Wx, Wh, b, gamma, beta, moving_mean, moving_var):
    x = np.asarray(x, F32)
    Wx = np.asarray(Wx, F32)
    Wh = np.asarray(Wh, F32)
    b = np.asarray(b, F32)
    # gate order along 4F: [i | f | g | o]; half1 = [i|f], half2 = [g|o].
    # Pre-scale i/f/o columns by 0.2 (hard_sigmoid slope).
    wstack = np.zeros((128, 18 * 128), F32)
    for k, (dy, dx) in enumerate(TAPS):
        ky, kx = dy + 1, dx + 1
        wstack[0:64, k * 128:(k + 1) * 128] = Wx[ky, kx, :, 0:128] * 0.2
        wstack[64:128, k * 128:(k + 1) * 128] = Wh[ky, kx, :, 0:128] * 0.2
        h2 = np.concatenate([Wx[ky, kx, :, 128:192],
                             Wx[ky, kx, :, 192:256] * 0.2], axis=1)
        wstack[0:64, 1152 + k * 128:1152 + (k + 1) * 128] = h2
        h2h = np.concatenate([Wh[ky, kx, :, 128:192],
                              Wh[ky, kx, :, 192:256] * 0.2], axis=1)
        wstack[64:128, 1152 + k * 128:1152 + (k + 1) * 128] = h2h
    wstack = wstack.astype(F16)

    b_if = (0.2 * b[0:128] + 0.5).reshape(128, 1).astype(F32)
    bg = b[128:192].reshape(64, 1).astype(F32)
    bo1 = 0.2 * b[192:256] + 0.5
    b_o = np.concatenate([bo1, bo1]).reshape(128, 1).astype(F32)
    inv = (np.asarray(gamma, F32) /
           np.sqrt(np.asarray(moving_var, F32) + 1e-3))
    bnb1 = (np.asarray(beta, F32) - np.asarray(moving_mean, F32) * inv)
    # duplicated into both partition halves: BN reads h at partitions 64-127
    bns = np.concatenate([inv, inv]).reshape(128, 1).astype(F32)
    bnb = np.concatenate([bnb1, bnb1]).reshape(128, 1).astype(F32)

    in_maps = []
    for core in range(8):
        bidx, sl = core // 4, core % 4
        r0 = 16 * sl
        glo, ghi = max(0, r0 - 5), min(64, r0 + 21)
        i0 = glo - (r0 - 5)
        xpad = np.zeros((T, NR, WP, C), F32)
        xpad[:, i0:i0 + (ghi - glo), 1:65, :] = x[bidx, :, glo:ghi, :, :]
        xT = np.ascontiguousarray(
            xpad.transpose(0, 3, 1, 2).reshape(T, C, NCOL)).astype(F16)
        m = np.zeros((NR, WP), F32)
        for i in range(NR):
            if 0 <= (r0 - 5 + i) < 64:
                m[i, 1:65] = 1.0
        mask = np.broadcast_to(
            m.reshape(1, NCOL), (64, NCOL)).astype(F16).copy()
        # select masks: sel[:, 0:4] = up (choose group-rank sl-1),
        # sel[:, 4:8] = down (choose group-rank sl+1)
        sel = np.zeros((64, 8), F32)
        if sl > 0:
            sel[:, sl - 1] = 1.0
        if sl < 3:
            sel[:, 4 + sl + 1] = 1.0
        in_maps.append({
            "xT": xT, "w": wstack, "mask": mask, "b_if": b_if,
            "bg": bg, "b_o": b_o, "bns": bns, "bnb": bnb, "sel": sel,
        })
    return in_maps


def kernel(x, Wx, Wh, b, gamma, beta, moving_mean, moving_var):
    global _PROG
    if _PROG is None:
        _PROG = _build()
    in_maps = _prep_inputs(x, Wx, Wh, b, gamma, beta, moving_mean, moving_var)
    res = run_bass_kernel_spmd(_PROG, in_maps, core_ids=list(range(8)))
    out = np.empty((2, T, 64, W, F), F32)
    for core in range(8):
        bidx, sl = core // 4, core % 4
        oc = res.results[core]["out"].astype(F32).reshape(
            T, F, 16, WP)[:, :, :, 1:65]
        out[bidx, :, 16 * sl:16 * sl + 16] = oc.transpose(0, 2, 3, 1)
    return out


# revision 68
# speedup vs baseline: 1.5664x; 1.0090x over previous
"""ConvLSTM2D (Keras gate order, hard_sigmoid) + inference BatchNorm on 8
Trainium2 NeuronCores.

Sharding: batch (2) x H-slabs (4) -> 8 cores. The T=16 recurrence is split
into 4 blocks of 4 steps. Within a block each core computes a shrinking halo
(depth 4); at block boundaries (t=4,8,12) cores exchange 4 boundary rows of
BOTH states (h and c) with their slab neighbors via one AllGather per epoch
(replica groups = the two 4-slab groups). This cuts redundant conv work from
1.94x (17-deep shrinking halo) to 1.19x while keeping the program uniform
SPMD: out-of-image edges are data (zero-padded x, h-mask, zero select masks).

Layout: channels-on-partitions. zin (128 x 1716 fp16) holds x_t on
partitions 0-63 and h_{t-1} on 64-127; 26 rows of 66 cols (64 + guard col
each side); a 3x3 conv tap (dy,dx) is the col offset dy*66+dx. One matmul
contracts x AND h channels at once (lhsT = [Wx_tap; Wh_tap]) so
z = conv(x,Wx)+conv(h,Wh) is 9 taps x 2 gate-halves of accumulating matmuls
per chunk (full 128x128 PE). Gate-i/f/o weight columns are pre-scaled by 0.2
so hard_sigmoid is Relu(psum + (0.2b+0.5)) then min(.,1). i|f activations are
fused into single 128-partition ops; pointwise math runs in fp16 on DVE
(2x rate) with c kept in fp32; BN is one DVE tensor_scalar (scale+bias) into
an fp16 output.
"""
import math
import numpy as np

import concourse.bass as bass
import concourse.mybir as mybir
import concourse.tile as tile
from concourse.bass_utils import run_bass_kernel_spmd

F16 = np.float16
F32 = np.float32

T, F, C, W = 16, 64, 64, 64
L = 4              # block length (steps between exchanges)
HALO = 4           # halo depth = L
NR = 16 + 2 * HALO + 2   # 26 buffer rows: [r0-5, r1+5)
WP = W + 1         # 65: one shared zero guard col between rows
# (row r's right-pad tap lands on row r+1's left guard, both always 0)
NCOL = NR * WP     # 1716
OWN_LO, OWN_HI = 5 * WP, 21 * WP     # own 16 rows: buffer rows [5, 21)
HB = HALO * WP     # 264: one halo bundle (4 rows)
TAPS = [(dy, dx) for dy in (-1, 0, 1) for dx in (-1, 0, 1)]

TRACE_SIM = False
_PROG = None
_LAST_TC = None

# ---------------------------------------------------------------------------
# Workaround: this walrus build accepts at most ONE sync wait per
# instruction; Tile attaches several. Hoist extras onto same-engine NOPs
# inserted right before the instruction (per-engine order preserved).
_MAX_WAITS = 1


def _split_multi_waits(nc):
    for fn in nc.m.functions:
        for bb in fn.blocks:
            lst = bb.instructions
            out, changed = [], False
            for ins in lst:
                si = ins.sync_info
                if si is not None and len(si.on_wait) > _MAX_WAITS:
                    waits = list(si.on_wait)
                    extra, keep = waits[:-_MAX_WAITS], waits[-_MAX_WAITS:]
                    for j, w in enumerate(extra):
                        nop = mybir.InstNoOp(
                            name=f"{ins.name}.sw{j}", ins=[], outs=[],
                            text_hint="split_wait", bass_nofuse=True)
                        nop.engine = ins.engine
                        nop.sync_info = mybir.SyncInfo(on_wait=[w], on_update=[])
                        out.append(nop)
                    ins.sync_info = mybir.SyncInfo(
                        on_wait=keep, on_update=list(si.on_update))
                    changed = True
                out.append(ins)
            if changed:
                try:
                    bb.instructions = out
                except Exception:
                    lst.clear()
                    lst.extend(out)


def _chunks(c0, c1, maxn=512):
    Ln = c1 - c0
    n = max(1, math.ceil(Ln / maxn))
    base, rem = divmod(Ln, n)
    sizes = [base + (1 if i < rem else 0) for i in range(n)]
    out, p = [], c0
    for s in sizes:
        out.append((p, s))
        p += s
    return out


def _build():
    nc = bass.Bass(target_bir_lowering=False)
    f32, f16 = mybir.dt.float32, mybir.dt.float16

    xT_d = nc.dram_tensor("xT", [T, C, NCOL], f16, kind="ExternalInput")
    w_d = nc.dram_tensor("w", [128, 18 * 128], f16, kind="ExternalInput")
    mask_d = nc.dram_tensor("mask", [F, NCOL], f16, kind="ExternalInput")
    bif_d = nc.dram_tensor("b_if", [128, 1], f32, kind="ExternalInput")
    bg_d = nc.dram_tensor("bg", [F, 1], f32, kind="ExternalInput")
    bo_d = nc.dram_tensor("b_o", [128, 1], f32, kind="ExternalInput")
    bns_d = nc.dram_tensor("bns", [128, 1], f32, kind="ExternalInput")
    bnb_d = nc.dram_tensor("bnb", [128, 1], f32, kind="ExternalInput")
    sel_d = nc.dram_tensor("sel", [F, 8], f32, kind="ExternalInput")
    out_d = nc.dram_tensor("out", [T, F, 16 * WP], f16, kind="ExternalOutput")

    Relu = mybir.ActivationFunctionType.Relu
    Tanh = mybir.ActivationFunctionType.Tanh
    MULT = mybir.AluOpType.mult
    ADD = mybir.AluOpType.add

    with tile.TileContext(nc, trace_sim=TRACE_SIM) as tc:
        with (
            tc.tile_pool(name="const", bufs=1) as cpool,
            tc.tile_pool(name="state", bufs=1) as spool,
            tc.tile_pool(name="work", bufs=4) as wpool,
            tc.tile_pool(name="ostage", bufs=2) as opool,
            tc.tile_pool(name="agx", bufs=2) as agpool,
            tc.tile_pool(name="dram", bufs=2, space="DRAM") as dpool,
            tc.psum_pool(name="ps", bufs=4) as pspool,
        ):
            w_sb = cpool.tile([128, 18 * 128], f16)
            mask_sb = cpool.tile([F, NCOL], f16)
            bif_sb = cpool.tile([128, 1], f32)
            bg_sb = cpool.tile([F, 1], f32)
            bo_sb = cpool.tile([128, 1], f32)
            bns_sb = cpool.tile([128, 1], f32)
            bnb_sb = cpool.tile([128, 1], f32)
            sel_sb = cpool.tile([F, 8], f32)
            # x on the SP queue (feed PE first); weights + small consts on
            # the ACT queue so they don't head-of-line-block the x DMAs
            # warm the ACT function table first: its engine time overlaps
            # the const DMAs' sequencer work on the same queue
            warm = cpool.tile([64, 1], f32)
            nc.vector.memset(warm[:], 0.0)
            nc.scalar.activation(warm[:], warm[:],
                                 mybir.ActivationFunctionType.Tanh)
            # ACT queue: only what the first gate chain needs, w first
            nc.scalar.dma_start(w_sb[:, 0:1152], w_d[:, 0:1152])
            nc.scalar.dma_start(w_sb[:, 1152:2304], w_d[:, 1152:2304])
            nc.scalar.dma_start(bif_sb[:], bif_d[:])
            nc.scalar.dma_start(mask_sb[:], mask_d[:])
            # Pool queue is idle early (BNs start late): other consts here
            nc.gpsimd.dma_start(bg_sb[:], bg_d[:])
            nc.gpsimd.dma_start(bo_sb[:], bo_d[:])

            # 4 rotating buffers: step t reads x_t+h_{t-1} from zin[(t-1)%4]
            # and writes h_t into zin[t%4]. 4 (not 2) so a step's x DMA never
            # lands in a buffer whose x a not-yet-emitted chunk still reads.
            zin = [spool.tile([128, NCOL], f16, name=f"zin{i}", tag=f"zin{i}")
                   for i in range(4)]
            c_sb = spool.tile([F, NCOL], f32, tag="cstate")
            # h-halves and x guard rows must start zero (NaN garbage would
            # survive the h mask multiply via guard-col taps); x interior is
            # fully overwritten by the per-step DMA.
            nc.gpsimd.memset(c_sb[:], 0.0)   # first: feeds t=1's f*c early
            for i in range(4):
                eng = nc.vector if i % 2 == 0 else nc.gpsimd
                eng.memset(zin[i][64:128, :], 0.0)
                eng.memset(zin[i][0:64, 0:WP], 0.0)
                eng.memset(zin[i][0:64, NCOL - WP:NCOL], 0.0)
            # BN/select consts after the state memsets (needed later)
            nc.gpsimd.dma_start(bns_sb[:], bns_d[:])
            nc.gpsimd.dma_start(bnb_sb[:], bnb_d[:])
            nc.gpsimd.dma_start(sel_sb[:], sel_d[:])

            def emit_chunk_head(t, p0, n):
                cur = zin[(t - 1) % 4]
                ps_if = pspool.tile([128, n], f32, tag="psif")
                ps_go = pspool.tile([128, n], f32, tag="psgo")
                for k, (dy, dx) in enumerate(TAPS):
                    off = p0 + dy * WP + dx
                    nc.tensor.matmul(
                        ps_if[:], w_sb[:, k * 128:(k + 1) * 128],
                        cur[:, off:off + n], start=(k == 0), stop=(k == 8))
                for k, (dy, dx) in enumerate(TAPS):
                    off = p0 + dy * WP + dx
                    nc.tensor.matmul(
                        ps_go[:], w_sb[:, 1152 + k * 128:1152 + (k + 1) * 128],
                        cur[:, off:off + n], start=(k == 0), stop=(k == 8))

                sig_if = wpool.tile([128, n], f16, tag="sig_if")
                f_low = wpool.tile([F, n], f16, tag="f_low")
                tanh_g = wpool.tile([F, n], f16, tag="tanh_g")
                sig_o = wpool.tile([F, n], f16, tag="sig_o")
                t1 = wpool.tile([F, n], f16, tag="t1")
                t2 = wpool.tile([F, n], f32, tag="t2")

                # i|f fused: weights pre-scaled by 0.2 -> Relu(ps + 0.2b+0.5)
                nc.scalar.activation(sig_if[:], ps_if[:], Relu,
                                     bias=bif_sb[:, 0:1])
                # DVE TensorTensor needs equal input base partitions: min the
                # f-half down to partitions 0-63 while clipping it
                nc.vector.tensor_scalar_min(sig_if[0:64, :],
                                            sig_if[0:64, :], 1.0)
                nc.vector.tensor_scalar_min(f_low[:], sig_if[64:128, :], 1.0)
                nc.scalar.activation(tanh_g[:], ps_go[0:64, :], Tanh,
                                     bias=bg_sb[:, 0:1])
                nc.scalar.activation(sig_o[:], ps_go[64:128, :], Relu,
                                     bias=bo_sb[64:128, 0:1])
                # min(o,1) and the h edge-mask fused: mask is 1 in-image (so
                # min(o, 1)) and 0 outside (so o -> 0 -> h = 0)
                nc.vector.tensor_tensor(sig_o[:], sig_o[:],
                                        mask_sb[:, p0:p0 + n],
                                        mybir.AluOpType.min)
                nc.vector.tensor_mul(t1[:], sig_if[0:64, :], tanh_g[:])
                nc.vector.tensor_mul(t2[:], f_low[:], c_sb[:, p0:p0 + n])
                nc.vector.tensor_add(c_sb[:, p0:p0 + n], t1[:], t2[:])
                return sig_o

            def emit_chunk_tail(t, stage, p0, n, sig_o):
                # second pass: tanh(c) and h, emitted after every chunk's
                # gate work so a blocked tanh_c can't head-of-line-block the
                # next chunk's ready activations in the ACT FIFO
                nxt = zin[t % 4]
                tanh_c = wpool.tile([F, n], f16, tag="tanh_c")
                nc.scalar.activation(tanh_c[:], c_sb[:, p0:p0 + n], Tanh)
                nc.vector.tensor_mul(nxt[64:128, p0:p0 + n],
                                     sig_o[:], tanh_c[:])
                lo, hi = max(p0, OWN_LO), min(p0 + n, OWN_HI)
                if lo < hi:
                    # BN on Pool: off the critical recurrence path, so the
                    # collective blocking Pool only delays the output stage
                    nc.gpsimd.tensor_scalar(
                        stage[:, lo - OWN_LO:hi - OWN_LO],
                        nxt[64:128, lo:hi],
                        bns_sb[64:128, 0:1], bnb_sb[64:128, 0:1], MULT, ADD)

            def emit_step(t, stage, chunk_list):
                heads = [(p0, n, emit_chunk_head(t, p0, n))
                         for p0, n in chunk_list]
                for p0, n, sig_o in heads:
                    emit_chunk_tail(t, stage, p0, n, sig_o)

            def emit_exchange_send(t):
                # ---- exchange epoch: ship h,c boundary rows (4 each) ----
                nxt = zin[t % 4]
                cbf = agpool.tile([F, 2 * HB], f16, tag="cbf")
                nc.vector.tensor_scalar_mul(
                    cbf[:, 0:HB], c_sb[:, OWN_LO:OWN_LO + HB], 1.0)
                nc.vector.tensor_scalar_mul(
                    cbf[:, HB:2 * HB], c_sb[:, OWN_HI - HB:OWN_HI], 1.0)
                agin = dpool.tile([F, 4 * HB], f16, tag="agin")
                agout = dpool.tile([4 * F, 4 * HB], f16, tag="agout")
                nc.sync.dma_start(agin[:, 0:HB],
                                  nxt[64:128, OWN_LO:OWN_LO + HB])
                nc.sync.dma_start(agin[:, HB:2 * HB],
                                  nxt[64:128, OWN_HI - HB:OWN_HI])
                nc.sync.dma_start(agin[:, 2 * HB:4 * HB], cbf[:])
                nc.gpsimd.collective_compute(
                    "AllGather", mybir.AluOpType.bypass,
                    ins=[agin.opt()], outs=[agout.opt()],
                    replica_groups=[[0, 1, 2, 3], [4, 5, 6, 7]],
                )
                return agout

            def _select(src, dst_ap, scol, tiles, eng):
                # dst = sum_j src[:, j*HB : +HB] * sel[:, scol+j]
                for j in range(4):
                    eng.tensor_scalar_mul(
                        tiles[j][:], src[:, j * HB:(j + 1) * HB],
                        sel_sb[:, scol + j:scol + j + 1])
                eng.tensor_add(tiles[0][:], tiles[0][:], tiles[1][:])
                eng.tensor_add(tiles[2][:], tiles[2][:], tiles[3][:])
                eng.tensor_add(dst_ap, tiles[0][:], tiles[2][:])

            def emit_exchange_recv_pre(t, agout):
                # Emitted BEFORE the next block's phase A. All 4 compact
                # DMAs go on SP (it stalls on the AG, which only delays the
                # deferred out DMAs queued behind), critical halves (_bot,
                # feeding the up-halo that gates B0-L) first. The c selects
                # go FIRST on the Pool queue: Pool is blocked by the in-
                # flight collective anyway, and this keeps them ahead of
                # the phase-A BN backlog that would otherwise gate B0.
                agsb = {}
                for key, col0 in (("hb", HB), ("cb", 3 * HB),
                                  ("ht", 0), ("ct", 2 * HB)):
                    tile_ = agpool.tile([F, 4 * HB], f16, name=f"agsb_{key}",
                                        tag=f"agsb_{key}")
                    nc.sync.dma_start(
                        tile_[:].rearrange("p (j c) -> p j c", j=4),
                        agout[:, col0:col0 + HB].rearrange(
                            "(j p) c -> p j c", j=4))
                    agsb[key] = tile_
                csel = [agpool.tile([F, HB], f32, name=f"csel{i}",
                                    tag=f"csel{i}") for i in range(4)]
                _select(agsb["cb"], c_sb[:, WP:WP + HB], 0, csel, nc.gpsimd)
                _select(agsb["ct"], c_sb[:, OWN_HI:OWN_HI + HB], 4, csel,
                        nc.gpsimd)
                return agsb

            def emit_exchange_recv_post(t, agsb):
                # h selects on DVE, emitted after phase A so the DVE gate
                # chains are dispatched first
                nxt = zin[t % 4]
                tsel = [agpool.tile([F, HB], f16, name=f"tsel{i}",
                                    tag=f"tsel{i}") for i in range(4)]
                _select(agsb["hb"], nxt[64:128, WP:WP + HB], 0, tsel,
                        nc.vector)
                _select(agsb["ht"], nxt[64:128, OWN_HI:OWN_HI + HB], 4, tsel,
                        nc.vector)

            # Nested halo-independent interiors: I_j needs only I_{j-1}'s h
            # (67-col tap margin), so all interiors of a block are runnable
            # while the preceding epoch's AllGather is still in flight.
            INT = [(5 * WP + 66 * (j + 1), 21 * WP - 66 * (j + 1))
                   for j in range(4)]

            pending = None           # (epoch step, agout) awaiting receive
            deferred_outs = []       # (step, stage tile) from previous block
            for b in range(4):
                stages = {}
                for j in range(4):
                    t = 4 * b + j + 1
                    cur = zin[(t - 1) % 4]
                    # full interior rows every step: keeps every buffer's
                    # x-half fresh (no stale-x reads from 4 steps ago)
                    if t == 1:
                        # first chunk's tap range first: unblocks MM #1 early
                        nc.sync.dma_start(cur[0:64, 5 * WP:943],
                                          xT_d[t - 1, :, 5 * WP:943])
                        nc.sync.dma_start(cur[0:64, WP:5 * WP],
                                          xT_d[t - 1, :, WP:5 * WP])
                        nc.sync.dma_start(cur[0:64, 943:NCOL - WP],
                                          xT_d[t - 1, :, 943:NCOL - WP])
                    else:
                        nc.sync.dma_start(cur[0:64, WP:NCOL - WP],
                                          xT_d[t - 1, :, WP:NCOL - WP])
                    stages[t] = opool.tile([F, 16 * WP], f16, name=f"stage{t}",
                                           tag=f"stage{j}")
                if pending is not None:
                    recv_tiles = emit_exchange_recv_pre(*pending)
                # phase A: interiors (halo-independent), step order — these
                # overlap the in-flight AllGather from the previous block.
                # Split nested (67-col stagger) so step j+1's first chunk
                # depends only on step j's first chunk's pointwise chain.
                # 3 cones per interior: shrinking left cone + two
                # constant-width sliding cones; cone k of step j+1 depends
                # only on cones <= k of step j, so each transition chain is
                # covered by the later cones' matmuls
                for j in range(4):
                    t = 4 * b + j + 1
                    i0, i1 = INT[j]
                    q1, q2 = 877 - 66 * j, 1088 - 66 * j
                    emit_step(t, stages[t],
                              [(i0, q1 - i0), (q1, q2 - q1), (q2, i1 - q2)])
                if pending is not None:
                    emit_exchange_recv_post(pending[0], recv_tiles)
                    pending = None
                # previous block's output DMAs: deferred to here so their
                # BN deps are long done (they'd stall whichever queue they
                # sat in otherwise); SP is past its x DMAs by now
                for t_prev, stg in deferred_outs:
                    nc.sync.dma_start(out_d[t_prev - 1], stg[:])
                deferred_outs = []
                # phase B: boundary chunks, step order; epoch step's boundary
                # feeds the exchange
                for j in range(4):
                    t = 4 * b + j + 1
                    s = 3 - j
                    i0, i1 = INT[j]
                    c0, c1 = (5 - s) * WP, (21 + s) * WP
                    if t == T:
                        # final step: halve the right chunk so the kernel's
                        # tail chain (gates -> BN -> out DMA) is shorter
                        m = (i1 + c1) // 2
                        emit_step(t, stages[t],
                                  [(c0, i0 - c0), (i1, m - i1), (m, c1 - m)])
                    else:
                        emit_step(t, stages[t],
                                  [(c0, i0 - c0), (i1, c1 - i1)])
                for j in range(4):
                    t = 4 * b + j + 1
                    if t == T:
                        # last block drains on the (idle-by-now) SP queue,
                        # t=16 split in halves so the first half ships while
                        # the last BN finishes
                        nc.sync.dma_start(out_d[t - 1, :, 0:12 * WP],
                                          stages[t][:, 0:12 * WP])
                        nc.sync.dma_start(out_d[t - 1, :, 12 * WP:16 * WP],
                                          stages[t][:, 12 * WP:16 * WP])
                    elif b == 3:
                        nc.sync.dma_start(out_d[t - 1], stages[t][:])
                    else:
                        deferred_outs.append((t, stages[t]))
                if b < 3:
                    te = 4 * b + 4
                    pending = (te, emit_exchange_send(te))

        global _LAST_TC
        _LAST_TC = tc
    _split_multi_waits(nc)
    return nc


def _prep_inputs(x, Wx, Wh, b, gamma, beta, moving_mean, moving_var):
    x = np.asarray(x, F32)
    Wx = np.asarray(Wx, F32)
    Wh = np.asarray(Wh, F32)
    b = np.asarray(b, F32)
    # gate order along 4F: [i | f | g | o]; half1 = [i|f], half2 = [g|o].
    # Pre-scale i/f/o columns by 0.2 (hard_sigmoid slope).
    wstack = np.zeros((128, 18 * 128), F32)
    for k, (dy, dx) in enumerate(TAPS):
        ky, kx = dy + 1, dx + 1
        wstack[0:64, k * 128:(k + 1) * 128] = Wx[ky, kx, :, 0:128] * 0.2
        wstack[64:128, k * 128:(k + 1) * 128] = Wh[ky, kx, :, 0:128] * 0.2
        h2 = np.concatenate([Wx[ky, kx, :, 128:192],
                             Wx[ky, kx, :, 192:256] * 0.2], axis=1)
        wstack[0:64, 1152 + k * 128:1152 + (k + 1) * 128] = h2
        h2h = np.concatenate([Wh[ky, kx, :, 128:192],
                              Wh[ky, kx, :, 192:256] * 0.2], axis=1)
        wstack[64:128, 1152 + k * 128:1152 + (k + 1) * 128] = h2h
    wstack = wstack.astype(F16)

    b_if = (0.2 * b[0:128] + 0.5).reshape(128, 1).astype(F32)
    bg = b[128:192].reshape(64, 1).astype(F32)
    bo1 = 0.2 * b[192:256] + 0.5
    b_o = np.concatenate([bo1, bo1]).reshape(128, 1).astype(F32)
    inv = (np.asarray(gamma, F32) /
           np.sqrt(np.asarray(moving_var, F32) + 1e-3))
    bnb1 = (np.asarray(beta, F32) - np.asarray(moving_mean, F32) * inv)
    # duplicated into both partition halves: BN reads h at partitions 64-127
    bns = np.concatenate([inv, inv]).reshape(128, 1).astype(F32)
    bnb = np.concatenate([bnb1, bnb1]).reshape(128, 1).astype(F32)

    in_maps = []
    for core in range(8):
        bidx, sl = core // 4, core % 4
        r0 = 16 * sl
        glo, ghi = max(0, r0 - 5), min(64, r0 + 21)
        i0 = glo - (r0 - 5)
        xpad = np.zeros((T, NR, WP, C), F32)
        xpad[:, i0:i0 + (ghi - glo), 1:65, :] = x[bidx, :, glo:ghi, :, :]
        xT = np.ascontiguousarray(
            xpad.transpose(0, 3, 1, 2).reshape(T, C, NCOL)).astype(F16)
        m = np.zeros((NR, WP), F32)
        for i in range(NR):
            if 0 <= (r0 - 5 + i) < 64:
                m[i, 1:65] = 1.0
        mask = np.broadcast_to(
            m.reshape(1, NCOL), (64, NCOL)).astype(F16).copy()
        # select masks: sel[:, 0:4] = up (choose group-rank sl-1),
        # sel[:, 4:8] = down (choose group-rank sl+1)
        sel = np.zeros((64, 8), F32)
        if sl > 0:
            sel[:, sl - 1] = 1.0
        if sl < 3:
            sel[:, 4 + sl + 1] = 1.0
        in_maps.append({
            "xT": xT, "w": wstack, "mask": mask, "b_if": b_if,
            "bg": bg, "b_o": b_o, "bns": bns, "bnb": bnb, "sel": sel,
        })
    return in_maps


def kernel(x, Wx, Wh, b, gamma, beta, moving_mean, moving_var):
    global _PROG
    if _PROG is None:
        _PROG = _build()
    in_maps = _prep_inputs(x, Wx, Wh, b, gamma, beta, moving_mean, moving_var)
    res = run_bass_kernel_spmd(_PROG, in_maps, core_ids=list(range(8)))
    out = np.empty((2, T, 64, W, F), F32)
    for core in range(8):
        bidx, sl = core // 4, core % 4
        oc = res.results[core]["out"].astype(F32).reshape(
            T, F, 16, WP)[:, :, :, 1:65]
        out[bidx, :, 16 * sl:16 * sl + 16] = oc.transpose(0, 2, 3, 1)
    return out


# revision 69
# speedup vs baseline: 1.5692x; 1.0018x over previous
"""ConvLSTM2D (Keras gate order, hard_sigmoid) + inference BatchNorm on 8
Trainium2 NeuronCores.

Sharding: batch (2) x H-slabs (4) -> 8 cores. The T=16 recurrence is split
into 4 blocks of 4 steps. Within a block each core computes a shrinking halo
(depth 4); at block boundaries (t=4,8,12) cores exchange 4 boundary rows of
BOTH states (h and c) with their slab neighbors via one AllGather per epoch
(replica groups = the two 4-slab groups). This cuts redundant conv work from
1.94x (17-deep shrinking halo) to 1.19x while keeping the program uniform
SPMD: out-of-image edges are data (zero-padded x, h-mask, zero select masks).

Layout: channels-on-partitions. zin (128 x 1716 fp16) holds x_t on
partitions 0-63 and h_{t-1} on 64-127; 26 rows of 66 cols (64 + guard col
each side); a 3x3 conv tap (dy,dx) is the col offset dy*66+dx. One matmul
contracts x AND h channels at once (lhsT = [Wx_tap; Wh_tap]) so
z = conv(x,Wx)+conv(h,Wh) is 9 taps x 2 gate-halves of accumulating matmuls
per chunk (full 128x128 PE). Gate-i/f/o weight columns are pre-scaled by 0.2
so hard_sigmoid is Relu(psum + (0.2b+0.5)) then min(.,1). i|f activations are
fused into single 128-partition ops; pointwise math runs in fp16 on DVE
(2x rate) with c kept in fp32; BN is one DVE tensor_scalar (scale+bias) into
an fp16 output.
"""
import math
import numpy as np

import concourse.bass as bass
import concourse.mybir as mybir
import concourse.tile as tile
from concourse.bass_utils import run_bass_kernel_spmd

F16 = np.float16
F32 = np.float32

T, F, C, W = 16, 64, 64, 64
L = 4              # block length (steps between exchanges)
HALO = 4           # halo depth = L
NR = 16 + 2 * HALO + 2   # 26 buffer rows: [r0-5, r1+5)
WP = W + 1         # 65: one shared zero guard col between rows
# (row r's right-pad tap lands on row r+1's left guard, both always 0)
NCOL = NR * WP     # 1716
OWN_LO, OWN_HI = 5 * WP, 21 * WP     # own 16 rows: buffer rows [5, 21)
HB = HALO * WP     # 264: one halo bundle (4 rows)
TAPS = [(dy, dx) for dy in (-1, 0, 1) for dx in (-1, 0, 1)]

TRACE_SIM = False
_PROG = None
_LAST_TC = None

# ---------------------------------------------------------------------------
# Workaround: this walrus build accepts at most ONE sync wait per
# instruction; Tile attaches several. Hoist extras onto same-engine NOPs
# inserted right before the instruction (per-engine order preserved).
_MAX_WAITS = 1


def _split_multi_waits(nc):
    for fn in nc.m.functions:
        for bb in fn.blocks:
            lst = bb.instructions
            out, changed = [], False
            for ins in lst:
                si = ins.sync_info
                if si is not None and len(si.on_wait) > _MAX_WAITS:
                    waits = list(si.on_wait)
                    extra, keep = waits[:-_MAX_WAITS], waits[-_MAX_WAITS:]
                    for j, w in enumerate(extra):
                        nop = mybir.InstNoOp(
                            name=f"{ins.name}.sw{j}", ins=[], outs=[],
                            text_hint="split_wait", bass_nofuse=True)
                        nop.engine = ins.engine
                        nop.sync_info = mybir.SyncInfo(on_wait=[w], on_update=[])
                        out.append(nop)
                    ins.sync_info = mybir.SyncInfo(
                        on_wait=keep, on_update=list(si.on_update))
                    changed = True
                out.append(ins)
            if changed:
                try:
                    bb.instructions = out
                except Exception:
                    lst.clear()
                    lst.extend(out)


def _chunks(c0, c1, maxn=512):
    Ln = c1 - c0
    n = max(1, math.ceil(Ln / maxn))
    base, rem = divmod(Ln, n)
    sizes = [base + (1 if i < rem else 0) for i in range(n)]
    out, p = [], c0
    for s in sizes:
        out.append((p, s))
        p += s
    return out


def _build():
    nc = bass.Bass(target_bir_lowering=False)
    f32, f16 = mybir.dt.float32, mybir.dt.float16

    xT_d = nc.dram_tensor("xT", [T, C, NCOL], f16, kind="ExternalInput")
    w_d = nc.dram_tensor("w", [128, 18 * 128], f16, kind="ExternalInput")
    mask_d = nc.dram_tensor("mask", [F, NCOL], f16, kind="ExternalInput")
    bif_d = nc.dram_tensor("b_if", [128, 1], f32, kind="ExternalInput")
    bg_d = nc.dram_tensor("bg", [F, 1], f32, kind="ExternalInput")
    bo_d = nc.dram_tensor("b_o", [128, 1], f32, kind="ExternalInput")
    bns_d = nc.dram_tensor("bns", [128, 1], f32, kind="ExternalInput")
    bnb_d = nc.dram_tensor("bnb", [128, 1], f32, kind="ExternalInput")
    sel_d = nc.dram_tensor("sel", [F, 8], f32, kind="ExternalInput")
    out_d = nc.dram_tensor("out", [T, F, 16 * WP], f16, kind="ExternalOutput")

    Relu = mybir.ActivationFunctionType.Relu
    Tanh = mybir.ActivationFunctionType.Tanh
    MULT = mybir.AluOpType.mult
    ADD = mybir.AluOpType.add

    with tile.TileContext(nc, trace_sim=TRACE_SIM) as tc:
        with (
            tc.tile_pool(name="const", bufs=1) as cpool,
            tc.tile_pool(name="state", bufs=1) as spool,
            tc.tile_pool(name="work", bufs=4) as wpool,
            tc.tile_pool(name="ostage", bufs=2) as opool,
            tc.tile_pool(name="agx", bufs=2) as agpool,
            tc.tile_pool(name="dram", bufs=2, space="DRAM") as dpool,
            tc.psum_pool(name="ps", bufs=4) as pspool,
        ):
            w_sb = cpool.tile([128, 18 * 128], f16)
            mask_sb = cpool.tile([F, NCOL], f16)
            bif_sb = cpool.tile([128, 1], f32)
            bg_sb = cpool.tile([F, 1], f32)
            bo_sb = cpool.tile([128, 1], f32)
            bns_sb = cpool.tile([128, 1], f32)
            bnb_sb = cpool.tile([128, 1], f32)
            sel_sb = cpool.tile([F, 8], f32)
            # x on the SP queue (feed PE first); weights + small consts on
            # the ACT queue so they don't head-of-line-block the x DMAs
            # warm the ACT function table first: its engine time overlaps
            # the const DMAs' sequencer work on the same queue
            warm = cpool.tile([64, 1], f32)
            nc.vector.memset(warm[:], 0.0)
            nc.scalar.activation(warm[:], warm[:],
                                 mybir.ActivationFunctionType.Tanh)
            # ACT queue: only what the first gate chain needs, w first
            nc.scalar.dma_start(w_sb[:, 0:1152], w_d[:, 0:1152])
            nc.scalar.dma_start(w_sb[:, 1152:2304], w_d[:, 1152:2304])
            nc.scalar.dma_start(bif_sb[:], bif_d[:])
            nc.scalar.dma_start(mask_sb[:], mask_d[:])
            # Pool queue is idle early (BNs start late): other consts here
            nc.gpsimd.dma_start(bg_sb[:], bg_d[:])
            nc.gpsimd.dma_start(bo_sb[:], bo_d[:])

            # 4 rotating buffers: step t reads x_t+h_{t-1} from zin[(t-1)%4]
            # and writes h_t into zin[t%4]. 4 (not 2) so a step's x DMA never
            # lands in a buffer whose x a not-yet-emitted chunk still reads.
            zin = [spool.tile([128, NCOL], f16, name=f"zin{i}", tag=f"zin{i}")
                   for i in range(4)]
            c_sb = spool.tile([F, NCOL], f32, tag="cstate")
            # h-halves and x guard rows must start zero (NaN garbage would
            # survive the h mask multiply via guard-col taps); x interior is
            # fully overwritten by the per-step DMA.
            nc.gpsimd.memset(c_sb[:], 0.0)   # first: feeds t=1's f*c early
            for i in range(4):
                eng = nc.vector if i % 2 == 0 else nc.gpsimd
                eng.memset(zin[i][64:128, :], 0.0)
                eng.memset(zin[i][0:64, 0:WP], 0.0)
                eng.memset(zin[i][0:64, NCOL - WP:NCOL], 0.0)
            # BN/select consts after the state memsets (needed later)
            nc.gpsimd.dma_start(bns_sb[:], bns_d[:])
            nc.gpsimd.dma_start(bnb_sb[:], bnb_d[:])
            nc.gpsimd.dma_start(sel_sb[:], sel_d[:])

            def emit_chunk_head(t, p0, n):
                cur = zin[(t - 1) % 4]
                ps_if = pspool.tile([128, n], f32, tag="psif")
                ps_go = pspool.tile([128, n], f32, tag="psgo")
                for k, (dy, dx) in enumerate(TAPS):
                    off = p0 + dy * WP + dx
                    nc.tensor.matmul(
                        ps_if[:], w_sb[:, k * 128:(k + 1) * 128],
                        cur[:, off:off + n], start=(k == 0), stop=(k == 8))
                for k, (dy, dx) in enumerate(TAPS):
                    off = p0 + dy * WP + dx
                    nc.tensor.matmul(
                        ps_go[:], w_sb[:, 1152 + k * 128:1152 + (k + 1) * 128],
                        cur[:, off:off + n], start=(k == 0), stop=(k == 8))

                sig_if = wpool.tile([128, n], f16, tag="sig_if")
                f_low = wpool.tile([F, n], f16, tag="f_low")
                tanh_g = wpool.tile([F, n], f16, tag="tanh_g")
                sig_o = wpool.tile([F, n], f16, tag="sig_o")
                t1 = wpool.tile([F, n], f16, tag="t1")
                t2 = wpool.tile([F, n], f32, tag="t2")

                # i|f fused: weights pre-scaled by 0.2 -> Relu(ps + 0.2b+0.5)
                nc.scalar.activation(sig_if[:], ps_if[:], Relu,
                                     bias=bif_sb[:, 0:1])
                # DVE TensorTensor needs equal input base partitions: min the
                # f-half down to partitions 0-63 while clipping it
                nc.vector.tensor_scalar_min(sig_if[0:64, :],
                                            sig_if[0:64, :], 1.0)
                nc.vector.tensor_scalar_min(f_low[:], sig_if[64:128, :], 1.0)
                nc.scalar.activation(tanh_g[:], ps_go[0:64, :], Tanh,
                                     bias=bg_sb[:, 0:1])
                nc.scalar.activation(sig_o[:], ps_go[64:128, :], Relu,
                                     bias=bo_sb[64:128, 0:1])
                # min(o,1) and the h edge-mask fused: mask is 1 in-image (so
                # min(o, 1)) and 0 outside (so o -> 0 -> h = 0)
                nc.vector.tensor_tensor(sig_o[:], sig_o[:],
                                        mask_sb[:, p0:p0 + n],
                                        mybir.AluOpType.min)
                nc.vector.tensor_mul(t1[:], sig_if[0:64, :], tanh_g[:])
                nc.vector.tensor_mul(t2[:], f_low[:], c_sb[:, p0:p0 + n])
                nc.vector.tensor_add(c_sb[:, p0:p0 + n], t1[:], t2[:])
                return sig_o

            def emit_chunk_tail(t, stage, p0, n, sig_o):
                # second pass: tanh(c) and h, emitted after every chunk's
                # gate work so a blocked tanh_c can't head-of-line-block the
                # next chunk's ready activations in the ACT FIFO
                nxt = zin[t % 4]
                tanh_c = wpool.tile([F, n], f16, tag="tanh_c")
                nc.scalar.activation(tanh_c[:], c_sb[:, p0:p0 + n], Tanh)
                nc.vector.tensor_mul(nxt[64:128, p0:p0 + n],
                                     sig_o[:], tanh_c[:])
                lo, hi = max(p0, OWN_LO), min(p0 + n, OWN_HI)
                if lo < hi:
                    # BN on Pool: off the critical recurrence path, so the
                    # collective blocking Pool only delays the output stage
                    nc.gpsimd.tensor_scalar(
                        stage[:, lo - OWN_LO:hi - OWN_LO],
                        nxt[64:128, lo:hi],
                        bns_sb[64:128, 0:1], bnb_sb[64:128, 0:1], MULT, ADD)

            def emit_step(t, stage, chunk_list):
                heads = [(p0, n, emit_chunk_head(t, p0, n))
                         for p0, n in chunk_list]
                for p0, n, sig_o in heads:
                    emit_chunk_tail(t, stage, p0, n, sig_o)

            def emit_exchange_send(t):
                # ---- exchange epoch: ship h,c boundary rows (4 each) ----
                nxt = zin[t % 4]
                cbf = agpool.tile([F, 2 * HB], f16, tag="cbf")
                nc.vector.tensor_scalar_mul(
                    cbf[:, 0:HB], c_sb[:, OWN_LO:OWN_LO + HB], 1.0)
                nc.vector.tensor_scalar_mul(
                    cbf[:, HB:2 * HB], c_sb[:, OWN_HI - HB:OWN_HI], 1.0)
                agin = dpool.tile([F, 4 * HB], f16, tag="agin")
                agout = dpool.tile([4 * F, 4 * HB], f16, tag="agout")
                nc.sync.dma_start(agin[:, 0:HB],
                                  nxt[64:128, OWN_LO:OWN_LO + HB])
                nc.sync.dma_start(agin[:, HB:2 * HB],
                                  nxt[64:128, OWN_HI - HB:OWN_HI])
                nc.sync.dma_start(agin[:, 2 * HB:4 * HB], cbf[:])
                nc.gpsimd.collective_compute(
                    "AllGather", mybir.AluOpType.bypass,
                    ins=[agin.opt()], outs=[agout.opt()],
                    replica_groups=[[0, 1, 2, 3], [4, 5, 6, 7]],
                )
                return agout

            def _select(src, dst_ap, scol, tiles, eng):
                # dst = sum_j src[:, j*HB : +HB] * sel[:, scol+j]
                for j in range(4):
                    eng.tensor_scalar_mul(
                        tiles[j][:], src[:, j * HB:(j + 1) * HB],
                        sel_sb[:, scol + j:scol + j + 1])
                eng.tensor_add(tiles[0][:], tiles[0][:], tiles[1][:])
                eng.tensor_add(tiles[2][:], tiles[2][:], tiles[3][:])
                eng.tensor_add(dst_ap, tiles[0][:], tiles[2][:])

            def emit_exchange_recv_pre(t, agout):
                # Emitted BEFORE the next block's phase A. All 4 compact
                # DMAs go on SP (it stalls on the AG, which only delays the
                # deferred out DMAs queued behind), critical halves (_bot,
                # feeding the up-halo that gates B0-L) first. The c selects
                # go FIRST on the Pool queue: Pool is blocked by the in-
                # flight collective anyway, and this keeps them ahead of
                # the phase-A BN backlog that would otherwise gate B0.
                agsb = {}
                for key, col0 in (("hb", HB), ("cb", 3 * HB),
                                  ("ht", 0), ("ct", 2 * HB)):
                    tile_ = agpool.tile([F, 4 * HB], f16, name=f"agsb_{key}",
                                        tag=f"agsb_{key}")
                    nc.sync.dma_start(
                        tile_[:].rearrange("p (j c) -> p j c", j=4),
                        agout[:, col0:col0 + HB].rearrange(
                            "(j p) c -> p j c", j=4))
                    agsb[key] = tile_
                csel = [agpool.tile([F, HB], f32, name=f"csel{i}",
                                    tag=f"csel{i}") for i in range(4)]
                _select(agsb["cb"], c_sb[:, WP:WP + HB], 0, csel, nc.gpsimd)
                _select(agsb["ct"], c_sb[:, OWN_HI:OWN_HI + HB], 4, csel,
                        nc.gpsimd)
                return agsb

            def emit_exchange_recv_post(t, agsb):
                # h selects on DVE, emitted after phase A so the DVE gate
                # chains are dispatched first
                nxt = zin[t % 4]
                tsel = [agpool.tile([F, HB], f16, name=f"tsel{i}",
                                    tag=f"tsel{i}") for i in range(4)]
                _select(agsb["hb"], nxt[64:128, WP:WP + HB], 0, tsel,
                        nc.vector)
                _select(agsb["ht"], nxt[64:128, OWN_HI:OWN_HI + HB], 4, tsel,
                        nc.vector)

            # Nested halo-independent interiors: I_j needs only I_{j-1}'s h
            # (67-col tap margin), so all interiors of a block are runnable
            # while the preceding epoch's AllGather is still in flight.
            INT = [(5 * WP + 66 * (j + 1), 21 * WP - 66 * (j + 1))
                   for j in range(4)]

            pending = None           # (epoch step, agout) awaiting receive
            deferred_outs = []       # (step, stage tile) from previous block
            for b in range(4):
                stages = {}
                for j in range(4):
                    t = 4 * b + j + 1
                    cur = zin[(t - 1) % 4]
                    # full interior rows every step: keeps every buffer's
                    # x-half fresh (no stale-x reads from 4 steps ago)
                    if t == 1:
                        # first chunk's tap range first: unblocks MM #1 early
                        nc.sync.dma_start(cur[0:64, 5 * WP:943],
                                          xT_d[t - 1, :, 5 * WP:943])
                        nc.sync.dma_start(cur[0:64, WP:5 * WP],
                                          xT_d[t - 1, :, WP:5 * WP])
                        nc.sync.dma_start(cur[0:64, 943:NCOL - WP],
                                          xT_d[t - 1, :, 943:NCOL - WP])
                    else:
                        nc.sync.dma_start(cur[0:64, WP:NCOL - WP],
                                          xT_d[t - 1, :, WP:NCOL - WP])
                    stages[t] = opool.tile([F, 16 * WP], f16, name=f"stage{t}",
                                           tag=f"stage{j}")
                if pending is not None:
                    recv_tiles = emit_exchange_recv_pre(*pending)
                # phase A: interiors (halo-independent), step order — these
                # overlap the in-flight AllGather from the previous block.
                # Split nested (67-col stagger) so step j+1's first chunk
                # depends only on step j's first chunk's pointwise chain.
                # 3 cones per interior: shrinking left cone + two
                # constant-width sliding cones; cone k of step j+1 depends
                # only on cones <= k of step j, so each transition chain is
                # covered by the later cones' matmuls
                for j in range(4):
                    t = 4 * b + j + 1
                    i0, i1 = INT[j]
                    q1, q2 = 877 - 66 * j, 1088 - 66 * j
                    emit_step(t, stages[t],
                              [(i0, q1 - i0), (q1, q2 - q1), (q2, i1 - q2)])
                if pending is not None:
                    emit_exchange_recv_post(pending[0], recv_tiles)
                    pending = None
                # previous block's output DMAs: deferred to here so their
                # BN deps are long done (they'd stall whichever queue they
                # sat in otherwise); SP is past its x DMAs by now
                for t_prev, stg in deferred_outs:
                    nc.sync.dma_start(out_d[t_prev - 1], stg[:])
                deferred_outs = []
                # phase B: boundary chunks, step order; epoch step's boundary
                # feeds the exchange
                for j in range(4):
                    t = 4 * b + j + 1
                    s = 3 - j
                    i0, i1 = INT[j]
                    c0, c1 = (5 - s) * WP, (21 + s) * WP
                    if t == T:
                        # final step: halve the right chunk so the kernel's
                        # tail chain (gates -> BN -> out DMA) is shorter
                        m = (i1 + c1) // 2
                        emit_step(t, stages[t],
                                  [(c0, i0 - c0), (i1, m - i1), (m, c1 - m)])
                    else:
                        emit_step(t, stages[t],
                                  [(c0, i0 - c0), (i1, c1 - i1)])
                for j in range(4):
                    t = 4 * b + j + 1
                    if t == T:
                        # last block drains on the (idle-by-now) SP queue,
                        # t=16 split in halves so the first half ships while
                        # the last BN finishes
                        nc.sync.dma_start(out_d[t - 1, :, 0:12 * WP],
                                          stages[t][:, 0:12 * WP])
                        # second half on the (end-idle) ACT queue: parallel
                        # with the first, not serialized behind it on SP
                        nc.scalar.dma_start(out_d[t - 1, :, 12 * WP:16 * WP],
                                            stages[t][:, 12 * WP:16 * WP])
                    elif b == 3:
                        nc.sync.dma_start(out_d[t - 1], stages[t][:])
                    else:
                        deferred_outs.append((t, stages[t]))
                if b < 3:
                    te = 4 * b + 4
                    pending = (te, emit_exchange_send(te))

        global _LAST_TC
        _LAST_TC = tc
    _split_multi_waits(nc)
    return nc


def _prep_inputs(x, Wx, Wh, b, gamma, beta, moving_mean, moving_var):
    x = np.asarray(x, F32)
    Wx = np.asarray(Wx, F32)
    Wh = np.asarray(Wh, F32)
    b = np.asarray(b, F32)
    # gate order along 4F: [i | f | g | o]; half1 = [i|f], half2 = [g|o].
    # Pre-scale i/f/o columns by 0.2 (hard_sigmoid slope).
    wstack = np.zeros((128, 18 * 128), F32)
    for k, (dy, dx) in enumerate(TAPS):
        ky, kx = dy + 1, dx + 1
        wstack[0:64, k * 128:(k + 1) * 128] = Wx[ky, kx, :, 0:128] * 0.2
        wstack[64:128, k * 128:(k + 1) * 128] = Wh[ky, kx, :, 0:128] * 0.2
        h2 = np.concatenate([Wx[ky, kx, :, 128:192],
                             Wx[ky, kx, :, 192:256] * 0.2], axis=1)
        wstack[0:64, 1152 + k * 128:1152 + (k + 1) * 128] = h2
        h2h = np.concatenate([Wh[ky, kx, :, 128:192],
                              Wh[ky, kx, :, 192:256] * 0.2], axis=1)
        wstack[64:128, 1152 + k * 128:1152 + (k + 1) * 128] = h2h
    wstack = wstack.astype(F16)

    b_if = (0.2 * b[0:128] + 0.5).reshape(128, 1).astype(F32)
    bg = b[128:192].reshape(64, 1).astype(F32)
    bo1 = 0.2 * b[192:256] + 0.5
    b_o = np.concatenate([bo1, bo1]).reshape(128, 1).astype(F32)
    inv = (np.asarray(gamma, F32) /
           np.sqrt(np.asarray(moving_var, F32) + 1e-3))
    bnb1 = (np.asarray(beta, F32) - np.asarray(moving_mean, F32) * inv)
    # duplicated into both partition halves: BN reads h at partitions 64-127
    bns = np.concatenate([inv, inv]).reshape(128, 1).astype(F32)
    bnb = np.concatenate([bnb1, bnb1]).reshape(128, 1).astype(F32)

    in_maps = []
    for core in range(8):
        bidx, sl = core // 4, core % 4
        r0 = 16 * sl
        glo, ghi = max(0, r0 - 5), min(64, r0 + 21)
        i0 = glo - (r0 - 5)
        xpad = np.zeros((T, NR, WP, C), F32)
        xpad[:, i0:i0 + (ghi - glo), 1:65, :] = x[bidx, :, glo:ghi, :, :]
        xT = np.ascontiguousarray(
            xpad.transpose(0, 3, 1, 2).reshape(T, C, NCOL)).astype(F16)
        m = np.zeros((NR, WP), F32)
        for i in range(NR):
            if 0 <= (r0 - 5 + i) < 64:
                m[i, 1:65] = 1.0
        mask = np.broadcast_to(
            m.reshape(1, NCOL), (64, NCOL)).astype(F16).copy()
        # select masks: sel[:, 0:4] = up (choose group-rank sl-1),
        # sel[:, 4:8] = down (choose group-rank sl+1)
        sel = np.zeros((64, 8), F32)
        if sl > 0:
            sel[:, sl - 1] = 1.0
        if sl < 3:
            sel[:, 4 + sl + 1] = 1.0
        in_maps.append({
            "xT": xT, "w": wstack, "mask": mask, "b_if": b_if,
            "bg": bg, "b_o": b_o, "bns": bns, "bnb": bnb, "sel": sel,
        })
    return in_maps


def kernel(x, Wx, Wh, b, gamma, beta, moving_mean, moving_var):
    global _PROG
    if _PROG is None:
        _PROG = _build()
    in_maps = _prep_inputs(x, Wx, Wh, b, gamma, beta, moving_mean, moving_var)
    res = run_bass_kernel_spmd(_PROG, in_maps, core_ids=list(range(8)))
    out = np.empty((2, T, 64, W, F), F32)
    for core in range(8):
        bidx, sl = core // 4, core % 4
        oc = res.results[core]["out"].astype(F32).reshape(
            T, F, 16, WP)[:, :, :, 1:65]
        out[bidx, :, 16 * sl:16 * sl + 16] = oc.transpose(0, 2, 3, 1)
    return out


# revision 72
# speedup vs baseline: 1.5833x; 1.0090x over previous
"""ConvLSTM2D (Keras gate order, hard_sigmoid) + inference BatchNorm on 8
Trainium2 NeuronCores.

Sharding: batch (2) x H-slabs (4) -> 8 cores. The T=16 recurrence is split
into 4 blocks of 4 steps. Within a block each core computes a shrinking halo
(depth 4); at block boundaries (t=4,8,12) cores exchange 4 boundary rows of
BOTH states (h and c) with their slab neighbors via one AllGather per epoch
(replica groups = the two 4-slab groups). This cuts redundant conv work from
1.94x (17-deep shrinking halo) to 1.19x while keeping the program uniform
SPMD: out-of-image edges are data (zero-padded x, h-mask, zero select masks).

Layout: channels-on-partitions. zin (128 x 1716 fp16) holds x_t on
partitions 0-63 and h_{t-1} on 64-127; 26 rows of 66 cols (64 + guard col
each side); a 3x3 conv tap (dy,dx) is the col offset dy*66+dx. One matmul
contracts x AND h channels at once (lhsT = [Wx_tap; Wh_tap]) so
z = conv(x,Wx)+conv(h,Wh) is 9 taps x 2 gate-halves of accumulating matmuls
per chunk (full 128x128 PE). Gate-i/f/o weight columns are pre-scaled by 0.2
so hard_sigmoid is Relu(psum + (0.2b+0.5)) then min(.,1). i|f activations are
fused into single 128-partition ops; pointwise math runs in fp16 on DVE
(2x rate) with c kept in fp32; BN is one DVE tensor_scalar (scale+bias) into
an fp16 output.
"""
import math
import numpy as np

import concourse.bass as bass
import concourse.mybir as mybir
import concourse.tile as tile
from concourse.bass_utils import run_bass_kernel_spmd

F16 = np.float16
F32 = np.float32

T, F, C, W = 16, 64, 64, 64
L = 4              # block length (steps between exchanges)
HALO = 4           # halo depth = L
NR = 16 + 2 * HALO + 2   # 26 buffer rows: [r0-5, r1+5)
WP = W + 1         # 65: one shared zero guard col between rows
# (row r's right-pad tap lands on row r+1's left guard, both always 0)
NCOL = NR * WP     # 1716
OWN_LO, OWN_HI = 5 * WP, 21 * WP     # own 16 rows: buffer rows [5, 21)
HB = HALO * WP     # 264: one halo bundle (4 rows)
TAPS = [(dy, dx) for dy in (-1, 0, 1) for dx in (-1, 0, 1)]

TRACE_SIM = False
_PROG = None
_LAST_TC = None

# ---------------------------------------------------------------------------
# Workaround: this walrus build accepts at most ONE sync wait per
# instruction; Tile attaches several. Hoist extras onto same-engine NOPs
# inserted right before the instruction (per-engine order preserved).
_MAX_WAITS = 1


def _split_multi_waits(nc):
    for fn in nc.m.functions:
        for bb in fn.blocks:
            lst = bb.instructions
            out, changed = [], False
            for ins in lst:
                si = ins.sync_info
                if si is not None and len(si.on_wait) > _MAX_WAITS:
                    waits = list(si.on_wait)
                    extra, keep = waits[:-_MAX_WAITS], waits[-_MAX_WAITS:]
                    for j, w in enumerate(extra):
                        nop = mybir.InstNoOp(
                            name=f"{ins.name}.sw{j}", ins=[], outs=[],
                            text_hint="split_wait", bass_nofuse=True)
                        nop.engine = ins.engine
                        nop.sync_info = mybir.SyncInfo(on_wait=[w], on_update=[])
                        out.append(nop)
                    ins.sync_info = mybir.SyncInfo(
                        on_wait=keep, on_update=list(si.on_update))
                    changed = True
                out.append(ins)
            if changed:
                try:
                    bb.instructions = out
                except Exception:
                    lst.clear()
                    lst.extend(out)


def _chunks(c0, c1, maxn=512):
    Ln = c1 - c0
    n = max(1, math.ceil(Ln / maxn))
    base, rem = divmod(Ln, n)
    sizes = [base + (1 if i < rem else 0) for i in range(n)]
    out, p = [], c0
    for s in sizes:
        out.append((p, s))
        p += s
    return out


def _build():
    nc = bass.Bass(target_bir_lowering=False)
    f32, f16 = mybir.dt.float32, mybir.dt.float16

    xT_d = nc.dram_tensor("xT", [T, C, NCOL], f16, kind="ExternalInput")
    w_d = nc.dram_tensor("w", [128, 18 * 128], f16, kind="ExternalInput")
    mask_d = nc.dram_tensor("mask", [F, NCOL], f16, kind="ExternalInput")
    bif_d = nc.dram_tensor("b_if", [128, 1], f32, kind="ExternalInput")
    bg_d = nc.dram_tensor("bg", [F, 1], f32, kind="ExternalInput")
    bo_d = nc.dram_tensor("b_o", [128, 1], f32, kind="ExternalInput")
    bns_d = nc.dram_tensor("bns", [128, 1], f32, kind="ExternalInput")
    bnb_d = nc.dram_tensor("bnb", [128, 1], f32, kind="ExternalInput")
    sel_d = nc.dram_tensor("sel", [F, 8], f32, kind="ExternalInput")
    out_d = nc.dram_tensor("out", [T, F, 16 * WP], f16, kind="ExternalOutput")

    Relu = mybir.ActivationFunctionType.Relu
    Tanh = mybir.ActivationFunctionType.Tanh
    MULT = mybir.AluOpType.mult
    ADD = mybir.AluOpType.add

    with tile.TileContext(nc, trace_sim=TRACE_SIM) as tc:
        with (
            tc.tile_pool(name="const", bufs=1) as cpool,
            tc.tile_pool(name="state", bufs=1) as spool,
            tc.tile_pool(name="work", bufs=6) as wpool,
            tc.tile_pool(name="ostage", bufs=3) as opool,
            tc.tile_pool(name="agx", bufs=2) as agpool,
            tc.tile_pool(name="dram", bufs=2, space="DRAM") as dpool,
            tc.psum_pool(name="ps", bufs=4) as pspool,
        ):
            w_sb = cpool.tile([128, 18 * 128], f16)
            mask_sb = cpool.tile([F, NCOL], f16)
            bif_sb = cpool.tile([128, 1], f32)
            bg_sb = cpool.tile([F, 1], f32)
            bo_sb = cpool.tile([128, 1], f32)
            bns_sb = cpool.tile([128, 1], f32)
            bnb_sb = cpool.tile([128, 1], f32)
            sel_sb = cpool.tile([F, 8], f32)
            # x on the SP queue (feed PE first); weights + small consts on
            # the ACT queue so they don't head-of-line-block the x DMAs
            # warm the ACT function table first: its engine time overlaps
            # the const DMAs' sequencer work on the same queue
            warm = cpool.tile([64, 1], f32)
            nc.vector.memset(warm[:], 0.0)
            nc.scalar.activation(warm[:], warm[:],
                                 mybir.ActivationFunctionType.Tanh)
            # ACT queue: only what the first gate chain needs, w first
            nc.scalar.dma_start(w_sb[:, 0:1152], w_d[:, 0:1152])
            nc.scalar.dma_start(w_sb[:, 1152:2304], w_d[:, 1152:2304])
            nc.scalar.dma_start(bif_sb[:], bif_d[:])
            nc.scalar.dma_start(mask_sb[:], mask_d[:])
            # Pool queue is idle early (BNs start late): other consts here
            nc.gpsimd.dma_start(bg_sb[:], bg_d[:])
            nc.gpsimd.dma_start(bo_sb[:], bo_d[:])

            # 4 rotating buffers: step t reads x_t+h_{t-1} from zin[(t-1)%4]
            # and writes h_t into zin[t%4]. 4 (not 2) so a step's x DMA never
            # lands in a buffer whose x a not-yet-emitted chunk still reads.
            zin = [spool.tile([128, NCOL], f16, name=f"zin{i}", tag=f"zin{i}")
                   for i in range(4)]
            c_sb = spool.tile([F, NCOL], f32, tag="cstate")
            # h-halves and x guard rows must start zero (NaN garbage would
            # survive the h mask multiply via guard-col taps); x interior is
            # fully overwritten by the per-step DMA.
            nc.gpsimd.memset(c_sb[:], 0.0)   # first: feeds t=1's f*c early
            for i in range(4):
                eng = nc.vector if i % 2 == 0 else nc.gpsimd
                eng.memset(zin[i][64:128, :], 0.0)
                eng.memset(zin[i][0:64, 0:WP], 0.0)
                eng.memset(zin[i][0:64, NCOL - WP:NCOL], 0.0)
            # BN/select consts after the state memsets (needed later)
            nc.gpsimd.dma_start(bns_sb[:], bns_d[:])
            nc.gpsimd.dma_start(bnb_sb[:], bnb_d[:])
            nc.gpsimd.dma_start(sel_sb[:], sel_d[:])

            def emit_chunk_head(t, p0, n):
                cur = zin[(t - 1) % 4]
                ps_if = pspool.tile([128, n], f32, tag="psif")
                ps_go = pspool.tile([128, n], f32, tag="psgo")
                for k, (dy, dx) in enumerate(TAPS):
                    off = p0 + dy * WP + dx
                    nc.tensor.matmul(
                        ps_if[:], w_sb[:, k * 128:(k + 1) * 128],
                        cur[:, off:off + n], start=(k == 0), stop=(k == 8))
                for k, (dy, dx) in enumerate(TAPS):
                    off = p0 + dy * WP + dx
                    nc.tensor.matmul(
                        ps_go[:], w_sb[:, 1152 + k * 128:1152 + (k + 1) * 128],
                        cur[:, off:off + n], start=(k == 0), stop=(k == 8))

                sig_if = wpool.tile([128, n], f16, tag="sig_if")
                f_low = wpool.tile([F, n], f16, tag="f_low")
                tanh_g = wpool.tile([F, n], f16, tag="tanh_g")
                sig_o = wpool.tile([F, n], f16, tag="sig_o")
                t1 = wpool.tile([F, n], f16, tag="t1")
                t2 = wpool.tile([F, n], f32, tag="t2")

                # i|f fused: weights pre-scaled by 0.2 -> Relu(ps + 0.2b+0.5)
                nc.scalar.activation(sig_if[:], ps_if[:], Relu,
                                     bias=bif_sb[:, 0:1])
                # DVE TensorTensor needs equal input base partitions: min the
                # f-half down to partitions 0-63 while clipping it
                nc.vector.tensor_scalar_min(sig_if[0:64, :],
                                            sig_if[0:64, :], 1.0)
                nc.vector.tensor_scalar_min(f_low[:], sig_if[64:128, :], 1.0)
                nc.scalar.activation(tanh_g[:], ps_go[0:64, :], Tanh,
                                     bias=bg_sb[:, 0:1])
                nc.scalar.activation(sig_o[:], ps_go[64:128, :], Relu,
                                     bias=bo_sb[64:128, 0:1])
                # min(o,1) and the h edge-mask fused: mask is 1 in-image (so
                # min(o, 1)) and 0 outside (so o -> 0 -> h = 0)
                nc.vector.tensor_tensor(sig_o[:], sig_o[:],
                                        mask_sb[:, p0:p0 + n],
                                        mybir.AluOpType.min)
                nc.vector.tensor_mul(t1[:], sig_if[0:64, :], tanh_g[:])
                nc.vector.tensor_mul(t2[:], f_low[:], c_sb[:, p0:p0 + n])
                nc.vector.tensor_add(c_sb[:, p0:p0 + n], t1[:], t2[:])
                return sig_o

            def emit_chunk_tail(t, stage, p0, n, sig_o):
                # second pass: tanh(c) and h, emitted after every chunk's
                # gate work so a blocked tanh_c can't head-of-line-block the
                # next chunk's ready activations in the ACT FIFO
                nxt = zin[t % 4]
                tanh_c = wpool.tile([F, n], f16, tag="tanh_c")
                nc.scalar.activation(tanh_c[:], c_sb[:, p0:p0 + n], Tanh)
                nc.vector.tensor_mul(nxt[64:128, p0:p0 + n],
                                     sig_o[:], tanh_c[:])
                lo, hi = max(p0, OWN_LO), min(p0 + n, OWN_HI)
                if lo < hi:
                    # BN on Pool: off the critical recurrence path, so the
                    # collective blocking Pool only delays the output stage
                    nc.gpsimd.tensor_scalar(
                        stage[:, lo - OWN_LO:hi - OWN_LO],
                        nxt[64:128, lo:hi],
                        bns_sb[64:128, 0:1], bnb_sb[64:128, 0:1], MULT, ADD)

            def emit_step(t, stage, chunk_list):
                heads = [(p0, n, emit_chunk_head(t, p0, n))
                         for p0, n in chunk_list]
                for p0, n, sig_o in heads:
                    emit_chunk_tail(t, stage, p0, n, sig_o)

            def emit_exchange_send(t):
                # ---- exchange epoch: ship h,c boundary rows (4 each) ----
                nxt = zin[t % 4]
                cbf = agpool.tile([F, 2 * HB], f16, tag="cbf")
                nc.vector.tensor_scalar_mul(
                    cbf[:, 0:HB], c_sb[:, OWN_LO:OWN_LO + HB], 1.0)
                nc.vector.tensor_scalar_mul(
                    cbf[:, HB:2 * HB], c_sb[:, OWN_HI - HB:OWN_HI], 1.0)
                agin = dpool.tile([F, 4 * HB], f16, tag="agin")
                agout = dpool.tile([4 * F, 4 * HB], f16, tag="agout")
                nc.sync.dma_start(agin[:, 0:HB],
                                  nxt[64:128, OWN_LO:OWN_LO + HB])
                nc.sync.dma_start(agin[:, HB:2 * HB],
                                  nxt[64:128, OWN_HI - HB:OWN_HI])
                nc.sync.dma_start(agin[:, 2 * HB:4 * HB], cbf[:])
                nc.gpsimd.collective_compute(
                    "AllGather", mybir.AluOpType.bypass,
                    ins=[agin.opt()], outs=[agout.opt()],
                    replica_groups=[[0, 1, 2, 3], [4, 5, 6, 7]],
                )
                return agout

            def _select(src, dst_ap, scol, tiles, eng):
                # dst = sum_j src[:, j*HB : +HB] * sel[:, scol+j].
                # Only 3 group-ranks can ever be the sender: rank 3 never
                # feeds an up-halo (me-1 <= 2), rank 0 never a down-halo
                # (me+1 >= 1) - so the tree is 3 muls + 2 adds, not 4 + 3.
                js = (0, 1, 2) if scol == 0 else (1, 2, 3)
                for i, j in enumerate(js):
                    eng.tensor_scalar_mul(
                        tiles[i][:], src[:, j * HB:(j + 1) * HB],
                        sel_sb[:, scol + j:scol + j + 1])
                eng.tensor_add(tiles[0][:], tiles[0][:], tiles[1][:])
                eng.tensor_add(dst_ap, tiles[0][:], tiles[2][:])

            def emit_exchange_recv_pre(t, agout):
                # Emitted BEFORE the next block's phase A. All 4 compact
                # DMAs go on SP (it stalls on the AG, which only delays the
                # deferred out DMAs queued behind), critical halves (_bot,
                # feeding the up-halo that gates B0-L) first. The c selects
                # go FIRST on the Pool queue: Pool is blocked by the in-
                # flight collective anyway, and this keeps them ahead of
                # the phase-A BN backlog that would otherwise gate B0.
                agsb = {}
                for key, col0 in (("hb", HB), ("cb", 3 * HB),
                                  ("ht", 0), ("ct", 2 * HB)):
                    tile_ = agpool.tile([F, 4 * HB], f16, name=f"agsb_{key}",
                                        tag=f"agsb_{key}")
                    nc.sync.dma_start(
                        tile_[:].rearrange("p (j c) -> p j c", j=4),
                        agout[:, col0:col0 + HB].rearrange(
                            "(j p) c -> p j c", j=4))
                    agsb[key] = tile_
                csel = [agpool.tile([F, HB], f32, name=f"csel{i}",
                                    tag=f"csel{i}") for i in range(3)]
                _select(agsb["cb"], c_sb[:, WP:WP + HB], 0, csel, nc.gpsimd)
                _select(agsb["ct"], c_sb[:, OWN_HI:OWN_HI + HB], 4, csel,
                        nc.gpsimd)
                return agsb

            def emit_exchange_recv_post(t, agsb):
                # h selects on DVE, emitted after phase A so the DVE gate
                # chains are dispatched first
                nxt = zin[t % 4]
                tsel = [agpool.tile([F, HB], f16, name=f"tsel{i}",
                                    tag=f"tsel{i}") for i in range(3)]
                _select(agsb["hb"], nxt[64:128, WP:WP + HB], 0, tsel,
                        nc.vector)
                _select(agsb["ht"], nxt[64:128, OWN_HI:OWN_HI + HB], 4, tsel,
                        nc.vector)

            # Nested halo-independent interiors: I_j needs only I_{j-1}'s h
            # (67-col tap margin), so all interiors of a block are runnable
            # while the preceding epoch's AllGather is still in flight.
            INT = [(5 * WP + 66 * (j + 1), 21 * WP - 66 * (j + 1))
                   for j in range(4)]

            pending = None           # (epoch step, agout) awaiting receive
            deferred_outs = []       # (step, stage tile) from previous block
            for b in range(4):
                stages = {}
                for j in range(4):
                    t = 4 * b + j + 1
                    cur = zin[(t - 1) % 4]
                    # full interior rows every step: keeps every buffer's
                    # x-half fresh (no stale-x reads from 4 steps ago)
                    if t == 1:
                        # first chunk's tap range first: unblocks MM #1 early
                        nc.sync.dma_start(cur[0:64, 5 * WP:943],
                                          xT_d[t - 1, :, 5 * WP:943])
                        nc.sync.dma_start(cur[0:64, WP:5 * WP],
                                          xT_d[t - 1, :, WP:5 * WP])
                        nc.sync.dma_start(cur[0:64, 943:NCOL - WP],
                                          xT_d[t - 1, :, 943:NCOL - WP])
                    else:
                        nc.sync.dma_start(cur[0:64, WP:NCOL - WP],
                                          xT_d[t - 1, :, WP:NCOL - WP])
                    stages[t] = opool.tile([F, 16 * WP], f16, name=f"stage{t}",
                                           tag=f"stage{j}")
                if pending is not None:
                    recv_tiles = emit_exchange_recv_pre(*pending)
                # phase A: interiors (halo-independent), step order — these
                # overlap the in-flight AllGather from the previous block.
                # Split nested (67-col stagger) so step j+1's first chunk
                # depends only on step j's first chunk's pointwise chain.
                # 3 cones per interior: shrinking left cone + two
                # constant-width sliding cones; cone k of step j+1 depends
                # only on cones <= k of step j, so each transition chain is
                # covered by the later cones' matmuls
                for j in range(4):
                    t = 4 * b + j + 1
                    i0, i1 = INT[j]
                    q1, q2 = 877 - 66 * j, 1088 - 66 * j
                    emit_step(t, stages[t],
                              [(i0, q1 - i0), (q1, q2 - q1), (q2, i1 - q2)])
                if pending is not None:
                    emit_exchange_recv_post(pending[0], recv_tiles)
                    pending = None
                # previous block's output DMAs: deferred to here so their
                # BN deps are long done (they'd stall whichever queue they
                # sat in otherwise); SP is past its x DMAs by now
                for t_prev, stg in deferred_outs:
                    nc.sync.dma_start(out_d[t_prev - 1], stg[:])
                deferred_outs = []
                # phase B: boundary chunks, step order; epoch step's boundary
                # feeds the exchange
                for j in range(4):
                    t = 4 * b + j + 1
                    s = 3 - j
                    i0, i1 = INT[j]
                    c0, c1 = (5 - s) * WP, (21 + s) * WP
                    if t == T:
                        # final step: halve the right chunk so the kernel's
                        # tail chain (gates -> BN -> out DMA) is shorter
                        m = (i1 + c1) // 2
                        emit_step(t, stages[t],
                                  [(c0, i0 - c0), (i1, m - i1), (m, c1 - m)])
                    else:
                        emit_step(t, stages[t],
                                  [(c0, i0 - c0), (i1, c1 - i1)])
                for j in range(4):
                    t = 4 * b + j + 1
                    if t == T:
                        # last block drains on the (idle-by-now) SP queue,
                        # t=16 split in halves so the first half ships while
                        # the last BN finishes
                        nc.sync.dma_start(out_d[t - 1, :, 0:12 * WP],
                                          stages[t][:, 0:12 * WP])
                        # second half on the (end-idle) ACT queue: parallel
                        # with the first, not serialized behind it on SP
                        nc.scalar.dma_start(out_d[t - 1, :, 12 * WP:16 * WP],
                                            stages[t][:, 12 * WP:16 * WP])
                    elif b == 3:
                        nc.sync.dma_start(out_d[t - 1], stages[t][:])
                    else:
                        deferred_outs.append((t, stages[t]))
                if b < 3:
                    te = 4 * b + 4
                    pending = (te, emit_exchange_send(te))

        global _LAST_TC
        _LAST_TC = tc
    _split_multi_waits(nc)
    return nc


def _prep_inputs(x, Wx, Wh, b, gamma, beta, moving_mean, moving_var):
    x = np.asarray(x, F32)
    Wx = np.asarray(Wx, F32)
    Wh = np.asarray(Wh, F32)
    b = np.asarray(b, F32)
    # gate order along 4F: [i | f | g | o]; half1 = [i|f], half2 = [g|o].
    # Pre-scale i/f/o columns by 0.2 (hard_sigmoid slope).
    wstack = np.zeros((128, 18 * 128), F32)
    for k, (dy, dx) in enumerate(TAPS):
        ky, kx = dy + 1, dx + 1
        wstack[0:64, k * 128:(k + 1) * 128] = Wx[ky, kx, :, 0:128] * 0.2
        wstack[64:128, k * 128:(k + 1) * 128] = Wh[ky, kx, :, 0:128] * 0.2
        h2 = np.concatenate([Wx[ky, kx, :, 128:192],
                             Wx[ky, kx, :, 192:256] * 0.2], axis=1)
        wstack[0:64, 1152 + k * 128:1152 + (k + 1) * 128] = h2
        h2h = np.concatenate([Wh[ky, kx, :, 128:192],
                              Wh[ky, kx, :, 192:256] * 0.2], axis=1)
        wstack[64:128, 1152 + k * 128:1152 + (k + 1) * 128] = h2h
    wstack = wstack.astype(F16)

    b_if = (0.2 * b[0:128] + 0.5).reshape(128, 1).astype(F32)
    bg = b[128:192].reshape(64, 1).astype(F32)
    bo1 = 0.2 * b[192:256] + 0.5
    b_o = np.concatenate([bo1, bo1]).reshape(128, 1).astype(F32)
    inv = (np.asarray(gamma, F32) /
           np.sqrt(np.asarray(moving_var, F32) + 1e-3))
    bnb1 = (np.asarray(beta, F32) - np.asarray(moving_mean, F32) * inv)
    # duplicated into both partition halves: BN reads h at partitions 64-127
    bns = np.concatenate([inv, inv]).reshape(128, 1).astype(F32)
    bnb = np.concatenate([bnb1, bnb1]).reshape(128, 1).astype(F32)

    in_maps = []
    for core in range(8):
        bidx, sl = core // 4, core % 4
        r0 = 16 * sl
        glo, ghi = max(0, r0 - 5), min(64, r0 + 21)
        i0 = glo - (r0 - 5)
        xpad = np.zeros((T, NR, WP, C), F32)
        xpad[:, i0:i0 + (ghi - glo), 1:65, :] = x[bidx, :, glo:ghi, :, :]
        xT = np.ascontiguousarray(
            xpad.transpose(0, 3, 1, 2).reshape(T, C, NCOL)).astype(F16)
        m = np.zeros((NR, WP), F32)
        for i in range(NR):
            if 0 <= (r0 - 5 + i) < 64:
                m[i, 1:65] = 1.0
        mask = np.broadcast_to(
            m.reshape(1, NCOL), (64, NCOL)).astype(F16).copy()
        # select masks: sel[:, 0:4] = up (choose group-rank sl-1),
        # sel[:, 4:8] = down (choose group-rank sl+1)
        sel = np.zeros((64, 8), F32)
        if sl > 0:
            sel[:, sl - 1] = 1.0
        if sl < 3:
            sel[:, 4 + sl + 1] = 1.0
        in_maps.append({
            "xT": xT, "w": wstack, "mask": mask, "b_if": b_if,
            "bg": bg, "b_o": b_o, "bns": bns, "bnb": bnb, "sel": sel,
        })
    return in_maps


def kernel(x, Wx, Wh, b, gamma, beta, moving_mean, moving_var):
    global _PROG
    if _PROG is None:
        _PROG = _build()
    in_maps = _prep_inputs(x, Wx, Wh, b, gamma, beta, moving_mean, moving_var)
    res = run_bass_kernel_spmd(_PROG, in_maps, core_ids=list(range(8)))
    out = np.empty((2, T, 64, W, F), F32)
    for core in range(8):
        bidx, sl = core // 4, core % 4
        oc = res.results[core]["out"].astype(F32).reshape(
            T, F, 16, WP)[:, :, :, 1:65]
        out[bidx, :, 16 * sl:16 * sl + 16] = oc.transpose(0, 2, 3, 1)
    return out
